# revision 51
# baseline (speedup 1.0000x reference)
"""LundNetTagger GNN on 8 Trainium2 NeuronCores (Bass/Tile).

Self-contained: kernel(**inputs) -> np.ndarray [1000, 2] float32.

Strategy: nodes are assigned to 100352 "slots" (8 cores x 98 windows x 128),
packed so each window receives <= 512 edges. Edges live on the core owning
their dst slot, in window-major order padded to 4x128-edge chunks per window.
Per-edge MLPs run in bf16 feature-major layout; EdgeConv cat[xi, xj-xi] is
folded into split weights WA = W[:C]-W[C:], WB = W[C:]. GraphNorm stats are
global AllReduces of per-core sums (conv1 layer-1 stats are computed exactly
on the host from the 10-dim message Gram; deeper layers use vector-engine
bn_stats on PSUM with a sentinel pad column for exact correction).
conv1 keeps h fully SBUF-resident (no z spills): layer 2 overwrites h in
place after its stats AllReduce, and layer 3 fuses into the scatter.
Mean-aggregation is a collision-free one-hot matmul scatter into PSUM per
window. Node tables are AllGathered in bf16 between convs in two chunk-major
halves (each half fires as soon as its windows are written, overlapping the
producing scatter); src-side gathers use per-chunk indirect DMA with
chunk-major global row indices.
"""
import numpy as np
import ml_dtypes

import concourse.bass as bass
import concourse.tile as tile
from concourse import bacc, mybir
from concourse.bass_utils import run_bass_kernel_spmd
from concourse import library_config

BF16 = mybir.dt.bfloat16
F32 = mybir.dt.float32
I16 = mybir.dt.int16
AOP = mybir.AluOpType
AFT = mybir.ActivationFunctionType
AX = mybir.AxisListType

N_NODES = 100000
N_EDGES = 400000
N_GRAPHS = 1000
NC = 8
WIN = 128
NWIN = 98
SPC = WIN * NWIN          # 12544
NSLOTS = SPC * NC         # 100352
QUAD = NSLOTS // 4        # 25088
B = 4                     # chunks per window
EPW = B * WIN             # 512
E_PAD = NWIN * EPW        # 50176
EPS = 1e-5

NW_BLK = 7
BLK = NW_BLK * EPW        # 3584
NBLK = NWIN // NW_BLK     # 14
NCHUNK = BLK // 128       # 28
NSEG = BLK // 512         # 7

# window-aligned AllGather chunk boundaries (local rows) and the scatter
# block index after which each chunk's windows are complete
AG_CHB = [0, 25 * WIN, 50 * WIN, 74 * WIN, SPC]   # 3200/3200/3072/3072 rows
AG_FIRE = [4, 8, 11]     # fire chunk k at top of block AG_FIRE[k]; last at end


_cache = {}


# ============================ host-side packing ============================

def _pack(edge_index, batch):
    src = np.asarray(edge_index[0], dtype=np.int64)
    dst = np.asarray(edge_index[1], dtype=np.int64)
    batch = np.asarray(batch, dtype=np.int64)
    cnt = np.bincount(dst, minlength=N_NODES)

    nvirt = NSLOTS - N_NODES
    cnt_all = np.concatenate([cnt, np.zeros(nvirt, dtype=cnt.dtype)])
    order = np.argsort(-cnt_all, kind="stable")
    GW = NWIN * NC
    rounds = NSLOTS // GW
    win_of_rank = np.empty(NSLOTS, dtype=np.int64)
    for r in range(rounds):
        seg = np.arange(GW) if r % 2 == 0 else np.arange(GW - 1, -1, -1)
        win_of_rank[r * GW:(r + 1) * GW] = seg
    win_of_node = np.empty(NSLOTS, dtype=np.int64)
    win_of_node[order] = win_of_rank
    wsum = np.bincount(win_of_node, weights=cnt_all.astype(np.float64),
                       minlength=GW).astype(np.int64)

    cap = EPW
    members_of = [list(np.where(win_of_node == w)[0]) for w in range(GW)]
    for _ in range(2000):
        over = np.where(wsum > cap)[0]
        if len(over) == 0:
            break
        w = int(over[0])
        # smallest-count >0 node in w
        mem = members_of[w]
        cs = [(int(cnt_all[n]), n) for n in mem if cnt_all[n] > 0]
        cs.sort()
        moved = False
        for c1, n in cs:
            # find target window with a smaller-count node to swap
            worder2 = np.argsort(wsum)
            for tw in worder2[:64]:
                tw = int(tw)
                if tw == w:
                    continue
                tmem = members_of[tw]
                best = None
                for m in tmem:
                    c2 = int(cnt_all[m])
                    if c2 < c1 and wsum[tw] + c1 - c2 <= cap:
                        if best is None or c2 < best[0]:
                            best = (c2, m)
                        if c2 == 0:
                            break
                if best is not None:
                    c2, m = best
                    members_of[tw].remove(m)
                    members_of[tw].append(n)
                    members_of[w].remove(n)
                    members_of[w].append(m)
                    win_of_node[n] = tw
                    win_of_node[m] = w
                    wsum[tw] += c1 - c2
                    wsum[w] -= c1 - c2
                    moved = True
                    break
            if moved:
                break
        if not moved:
            raise RuntimeError("packing fixup stuck")
    assert wsum.max() <= cap, f"window packing failed: max={wsum.max()}"

    worder = np.argsort(-wsum, kind="stable")
    core_load = np.zeros(NC, dtype=np.int64)
    core_nwin = np.zeros(NC, dtype=np.int64)
    core_of_win = np.empty(GW, dtype=np.int64)
    for w in worder:
        cands = np.where(core_nwin < NWIN)[0]
        c = cands[np.argmin(core_load[cands])]
        core_of_win[w] = c
        core_load[c] += wsum[w]
        core_nwin[c] += 1

    win_lists = [[] for _ in range(NC)]
    for w in range(GW):
        win_lists[core_of_win[w]].append(w)
    for c in range(NC):
        wl = win_lists[c]
        j = int(np.argmin(wsum[wl]))
        assert wsum[wl[j]] < cap, "no sentinel room"
        wl[j], wl[-1] = wl[-1], wl[j]

    slot_of_node = np.empty(NSLOTS, dtype=np.int64)
    for c in range(NC):
        for wi, w in enumerate(win_lists[c]):
            mem = np.sort(np.array(members_of[w], dtype=np.int64))
            assert len(mem) == WIN
            slot_of_node[mem] = c * SPC + wi * WIN + np.arange(WIN)
    node_of_slot = np.empty(NSLOTS, dtype=np.int64)
    node_of_slot[slot_of_node] = np.arange(NSLOTS)
    cnt_of_slot = cnt_all[node_of_slot]

    qzero = []
    for q in range(4):
        z = np.where(cnt_of_slot[q * QUAD:(q + 1) * QUAD] == 0)[0]
        assert len(z) > 0
        assert z[0] < 32768
        qzero.append(int(z[0]))  # local to quadrant
    czero = []
    for c in range(NC):
        z = np.where(cnt_of_slot[c * SPC:(c + 1) * SPC] == 0)[0]
        assert len(z) > 0
        czero.append(int(z[0]))  # local to core

    dslot = slot_of_node[dst]
    sslot = slot_of_node[src]
    ecore = dslot // SPC
    ewin = (dslot % SPC) // WIN
    key = ecore * (NWIN * WIN) + ewin * WIN + (dslot % WIN)
    eorder = np.argsort(key, kind="stable")
    dsl, ssl = dslot[eorder], sslot[eorder]
    ec, ew = ecore[eorder], ewin[eorder]

    cw = ec * NWIN + ew
    cw_cnt = np.bincount(cw, minlength=NC * NWIN)
    assert cw_cnt.max() <= EPW

    xi_idx = np.zeros((NC, E_PAD), dtype=np.int64)
    xj_idx = np.zeros((NC, E_PAD), dtype=np.int64)
    dstwin = np.full((NC, E_PAD), -1.0, dtype=np.float32)
    valid = np.zeros((NC, E_PAD), dtype=bool)

    ofs = (np.arange(NC * NWIN) % NWIN) * EPW
    start = np.concatenate([[0], np.cumsum(cw_cnt)[:-1]])
    within = np.arange(N_EDGES) - start[cw]
    pos = ofs[cw] + within
    xi_idx[ec, pos] = dsl % SPC
    xj_idx[ec, pos] = ssl
    dstwin[ec, pos] = (dsl % WIN).astype(np.float32)
    valid[ec, pos] = True
    for c in range(NC):
        xi_idx[c, ~valid[c]] = czero[c]
    pad_cnt = (~valid).sum(axis=1).astype(np.float32)
    assert np.all(~valid[:, -1]), "sentinel column must be padding"

    gzero = qzero[0]  # global slot with zero row
    xj_glob = np.where(valid, xj_idx, gzero).astype(np.int32)

    # Chunk-major AllGather table layout: local rows split into 4
    # window-aligned chunks; the full table stores [chunk][core][rows] so
    # each AG chunk output is a contiguous row block.
    sl_ = np.arange(NSLOTS)
    n_, s_ = sl_ // SPC, sl_ % SPC
    c_ = np.searchsorted(np.array(AG_CHB), s_, side="right") - 1
    sizes = np.diff(np.array(AG_CHB))
    base_full = np.concatenate([[0], np.cumsum(sizes * NC)[:-1]])
    row_of_slot = (base_full[c_] + n_ * sizes[c_]
                   + (s_ - np.array(AG_CHB)[c_]))

    inv_cnt = (1.0 / np.maximum(cnt_of_slot.reshape(NC, SPC), 1.0)).astype(np.float32)

    g_of_slot = np.full(NSLOTS, -1, dtype=np.int64)
    real = node_of_slot < N_NODES
    g_of_slot[real] = batch[node_of_slot[real]]
    NGW = 8
    Bg = 0
    pools = [[None] * NGW for _ in range(NC)]
    for c in range(NC):
        gl = g_of_slot[c * SPC:(c + 1) * SPC]
        for gw in range(NGW):
            m = np.where((gl >= gw * 128) & (gl < (gw + 1) * 128))[0]
            pools[c][gw] = m
            Bg = max(Bg, (len(m) + 127) // 128)
    NPG = Bg * 128
    pool_idx = np.zeros((NC, NGW, NPG), dtype=np.int16)
    pool_gwl = np.full((NC, NGW, NPG), -1.0, dtype=np.float32)
    for c in range(NC):
        for gw in range(NGW):
            m = pools[c][gw]
            pool_idx[c, gw, :len(m)] = m.astype(np.int16)
            pool_idx[c, gw, len(m):] = czero[c]
            pool_gwl[c, gw, :len(m)] = (g_of_slot[c * SPC + m] - gw * 128).astype(np.float32)

    gcnt = np.bincount(batch, minlength=N_GRAPHS).astype(np.float32)
    inv_g = np.zeros(1024, dtype=np.float32)
    inv_g[:N_GRAPHS] = 1.0 / np.maximum(gcnt, 1.0)

    return dict(slot_of_node=slot_of_node, node_of_slot=node_of_slot,
                row_of_slot=row_of_slot,
                xj_glob=xj_glob, dstwin=dstwin, pad_cnt=pad_cnt,
                inv_cnt=inv_cnt, valid=valid, eorder=eorder, ec=ec, pos=pos,
                pool_idx=pool_idx, pool_gwl=pool_gwl, inv_g=inv_g, Bg=Bg)


def _wrap_idx(a):
    """[.., n] int -> [.., 128, n//16]: element i -> partition i%16 col i//16,
    replicated to 8 groups of 16 partitions."""
    n = a.shape[-1]
    assert n % 16 == 0
    w = a.reshape(*a.shape[:-1], n // 16, 16)
    w = np.swapaxes(w, -1, -2)
    w = np.broadcast_to(w[..., None, :, :], (*a.shape[:-1], 8, 16, n // 16))
    return np.ascontiguousarray(w).reshape(*a.shape[:-1], 128, n // 16).astype(np.int16)


def _bf(x):
    return np.ascontiguousarray(np.asarray(x, dtype=np.float32)).astype(ml_dtypes.bfloat16)


def _tile_w(w):
    K, M = w.shape
    nk, nm = (K + 127) // 128, (M + 127) // 128
    out = np.zeros((nk, nm, 128, 128), dtype=ml_dtypes.bfloat16)
    for i in range(nk):
        for j in range(nm):
            blk = np.asarray(w, dtype=np.float32)[i * 128:(i + 1) * 128, j * 128:(j + 1) * 128]
            out[i, j, :blk.shape[0], :blk.shape[1]] = _bf(blk)
    return out


# ============================ device kernel ============================

EHALF = E_PAD // 2        # 25088
NSEG_H = EHALF // 512     # 49


def _build(Bg, debug=False, phases=4):
    nc = bacc.Bacc("TRN2", target_bir_lowering=False, debug=False, num_devices=NC)

    def din(name, shape, dt):
        return nc.dram_tensor(name, shape, dt, kind="ExternalInput").ap()

    NIDX = E_PAD // 16
    t_msgT = din("msgT", [48, EHALF], BF16)
    t_xj = din("xj_idx", [128, E_PAD // 128], mybir.dt.int32)
    t_dstwin = din("dstwin", [128, E_PAD // 128], F32)
    t_invcnt = din("invcnt", [128, NWIN], F32)
    t_padcnt = din("padcnt", [128, 1], F32)
    t_iota = din("iota", [128, 128], F32)
    t_ident = din("ident", [128, 128], BF16)
    t_c1w = din("c1w", [3, 128, 128], BF16)
    t_c1a = din("c1a", [2, 128, 1], F32)
    t_c1b = din("c1b", [3, 128, 1], F32)
    t_c1gn = din("c1gn", [3, 3, 128, 1], F32)
    t_c2wa = din("c2wa", [2, 128, 128], BF16)
    t_c2wb = din("c2wb", [2, 128, 128], BF16)
    t_c2w2 = din("c2w2", [2, 2, 128, 128], BF16)
    t_c2b = din("c2b", [2, 2, 128, 1], F32)
    t_c2gn = din("c2gn", [2, 3, 2, 128, 1], F32)
    t_c3wa = din("c3wa", [2, 2, 128, 128], BF16)
    t_c3wb = din("c3wb", [2, 2, 128, 128], BF16)
    t_c3b = din("c3b", [2, 128, 1], F32)
    t_c3gn = din("c3gn", [3, 2, 128, 1], F32)
    t_lw1 = din("lw1", [2, 2, 128, 128], BF16)
    t_lb1 = din("lb1", [2, 128, 1], F32)
    t_lw2 = din("lw2", [2, 128, 2], BF16)
    t_lb2 = din("lb2", [2, 1], F32)
    t_pidx16 = din("pidx16", [128, 8 * Bg * 128 // 16], I16)
    t_pgwl = din("pool_gwl", [128, 8 * Bg], F32)
    t_invg = din("invg", [128, 8], F32)

    o_out = nc.dram_tensor("out", [2, N_GRAPHS], F32, kind="ExternalOutput").ap()
    dbg = {}
    if debug:
        dbg["x1"] = nc.dram_tensor("dbg_x1", [NSLOTS, 128], BF16, kind="ExternalOutput").ap()
        dbg["x2"] = nc.dram_tensor("dbg_x2", [NSLOTS, 256], BF16, kind="ExternalOutput").ap()
        dbg["x3"] = nc.dram_tensor("dbg_x3", [SPC, 256], BF16, kind="ExternalOutput").ap()
        dbg["pool"] = nc.dram_tensor("dbg_pool", [1024, 256], F32, kind="ExternalOutput").ap()

    with tile.TileContext(nc) as tc:
        with tc.tile_pool(name="dram", bufs=1, space="DRAM") as dram, \
             tc.tile_pool(name="cp", bufs=1) as cp:
            z_scr = [dram.tile([2, 128, E_PAD], BF16, tag=f"zscr{i}", name=f"zscr{i}") for i in range(2)]
            tab1_loc = dram.tile([SPC, 128], BF16)
            tab1 = dram.tile([NSLOTS, 128], BF16)
            tab2_loc = dram.tile([SPC, 256], BF16)
            tab2 = dram.tile([NSLOTS, 256], BF16)
            tab3_loc = dram.tile([SPC, 256], BF16)
            st_in = dram.tile([128, 8], F32)
            st_out = dram.tile([128, 8], F32)
            pool_in = dram.tile([1024, 256], F32)
            pool_out = dram.tile([1024, 256], F32)

            ident = cp.tile([128, 128], BF16)
            nc.sync.dma_start(ident[:], t_ident[:])
            iota = cp.tile([128, 128], F32)
            nc.sync.dma_start(iota[:], t_iota[:])
            invcnt = cp.tile([128, NWIN], F32)
            nc.sync.dma_start(invcnt[:], t_invcnt[:])
            dwin = cp.tile([128, E_PAD // 128], F32)
            nc.sync.dma_start(dwin[:], t_dstwin[:])
            padcnt = cp.tile([128, 1], F32)
            nc.sync.dma_start(padcnt[:], t_padcnt[:])

            # ---------- helpers ----------
            def allreduce_stats(s_acc, q_acc, n_mb, sb):
                st = sb.tile([128, 8], F32, tag="st_")
                nc.vector.memset(st[:], 0.0)
                nc.vector.tensor_copy(st[:, 0:n_mb], s_acc[:])
                nc.vector.tensor_copy(st[:, 4:4 + n_mb], q_acc[:])
                nc.sync.dma_start(st_in[:], st[:])
                nc.gpsimd.collective_compute(
                    "AllReduce", AOP.add, replica_groups=[list(range(NC))],
                    ins=[st_in.opt()], outs=[st_out.opt()])
                stg = sb.tile([128, 8], F32, tag="stg_")
                nc.sync.dma_start(stg[:], st_out[:])
                return stg

            def affine_from_stats(stg, n_mb, b_lin, gn, sb):
                A, Cc = [], []
                for mb in range(n_mb):
                    s = stg[:, mb:mb + 1]
                    q = stg[:, 4 + mb:5 + mb]
                    g, bgn, ms = gn[0][mb], gn[1][mb], gn[2][mb]
                    bl = b_lin[mb]
                    m = sb.tile([128, 1], F32, tag="af_m")
                    nc.vector.tensor_scalar(m[:], s, 1.0 / N_EDGES, None, AOP.mult)
                    nc.vector.tensor_tensor(m[:], m[:], bl, op=AOP.add)
                    e2 = sb.tile([128, 1], F32, tag="af_e2")
                    nc.vector.tensor_scalar(e2[:], q, 1.0 / N_EDGES, None, AOP.mult)
                    tmp = sb.tile([128, 1], F32, tag="af_t")
                    nc.vector.tensor_tensor(tmp[:], m[:], bl, op=AOP.mult)
                    nc.vector.tensor_scalar(tmp[:], tmp[:], 2.0, None, AOP.mult)
                    nc.vector.tensor_tensor(e2[:], e2[:], tmp[:], op=AOP.add)
                    nc.vector.tensor_tensor(tmp[:], bl, bl, op=AOP.mult)
                    nc.vector.tensor_tensor(e2[:], e2[:], tmp[:], op=AOP.subtract)
                    msm = sb.tile([128, 1], F32, tag="af_msm")
                    nc.vector.tensor_tensor(msm[:], ms, m[:], op=AOP.mult)
                    var = sb.tile([128, 1], F32, tag="af_v")
                    nc.vector.tensor_tensor(var[:], msm[:], msm[:], op=AOP.mult)
                    nc.vector.tensor_tensor(tmp[:], msm[:], m[:], op=AOP.mult)
                    nc.vector.tensor_scalar(tmp[:], tmp[:], 2.0, None, AOP.mult)
                    nc.vector.tensor_tensor(var[:], var[:], tmp[:], op=AOP.subtract)
                    nc.vector.tensor_tensor(var[:], var[:], e2[:], op=AOP.add)
                    a = sb.tile([128, 1], F32, tag="af_a")
                    nc.vector.tensor_scalar(var[:], var[:], EPS, None, AOP.add)
                    nc.scalar.activation(a[:], var[:], AFT.Sqrt)
                    nc.vector.reciprocal(a[:], a[:])
                    nc.vector.tensor_tensor(a[:], a[:], g, op=AOP.mult)
                    cc = sb.tile([128, 1], F32, tag="af_c")
                    nc.vector.tensor_tensor(cc[:], bl, msm[:], op=AOP.subtract)
                    nc.vector.tensor_tensor(cc[:], cc[:], a[:], op=AOP.mult)
                    nc.vector.tensor_tensor(cc[:], cc[:], bgn, op=AOP.add)
                    A.append(a)
                    Cc.append(cc)
                return A, Cc

            def acc_stats(ps_ap, s_col, q_col, sb):
                t1 = sb.tile([128, 1], F32, tag="rs_t1")
                nc.vector.reduce_sum(out=t1[:], in_=ps_ap, axis=AX.X)
                nc.vector.tensor_tensor(s_col, s_col, t1[:], op=AOP.add)
                n = ps_ap.shape[-1]
                sq = sb.tile([128, 512], BF16, tag="rs_sq")
                qa = sb.tile([128, 1], F32, tag="rs_qa")
                nc.scalar.activation(sq[:, :n], ps_ap, AFT.Square, accum_out=qa[:])
                nc.vector.tensor_tensor(q_col, q_col, qa[:], op=AOP.add)

            def bn_finish(st, s_col, q_col, sb, tag):
                # bn_stats 6-tuples (equal 512-col groups) -> sum / sq-sum
                agg = sb.tile([128, 2], F32, tag=tag + "g")
                nc.vector.bn_aggr(agg[:], st[:])
                nc.vector.tensor_scalar(s_col, agg[:, 0:1], float(E_PAD),
                                        None, AOP.mult)
                t = sb.tile([128, 1], F32, tag=tag + "t")
                nc.vector.tensor_tensor(t[:], agg[:, 0:1], agg[:, 0:1], op=AOP.mult)
                nc.vector.tensor_tensor(t[:], t[:], agg[:, 1:2], op=AOP.add)
                nc.vector.tensor_scalar(q_col, t[:], float(E_PAD), None, AOP.mult)

            def sentinel_correct(s_acc, q_acc, zsent_cols, n_mb, sb):
                for mb in range(n_mb):
                    zs = zsent_cols[mb]
                    t1 = sb.tile([128, 1], F32, tag="sc_t1")
                    nc.vector.tensor_tensor(t1[:], zs, padcnt[:], op=AOP.mult)
                    nc.vector.tensor_tensor(s_acc[:, mb:mb + 1], s_acc[:, mb:mb + 1],
                                            t1[:], op=AOP.subtract)
                    nc.vector.tensor_tensor(t1[:], zs, zs, op=AOP.mult)
                    nc.vector.tensor_tensor(t1[:], t1[:], padcnt[:], op=AOP.mult)
                    nc.vector.tensor_tensor(q_acc[:, mb:mb + 1], q_acc[:, mb:mb + 1],
                                            t1[:], op=AOP.subtract)

            def load_vec(t_ap, sb, tag):
                v = sb.tile([128, 1], F32, tag=tag)
                nc.sync.dma_start(v[:], t_ap)
                return v[:]

            AG_BASE = [0]
            for _c in range(3):
                AG_BASE.append(AG_BASE[-1] + NC * (AG_CHB[_c + 1] - AG_CHB[_c]))

            def fire_ag(tab_loc, tab_full, c):
                # chunk-major table: AG chunk c is a contiguous row block
                lo, hi = AG_CHB[c], AG_CHB[c + 1]
                nc.gpsimd.collective_compute(
                    "AllGather", AOP.bypass, replica_groups=[list(range(NC))],
                    ins=[tab_loc[lo:hi, :].opt()],
                    outs=[tab_full[AG_BASE[c]:AG_BASE[c] + NC * (hi - lo),
                                   :].opt()])

            def scatter_pass(zsrc, n_mb, A, Cc, tab_loc, Cout, ag=None):
                with tc.tile_pool(name="sc_sb", bufs=2) as sb, \
                     tc.tile_pool(name="sc_tp", bufs=2, space="PSUM") as ps_tp, \
                     tc.tile_pool(name="sc_sc", bufs=2, space="PSUM") as ps_sc:
                    for b in range(NBLK):
                        if ag is not None and b in AG_FIRE:
                            fire_ag(tab_loc, ag, AG_FIRE.index(b))
                        hs = []
                        for mb in range(n_mb):
                            z = sb.tile([128, BLK], BF16, tag=f"sp_z{mb}")
                            nc.sync.dma_start(z[:], zsrc[mb, :, b * BLK:(b + 1) * BLK])
                            h = sb.tile([128, BLK], BF16, tag=f"sp_h{mb}")
                            nc.scalar.activation(h[:], z[:], AFT.Relu,
                                                 bias=Cc[mb], scale=A[mb])
                            hs.append(h)
                        hE = sb.tile([128, NCHUNK * Cout], BF16, tag="sp_hE")
                        for ch in range(NCHUNK):
                            for mb in range(n_mb):
                                tp = ps_tp.tile([128, 128], BF16, tag="sp_tp", space="PSUM")
                                nc.tensor.transpose(tp[:], hs[mb][:, ch * 128:(ch + 1) * 128],
                                                    ident[:])
                                nc.vector.tensor_copy(
                                    hE[:, ch * Cout + mb * 128:ch * Cout + (mb + 1) * 128],
                                    tp[:])
                        for w in range(NW_BLK):
                            gw = b * NW_BLK + w
                            sc = ps_sc.tile([128, Cout], F32, tag="sp_sc", space="PSUM")
                            for cb in range(B):
                                ch = w * B + cb
                                col = b * NCHUNK + ch
                                oh = sb.tile([128, 128], BF16, tag="sp_oh")
                                nc.vector.tensor_tensor(
                                    out=oh[:],
                                    in0=dwin[:, col:col + 1].to_broadcast([128, 128]),
                                    in1=iota[:], op=AOP.is_equal)
                                nc.tensor.matmul(sc[:], oh[:],
                                                 hE[:, ch * Cout:(ch + 1) * Cout],
                                                 start=(cb == 0), stop=(cb == B - 1))
                            nt = sb.tile([128, Cout], BF16, tag="sp_nt")
                            nc.vector.tensor_scalar(nt[:], sc[:], invcnt[:, gw:gw + 1],
                                                    None, AOP.mult)
                            nc.sync.dma_start(tab_loc[gw * WIN:(gw + 1) * WIN, :], nt[:])
                    if ag is not None:
                        fire_ag(tab_loc, ag, 3)

            # ======================= CONV 1 =======================
            # SBUF-resident: L1 stats precomputed on host; h kept on-chip,
            # L2 overwrites it in place; L3 fused with the scatter.
            NSEG_T = E_PAD // 512  # 98
            with tc.tile_pool(name="c1sb", bufs=2) as sb:
                c1b = [[load_vec(t_c1b[i], sb, f"c1b{i}")] for i in range(3)]
                c1gn = [[[load_vec(t_c1gn[i, j], sb, f"c1gn{i}{j}")] for j in range(3)]
                        for i in range(3)]
                A1h = load_vec(t_c1a[0], sb, "c1a0")
                C1h = load_vec(t_c1a[1], sb, "c1a1")
                with tc.tile_pool(name="c1h", bufs=1) as hp, \
                     tc.tile_pool(name="c1ps", bufs=2, space="PSUM") as ps, \
                     tc.tile_pool(name="c1p2", bufs=2, space="PSUM") as ps2:
                    c1w = []
                    for i in range(3):
                        w = sb.tile([128, 128], BF16, tag=f"c1w{i}")
                        nc.sync.dma_start(w[:], t_c1w[i])
                        c1w.append(w)
                    msgT = hp.tile([48, EHALF], BF16, tag="msgT")
                    nc.sync.dma_start(msgT[:], t_msgT[:])
                    h_full = hp.tile([128, E_PAD], BF16)

                    def bn_to_sq(st, tag):
                        agg = sb.tile([128, 2], F32, tag=tag + "agg")
                        nc.vector.bn_aggr(agg[:], st[:])
                        s_acc = sb.tile([128, 1], F32, tag=tag + "s")
                        q_acc = sb.tile([128, 1], F32, tag=tag + "q")
                        nc.vector.tensor_scalar(s_acc[:], agg[:, 0:1],
                                                float(E_PAD), None, AOP.mult)
                        nc.vector.tensor_tensor(q_acc[:], agg[:, 0:1], agg[:, 0:1],
                                                op=AOP.mult)
                        nc.vector.tensor_tensor(q_acc[:], q_acc[:], agg[:, 1:2],
                                                op=AOP.add)
                        nc.vector.tensor_scalar(q_acc[:], q_acc[:],
                                                float(E_PAD), None, AOP.mult)
                        return s_acc, q_acc

                    # pass 1: L1 -> h_full; L2 stats
                    st2 = hp.tile([128, NSEG_T * 6], F32, tag="st2")
                    zs2 = sb.tile([128, 1], F32, tag="zs2")
                    for g in range(NSEG_T):
                        hh, shalf = g // NSEG_H, g % NSEG_H
                        zp = ps.tile([128, 512], F32, tag="zp")
                        nc.tensor.matmul(zp[:], c1w[0][32 * hh:32 * hh + 10, :],
                                         msgT[32 * hh:32 * hh + 10,
                                              shalf * 512:(shalf + 1) * 512],
                                         start=True, stop=True)
                        nc.scalar.activation(h_full[:, g * 512:(g + 1) * 512], zp[:],
                                             AFT.Relu, bias=C1h, scale=A1h)
                        zp2 = ps2.tile([128, 512], F32, tag="zp2")
                        nc.tensor.matmul(zp2[:], c1w[1][:],
                                         h_full[:, g * 512:(g + 1) * 512],
                                         start=True, stop=True)
                        nc.vector.bn_stats(st2[:, g * 6:(g + 1) * 6], zp2[:])
                        if g == NSEG_T - 1:
                            nc.vector.tensor_copy(zs2[:], zp2[:, 511:512])
                    s2, q2 = bn_to_sq(st2, "b2")
                    sentinel_correct(s2, q2, [zs2[:]], 1, sb)
                    stg2 = allreduce_stats(s2, q2, 1, sb)
                    A2, C2 = affine_from_stats(stg2, 1, c1b[1], c1gn[1], sb)

                    # pass 2: L2 -> h_full (in place); L3 stats
                    st3 = hp.tile([128, NSEG_T * 6], F32, tag="st3")
                    zs3 = sb.tile([128, 1], F32, tag="zs3")
                    for g in range(NSEG_T):
                        zp = ps.tile([128, 512], F32, tag="zp")
                        nc.tensor.matmul(zp[:], c1w[1][:],
                                         h_full[:, g * 512:(g + 1) * 512],
                                         start=True, stop=True)
                        nc.scalar.activation(h_full[:, g * 512:(g + 1) * 512], zp[:],
                                             AFT.Relu, bias=C2[0], scale=A2[0])
                        zp3 = ps2.tile([128, 512], F32, tag="zp2")
                        nc.tensor.matmul(zp3[:], c1w[2][:],
                                         h_full[:, g * 512:(g + 1) * 512],
                                         start=True, stop=True)
                        nc.vector.bn_stats(st3[:, g * 6:(g + 1) * 6], zp3[:])
                        if g == NSEG_T - 1:
                            nc.vector.tensor_copy(zs3[:], zp3[:, 511:512])
                    s3, q3 = bn_to_sq(st3, "b3")
                    sentinel_correct(s3, q3, [zs3[:]], 1, sb)
                    stg3 = allreduce_stats(s3, q3, 1, sb)
                    A3, C3 = affine_from_stats(stg3, 1, c1b[2], c1gn[2], sb)

                    # pass 3: L3 + fused scatter
                    with tc.tile_pool(name="c1sc", bufs=2) as scb, \
                         tc.tile_pool(name="c1tp", bufs=2, space="PSUM") as ps_tp, \
                         tc.tile_pool(name="c1s2", bufs=2, space="PSUM") as ps_sc:
                        for b in range(NBLK):
                            if b in AG_FIRE:
                                fire_ag(tab1_loc, tab1, AG_FIRE.index(b))
                            h3 = scb.tile([128, BLK], BF16, tag="c1h3")
                            for s in range(NSEG):
                                g = b * NSEG + s
                                zp = ps.tile([128, 512], F32, tag="zp")
                                nc.tensor.matmul(zp[:], c1w[2][:],
                                                 h_full[:, g * 512:(g + 1) * 512],
                                                 start=True, stop=True)
                                nc.scalar.activation(h3[:, s * 512:(s + 1) * 512],
                                                     zp[:], AFT.Relu,
                                                     bias=C3[0], scale=A3[0])
                            hE = scb.tile([128, NCHUNK * 128], BF16, tag="c1hE")
                            for ch in range(NCHUNK):
                                tp = ps_tp.tile([128, 128], BF16, tag="c1tp",
                                                space="PSUM")
                                nc.tensor.transpose(tp[:], h3[:, ch * 128:(ch + 1) * 128],
                                                    ident[:])
                                nc.vector.tensor_copy(hE[:, ch * 128:(ch + 1) * 128],
                                                      tp[:])
                            for w in range(NW_BLK):
                                gw = b * NW_BLK + w
                                sc = ps_sc.tile([128, 128], F32, tag="c1sc",
                                                space="PSUM")
                                for cb in range(B):
                                    ch = w * B + cb
                                    col = b * NCHUNK + ch
                                    oh = scb.tile([128, 128], BF16, tag="c1oh")
                                    nc.vector.tensor_tensor(
                                        out=oh[:],
                                        in0=dwin[:, col:col + 1].to_broadcast([128, 128]),
                                        in1=iota[:], op=AOP.is_equal)
                                    nc.tensor.matmul(sc[:], oh[:],
                                                     hE[:, ch * 128:(ch + 1) * 128],
                                                     start=(cb == 0), stop=(cb == B - 1))
                                nt = scb.tile([128, 128], BF16, tag="c1nt")
                                nc.vector.tensor_scalar(nt[:], sc[:],
                                                        invcnt[:, gw:gw + 1],
                                                        None, AOP.mult)
                                nc.sync.dma_start(tab1_loc[gw * WIN:(gw + 1) * WIN, :],
                                                  nt[:])
                        fire_ag(tab1_loc, tab1, 3)

            if debug:
                nc.sync.dma_start(dbg["x1"][:], tab1[:])

            # ============== gather-based first layer (conv2/conv3) ==============
            def gather_layer(tab_full, tab_loc, Cin, wa_t, wb_t, n_kb, zdst, sb):
                mb_in = Cin // 128
                s_acc = sb.tile([128, 2], F32, tag="gl_s")
                q_acc = sb.tile([128, 2], F32, tag="gl_q")
                sts = [sb.tile([128, (E_PAD // 512) * 6], F32, tag=f"gl_st{mo}",
                               name=f"gl_st{mo}")
                       for mo in range(2)]
                with tc.tile_pool(name="gl_g2", bufs=3) as g2, \
                     tc.tile_pool(name="gl_g1", bufs=2) as g1, \
                     tc.tile_pool(name="gl_zw", bufs=2) as zwp, \
                     tc.tile_pool(name="gl_ps", bufs=2, space="PSUM") as ps, \
                     tc.tile_pool(name="gl_tp", bufs=2, space="PSUM") as ps_tp, \
                     tc.tile_pool(name="gl_xp", bufs=2, space="PSUM") as ps_xp:
                    was, wbs = [], []
                    for ki in range(n_kb):
                        for mo in range(2):
                            wta = sb.tile([128, 128], BF16, tag=f"gl_wa{ki}{mo}")
                            nc.sync.dma_start(wta[:], wa_t[ki, mo] if n_kb > 1 else wa_t[mo])
                            was.append(wta)
                            wtb = sb.tile([128, 128], BF16, tag=f"gl_wb{ki}{mo}")
                            nc.sync.dma_start(wtb[:], wb_t[ki, mo] if n_kb > 1 else wb_t[mo])
                            wbs.append(wtb)
                    for b in range(NBLK):
                        ixj = g2.tile([128, NCHUNK], mybir.dt.int32, tag="gl_ixj")
                        nc.sync.dma_start(ixj[:], t_xj[:, b * NCHUNK:(b + 1) * NCHUNK])
                        gxj = g2.tile([128, NCHUNK * Cin], BF16, tag="gl_gxj")
                        for ch in range(NCHUNK):
                            nc.gpsimd.indirect_dma_start(
                                out=gxj[:, ch * Cin:(ch + 1) * Cin],
                                out_offset=None,
                                in_=tab_full[:],
                                in_offset=bass.IndirectOffsetOnAxis(
                                    ap=ixj[:, ch:ch + 1], axis=0))
                        xjT = g1.tile([128, mb_in * BLK], BF16, tag="gl_xjT")
                        for ch in range(NCHUNK):
                            for kb in range(mb_in):
                                tp2 = ps_tp.tile([128, 128], BF16, tag="gl_ohp",
                                                 space="PSUM")
                                nc.tensor.transpose(
                                    tp2[:],
                                    gxj[:, ch * Cin + kb * 128:ch * Cin + (kb + 1) * 128],
                                    ident[:])
                                nc.vector.tensor_copy(
                                    xjT[:, kb * BLK + ch * 128:kb * BLK + (ch + 1) * 128],
                                    tp2[:])
                        # xi via window expansion
                        xiT = g1.tile([128, mb_in * BLK], BF16, tag="gl_xiT")
                        for w in range(NW_BLK):
                            gw = b * NW_BLK + w
                            twin = g2.tile([128, Cin], BF16, tag="gl_twin")
                            nc.sync.dma_start(twin[:], tab_loc[gw * WIN:(gw + 1) * WIN, :])
                            for cb in range(B):
                                ch = w * B + cb
                                col = b * NCHUNK + ch
                                oh = g2.tile([128, 128], BF16, tag="gl_oh")
                                nc.vector.tensor_tensor(
                                    out=oh[:],
                                    in0=dwin[:, col:col + 1].to_broadcast([128, 128]),
                                    in1=iota[:], op=AOP.is_equal)
                                ohp = ps_tp.tile([128, 128], BF16, tag="gl_ohp", space="PSUM")
                                nc.tensor.transpose(ohp[:], oh[:], ident[:])
                                oh2 = g2.tile([128, 128], BF16, tag="gl_oh2")
                                nc.vector.tensor_copy(oh2[:], ohp[:])
                                for kb in range(mb_in):
                                    xp = ps_xp.tile([128, 128], F32, tag="gl_xp", space="PSUM")
                                    nc.tensor.matmul(xp[:], twin[:, kb * 128:(kb + 1) * 128],
                                                     oh2[:], start=True, stop=True)
                                    nc.vector.tensor_copy(
                                        xiT[:, kb * BLK + ch * 128:kb * BLK + (ch + 1) * 128],
                                        xp[:])
                        for mo in range(2):
                            zw = zwp.tile([128, BLK], BF16, tag=f"gl_z{mo}")
                            for sg in range(NSEG):
                                g6 = (b * NSEG + sg) * 6
                                zp = ps.tile([128, 512], F32, tag="gl_zp")
                                for ki in range(mb_in):
                                    nc.tensor.matmul(
                                        zp[:], was[ki * 2 + mo][:],
                                        xiT[:, ki * BLK + sg * 512:ki * BLK + (sg + 1) * 512],
                                        start=(ki == 0), stop=False)
                                for ki in range(mb_in):
                                    nc.tensor.matmul(
                                        zp[:], wbs[ki * 2 + mo][:],
                                        xjT[:, ki * BLK + sg * 512:ki * BLK + (sg + 1) * 512],
                                        start=False, stop=(ki == mb_in - 1))
                                nc.vector.bn_stats(sts[mo][:, g6:g6 + 6], zp[:])
                                nc.vector.tensor_copy(zw[:, sg * 512:(sg + 1) * 512],
                                                      zp[:])
                            nc.sync.dma_start(zdst[mo, :, b * BLK:(b + 1) * BLK], zw[:])
                for mo in range(2):
                    bn_finish(sts[mo], s_acc[:, mo:mo + 1], q_acc[:, mo:mo + 1],
                              sb, f"glf{mo}")
                return s_acc, q_acc

            # ======================= CONV 2 =======================
            if phases >= 2:
              with tc.tile_pool(name="c2sb", bufs=2) as sb:
                  c2b = [[load_vec(t_c2b[i, mb], sb, f"c2b{i}{mb}") for mb in range(2)]
                         for i in range(2)]
                  c2gn = [[[load_vec(t_c2gn[i, j, mb], sb, f"c2gn{i}{j}{mb}")
                            for mb in range(2)] for j in range(3)] for i in range(2)]
                  sA, qA = gather_layer(tab1, tab1_loc, 128, t_c2wa, t_c2wb, 1,
                                        z_scr[0], sb)
                  stg = allreduce_stats(sA, qA, 2, sb)
                  A1, C1 = affine_from_stats(stg, 2, c2b[0], c2gn[0], sb)

                  s2 = sb.tile([128, 2], F32, tag="c2s2")
                  q2 = sb.tile([128, 2], F32, tag="c2q2")
                  st2s = [sb.tile([128, (E_PAD // 512) * 6], F32, tag=f"c2st{mo}",
                                  name=f"c2st{mo}")
                          for mo in range(2)]
                  zlast = [None, None]
                  with tc.tile_pool(name="c2mid", bufs=2) as mp, \
                       tc.tile_pool(name="c2ps", bufs=2, space="PSUM") as ps:
                      w2s = []
                      for ki in range(2):
                          for mo in range(2):
                              w = sb.tile([128, 128], BF16, tag=f"c2w2{ki}{mo}")
                              nc.sync.dma_start(w[:], t_c2w2[ki, mo])
                              w2s.append(w)
                      for b in range(NBLK):
                          h1 = []
                          for mb in range(2):
                              z = mp.tile([128, BLK], BF16, tag=f"c2z1r{mb}")
                              nc.sync.dma_start(z[:], z_scr[0][mb, :, b * BLK:(b + 1) * BLK])
                              hh = mp.tile([128, BLK], BF16, tag=f"c2h1{mb}")
                              nc.scalar.activation(hh[:], z[:], AFT.Relu,
                                                   bias=C1[mb], scale=A1[mb])
                              h1.append(hh)
                          for mo in range(2):
                              zw = mp.tile([128, BLK], BF16, tag=f"c2z2w{mo}")
                              for s in range(NSEG):
                                  g6 = (b * NSEG + s) * 6
                                  zp = ps.tile([128, 512], F32, tag="c2zp")
                                  for ki in range(2):
                                      nc.tensor.matmul(zp[:], w2s[ki * 2 + mo][:],
                                                       h1[ki][:, s * 512:(s + 1) * 512],
                                                       start=(ki == 0), stop=(ki == 1))
                                  nc.vector.bn_stats(st2s[mo][:, g6:g6 + 6], zp[:])
                                  nc.vector.tensor_copy(zw[:, s * 512:(s + 1) * 512],
                                                        zp[:])
                              nc.sync.dma_start(z_scr[1][mo, :, b * BLK:(b + 1) * BLK], zw[:])
                              zlast[mo] = zw
                      zsent = []
                      for mo in range(2):
                          zc = sb.tile([128, 1], F32, tag=f"c2zs{mo}")
                          nc.vector.tensor_copy(zc[:], zlast[mo][:, BLK - 1:BLK])
                          zsent.append(zc[:])
                  for mo in range(2):
                      bn_finish(st2s[mo], s2[:, mo:mo + 1], q2[:, mo:mo + 1],
                                sb, f"c2f{mo}")
                  sentinel_correct(s2, q2, zsent, 2, sb)
                  stg2 = allreduce_stats(s2, q2, 2, sb)
                  A2, C2 = affine_from_stats(stg2, 2, c2b[1], c2gn[1], sb)
                  scatter_pass(z_scr[1], 2, A2, C2, tab2_loc, 256, ag=tab2)

            if debug:
                nc.sync.dma_start(dbg["x2"][:], tab2[:])

            # ======================= CONV 3 =======================
            if phases >= 3:
              with tc.tile_pool(name="c3sb", bufs=2) as sb:
                  c3b = [load_vec(t_c3b[mb], sb, f"c3b{mb}") for mb in range(2)]
                  c3gn = [[load_vec(t_c3gn[j, mb], sb, f"c3gn{j}{mb}") for mb in range(2)]
                          for j in range(3)]
                  sA, qA = gather_layer(tab2, tab2_loc, 256, t_c3wa, t_c3wb, 2,
                                        z_scr[0], sb)
                  stg = allreduce_stats(sA, qA, 2, sb)
                  A1, C1 = affine_from_stats(stg, 2, c3b, c3gn, sb)
                  scatter_pass(z_scr[0], 2, A1, C1, tab3_loc, 256)

            if debug:
                nc.sync.dma_start(dbg["x3"][:], tab3_loc[:])

            # ======================= POOL + HEAD =======================
            if phases >= 4:
              with tc.tile_pool(name="p_sb", bufs=2) as sb, \
                 tc.tile_pool(name="p_ps", bufs=2, space="PSUM") as ps:
                  pgwl = sb.tile([128, 8 * Bg], F32, tag="p_pgwl")
                  nc.sync.dma_start(pgwl[:], t_pgwl[:])
                  NPG = Bg * 128
                  pidxw = sb.tile([128, 8 * NPG // 16], I16, tag="p_idx16")
                  nc.sync.dma_start(pidxw[:], t_pidx16[:])
                  for gw in range(8):
                      gp = sb.tile([128, Bg, 256], BF16, tag="p_gp")
                      nc.gpsimd.dma_gather(
                          out_ap=gp[:], in_ap=tab3_loc[:],
                          idxs_ap=pidxw[:, gw * (NPG // 16):(gw + 1) * (NPG // 16)],
                          num_idxs=NPG, num_idxs_reg=NPG, elem_size=256,
                          transpose=False, single_packet=(NPG <= 896))
                      pp = ps.tile([128, 256], F32, tag="p_pp", space="PSUM")
                      for c in range(Bg):
                          oh = sb.tile([128, 128], BF16, tag="p_oh")
                          nc.vector.tensor_tensor(
                              out=oh[:],
                              in0=pgwl[:, gw * Bg + c:gw * Bg + c + 1].to_broadcast([128, 128]),
                              in1=iota[:], op=AOP.is_equal)
                          nc.tensor.matmul(pp[:], oh[:], gp[:, c, :],
                                           start=(c == 0), stop=(c == Bg - 1))
                      pf = sb.tile([128, 256], F32, tag="p_pf")
                      nc.vector.tensor_copy(pf[:], pp[:])
                      nc.sync.dma_start(pool_in[gw * 128:(gw + 1) * 128, :], pf[:])
                  nc.gpsimd.collective_compute(
                      "AllReduce", AOP.add, replica_groups=[list(range(NC))],
                      ins=[pool_in.opt()], outs=[pool_out.opt()])
                  if debug:
                      nc.sync.dma_start(dbg["pool"][:], pool_out[:])

                  invg = sb.tile([128, 8], F32, tag="p_invg")
                  nc.sync.dma_start(invg[:], t_invg[:])
                  lw1 = []
                  for ki in range(2):
                      for mo in range(2):
                          w = sb.tile([128, 128], BF16, tag=f"p_lw1{ki}{mo}")
                          nc.sync.dma_start(w[:], t_lw1[ki, mo])
                          lw1.append(w)
                  lw2 = []
                  for ki in range(2):
                      w = sb.tile([128, 2], BF16, tag=f"p_lw2{ki}")
                      nc.sync.dma_start(w[:], t_lw2[ki])
                      lw2.append(w)
                  lb1 = [load_vec(t_lb1[mb], sb, f"p_lb1{mb}") for mb in range(2)]
                  lb2 = sb.tile([2, 1], F32, tag="p_lb2")
                  nc.sync.dma_start(lb2[:], t_lb2[:])
                  ofin = sb.tile([2, 1024], F32, tag="p_out")
                  for gw in range(8):
                      g = sb.tile([128, 256], F32, tag="p_g")
                      nc.sync.dma_start(g[:], pool_out[gw * 128:(gw + 1) * 128, :])
                      gm = sb.tile([128, 256], BF16, tag="p_gm")
                      nc.vector.tensor_scalar(gm[:], g[:], invg[:, gw:gw + 1], None, AOP.mult)
                      gT = sb.tile([128, 2 * 128], BF16, tag="p_gT")
                      for kb in range(2):
                          tp = ps.tile([128, 128], BF16, tag="p_tp", space="PSUM")
                          nc.tensor.transpose(tp[:], gm[:, kb * 128:(kb + 1) * 128], ident[:])
                          nc.vector.tensor_copy(gT[:, kb * 128:(kb + 1) * 128], tp[:])
                      hT = sb.tile([128, 2 * 128], BF16, tag="p_hT")
                      for mo in range(2):
                          hp = ps.tile([128, 128], F32, tag="p_hp", space="PSUM")
                          for ki in range(2):
                              nc.tensor.matmul(hp[:], lw1[ki * 2 + mo][:],
                                               gT[:, ki * 128:(ki + 1) * 128],
                                               start=(ki == 0), stop=(ki == 1))
                          nc.scalar.activation(hT[:, mo * 128:(mo + 1) * 128], hp[:],
                                               AFT.Relu, bias=lb1[mo])
                      op_ = ps.tile([2, 128], F32, tag="p_op", space="PSUM")
                      for ki in range(2):
                          nc.tensor.matmul(op_[:], lw2[ki][:],
                                           hT[:, ki * 128:(ki + 1) * 128],
                                           start=(ki == 0), stop=(ki == 1))
                      nc.vector.tensor_scalar(ofin[:, gw * 128:(gw + 1) * 128],
                                              op_[:], lb2[:], None, AOP.add)
                  nc.sync.dma_start(o_out[:], ofin[:, :N_GRAPHS])

    nc.compile()
    return nc


# ============================ entry point ============================


def kernel(**inputs):
    x = np.asarray(inputs["x"], dtype=np.float32)
    edge_index = np.asarray(inputs["edge_index"])
    batch = np.asarray(inputs["batch"])

    meta = _pack(edge_index, batch)
    Bg = meta["Bg"]

    import os as _os
    phases = int(_os.environ.get("KPHASES", "4"))
    key = ("mod", Bg, phases, _DEBUG[0])
    if key not in _cache:
        _cache[key] = _build(Bg, debug=bool(inputs.get("_debug", False)) or _DEBUG[0],
                             phases=phases)
    nc = _cache[key]

    # ---- per-core input arrays ----
    slot_of_node = meta["slot_of_node"]
    src = np.asarray(edge_index[0], dtype=np.int64)
    dst = np.asarray(edge_index[1], dtype=np.int64)

    # conv1 msgT: [core, 20, E_PAD//2] bf16; edge e<EHALF -> rows 0..9 col e,
    # e>=EHALF -> rows 10..19 col e-EHALF
    EHALF = E_PAD // 2
    xi_v = x[dst]
    xj_v = x[src]
    msg = np.concatenate([xi_v, xj_v - xi_v], axis=1)       # [E, 10]

    # exact conv1-L1 GraphNorm stats on host (tiny 10-dim Gram)
    msg64 = msg.astype(np.float64)
    W1 = np.asarray(inputs["c1_w1"], np.float64)            # [10, 128]
    b1 = np.asarray(inputs["c1_b1"], np.float64)            # [128]
    S = msg64.sum(0)
    G = msg64.T @ msg64
    SW = S @ W1
    qz = np.einsum('ij,ik,kj->j', W1, G, W1) + 2 * b1 * SW + N_EDGES * b1 * b1
    m1 = (SW + N_EDGES * b1) / N_EDGES
    e2 = qz / N_EDGES
    gn1 = np.asarray(inputs["c1_gn1"], np.float64)          # [3, 128]
    msm = gn1[2] * m1
    var1 = e2 - 2 * msm * m1 + msm * msm
    A1h = gn1[0] / np.sqrt(var1 + EPS)
    C1h = gn1[1] + A1h * (b1 - msm)
    c1a_in = np.stack([A1h, C1h]).astype(np.float32).reshape(2, 128, 1)

    msg_full = np.zeros((NC, E_PAD, 10), dtype=np.float32)
    ec, pos = meta["ec"], meta["pos"]
    msg_full[ec, pos] = msg[meta["eorder"]]
    msgT = np.zeros((NC, 48, EHALF), dtype=ml_dtypes.bfloat16)
    msgT[:, :10, :] = _bf(msg_full[:, :EHALF].transpose(0, 2, 1))
    msgT[:, 32:42, :] = _bf(msg_full[:, EHALF:].transpose(0, 2, 1))

    dstwin = meta["dstwin"]  # [NC, E_PAD]
    dwin_in = np.ascontiguousarray(
        dstwin.reshape(NC, E_PAD // 128, 128).transpose(0, 2, 1)).astype(np.float32)
    invcnt_in = np.ascontiguousarray(
        meta["inv_cnt"].reshape(NC, NWIN, 128).transpose(0, 2, 1)).astype(np.float32)
    padcnt_in = np.repeat(meta["pad_cnt"][:, None], 128, axis=1)[:, :, None].astype(np.float32)

    iota_in = np.broadcast_to(np.arange(128, dtype=np.float32)[None, :], (128, 128))
    iota_in = np.ascontiguousarray(iota_in)
    ident_in = np.eye(128, dtype=np.float32).astype(ml_dtypes.bfloat16)

    xj_row = meta["row_of_slot"][meta["xj_glob"]]  # [NC, E_PAD] chunk-major rows
    xj_in = np.ascontiguousarray(
        xj_row.reshape(NC, E_PAD // 128, 128).transpose(0, 2, 1)).astype(np.int32)

    # weights
    c1w = np.zeros((3, 128, 128), dtype=ml_dtypes.bfloat16)
    c1w[0, :10, :] = _bf(inputs["c1_w1"])
    c1w[0, 32:42, :] = _bf(inputs["c1_w1"])
    c1w[1] = _bf(inputs["c1_w2"])
    c1w[2] = _bf(inputs["c1_w3"])
    c1b = np.stack([np.asarray(inputs[f"c1_b{i}"], dtype=np.float32).reshape(128, 1)
                    for i in (1, 2, 3)])
    c1gn = np.stack([np.asarray(inputs[f"c1_gn{i}"], dtype=np.float32).reshape(3, 128, 1)
                     for i in (1, 2, 3)])

    w2a = np.asarray(inputs["c2_w1"], dtype=np.float32)   # [256, 256]
    WA2 = w2a[:128] - w2a[128:]
    WB2 = w2a[128:]
    c2wa = _tile_w(WA2)[0]                                # [2, 128, 128]
    c2wb = _tile_w(WB2)[0]
    c2w2 = _tile_w(np.asarray(inputs["c2_w2"], dtype=np.float32))  # [2,2,128,128]
    c2b = np.stack([np.asarray(inputs["c2_b1"], dtype=np.float32).reshape(2, 128, 1),
                    np.asarray(inputs["c2_b2"], dtype=np.float32).reshape(2, 128, 1)])
    c2gn = np.stack([np.asarray(inputs["c2_gn1"], dtype=np.float32).reshape(3, 2, 128, 1),
                     np.asarray(inputs["c2_gn2"], dtype=np.float32).reshape(3, 2, 128, 1)])

    w3a = np.asarray(inputs["c3_w1"], dtype=np.float32)   # [512, 256]
    WA3 = w3a[:256] - w3a[256:]
    WB3 = w3a[256:]
    c3wa = _tile_w(WA3)                                   # [2,2,128,128]
    c3wb = _tile_w(WB3)
    c3b = np.asarray(inputs["c3_b1"], dtype=np.float32).reshape(2, 128, 1)
    c3gn = np.asarray(inputs["c3_gn1"], dtype=np.float32).reshape(3, 2, 128, 1)

    lw1 = _tile_w(np.asarray(inputs["lin_w1"], dtype=np.float32))
    lb1 = np.asarray(inputs["lin_b1"], dtype=np.float32).reshape(2, 128, 1)
    lw2_f = np.asarray(inputs["lin_w2"], dtype=np.float32)  # [256, 2]
    lw2 = np.stack([_bf(lw2_f[:128]), _bf(lw2_f[128:])])    # [2, 128, 2]
    lb2 = np.asarray(inputs["lin_b2"], dtype=np.float32).reshape(2, 1)

    Bg0 = meta["Bg"]
    pidx16_in = _wrap_idx(meta["pool_idx"].reshape(NC, 8 * Bg0 * 128))
    pidx16_in = pidx16_in.reshape(NC, 128, -1)
    pgwl = meta["pool_gwl"]                # [NC, 8, NPG]
    Bg_ = meta["Bg"]
    pgwl_in = np.ascontiguousarray(
        pgwl.reshape(NC, 8, Bg_, 128).transpose(0, 3, 1, 2)).reshape(NC, 128, 8 * Bg_)
    invg_in = np.broadcast_to(
        meta["inv_g"].reshape(8, 128).T[None], (NC, 128, 8)).astype(np.float32)
    invg_in = np.ascontiguousarray(invg_in)

    in_maps = []
    for c in range(NC):
        im = {
            "msgT": msgT[c],
            "xj_idx": xj_in[c],
            "dstwin": dwin_in[c],
            "invcnt": invcnt_in[c],
            "padcnt": padcnt_in[c],
            "iota": iota_in,
            "ident": ident_in,
            "c1w": c1w, "c1a": c1a_in, "c1b": c1b, "c1gn": c1gn,
            "c2wa": c2wa, "c2wb": c2wb, "c2w2": c2w2, "c2b": c2b, "c2gn": c2gn,
            "c3wa": c3wa, "c3wb": c3wb, "c3b": c3b, "c3gn": c3gn,
            "lw1": lw1, "lb1": lb1, "lw2": lw2, "lb2": lb2,
            "pidx16": pidx16_in[c],
            "pool_gwl": pgwl_in[c].astype(np.float32),
            "invg": invg_in[c],
        }
        in_maps.append(im)

    res = run_bass_kernel_spmd(nc, in_maps, core_ids=list(range(NC)),
                               trace=_TRACE[0])
    kernel.last_result = res
    kernel.last_meta = meta
    out = res.results[0]["out"]            # [2, 1000]
    return np.ascontiguousarray(out.T).astype(np.float32)


_DEBUG = [False]
_TRACE = [False]



# revision 53
# speedup vs baseline: 1.2393x; 1.2393x over previous
"""LundNetTagger GNN on 8 Trainium2 NeuronCores (Bass/Tile).

Self-contained: kernel(**inputs) -> np.ndarray [1000, 2] float32.

Strategy: nodes are assigned to 100352 "slots" (8 cores x 98 windows x 128),
packed so each window receives <= 512 edges. Edges live on the core owning
their dst slot, in window-major order padded to 4x128-edge chunks per window.
Per-edge MLPs run in bf16 feature-major layout; EdgeConv cat[xi, xj-xi] is
folded into split weights WA = W[:C]-W[C:], WB = W[C:]. GraphNorm stats are
global AllReduces of per-core sums (conv1 layer-1 stats are computed exactly
on the host from the 10-dim message Gram; deeper layers use vector-engine
bn_stats on PSUM with a sentinel pad column for exact correction).
conv1 keeps h fully SBUF-resident (no z spills): layer 2 overwrites h in
place after its stats AllReduce, and layer 3 fuses into the scatter.
Mean-aggregation is a collision-free one-hot matmul scatter into PSUM per
window. Node tables are AllGathered in bf16 between convs in two chunk-major
halves (each half fires as soon as its windows are written, overlapping the
producing scatter); src-side gathers use per-chunk indirect DMA with
chunk-major global row indices.
"""
import numpy as np
import ml_dtypes

import concourse.bass as bass
import concourse.tile as tile
from concourse import bacc, mybir
from concourse.bass_utils import run_bass_kernel_spmd
from concourse import library_config

BF16 = mybir.dt.bfloat16
F32 = mybir.dt.float32
I16 = mybir.dt.int16
AOP = mybir.AluOpType
AFT = mybir.ActivationFunctionType
AX = mybir.AxisListType

N_NODES = 100000
N_EDGES = 400000
N_GRAPHS = 1000
NC = 8
WIN = 128
NWIN = 98
SPC = WIN * NWIN          # 12544
NSLOTS = SPC * NC         # 100352
QUAD = NSLOTS // 4        # 25088
B = 4                     # chunks per window
EPW = B * WIN             # 512
E_PAD = NWIN * EPW        # 50176
EPS = 1e-5

NW_BLK = 7
BLK = NW_BLK * EPW        # 3584
NBLK = NWIN // NW_BLK     # 14
NCHUNK = BLK // 128       # 28
NSEG = BLK // 512         # 7

# window-aligned AllGather chunk boundaries (local rows) and the scatter
# block index after which each chunk's windows are complete
AG_CHB = [0, 25 * WIN, 50 * WIN, 74 * WIN, SPC]   # 3200/3200/3072/3072 rows
AG_FIRE = [4, 8, 11]     # fire chunk k at top of block AG_FIRE[k]; last at end


_cache = {}


# ============================ host-side packing ============================

def _pack(edge_index, batch):
    src = np.asarray(edge_index[0], dtype=np.int64)
    dst = np.asarray(edge_index[1], dtype=np.int64)
    batch = np.asarray(batch, dtype=np.int64)
    cnt = np.bincount(dst, minlength=N_NODES)

    nvirt = NSLOTS - N_NODES
    cnt_all = np.concatenate([cnt, np.zeros(nvirt, dtype=cnt.dtype)])
    order = np.argsort(-cnt_all, kind="stable")
    GW = NWIN * NC
    rounds = NSLOTS // GW
    win_of_rank = np.empty(NSLOTS, dtype=np.int64)
    for r in range(rounds):
        seg = np.arange(GW) if r % 2 == 0 else np.arange(GW - 1, -1, -1)
        win_of_rank[r * GW:(r + 1) * GW] = seg
    win_of_node = np.empty(NSLOTS, dtype=np.int64)
    win_of_node[order] = win_of_rank
    wsum = np.bincount(win_of_node, weights=cnt_all.astype(np.float64),
                       minlength=GW).astype(np.int64)

    cap = EPW
    members_of = [list(np.where(win_of_node == w)[0]) for w in range(GW)]
    for _ in range(2000):
        over = np.where(wsum > cap)[0]
        if len(over) == 0:
            break
        w = int(over[0])
        # smallest-count >0 node in w
        mem = members_of[w]
        cs = [(int(cnt_all[n]), n) for n in mem if cnt_all[n] > 0]
        cs.sort()
        moved = False
        for c1, n in cs:
            # find target window with a smaller-count node to swap
            worder2 = np.argsort(wsum)
            for tw in worder2[:64]:
                tw = int(tw)
                if tw == w:
                    continue
                tmem = members_of[tw]
                best = None
                for m in tmem:
                    c2 = int(cnt_all[m])
                    if c2 < c1 and wsum[tw] + c1 - c2 <= cap:
                        if best is None or c2 < best[0]:
                            best = (c2, m)
                        if c2 == 0:
                            break
                if best is not None:
                    c2, m = best
                    members_of[tw].remove(m)
                    members_of[tw].append(n)
                    members_of[w].remove(n)
                    members_of[w].append(m)
                    win_of_node[n] = tw
                    win_of_node[m] = w
                    wsum[tw] += c1 - c2
                    wsum[w] -= c1 - c2
                    moved = True
                    break
            if moved:
                break
        if not moved:
            raise RuntimeError("packing fixup stuck")
    assert wsum.max() <= cap, f"window packing failed: max={wsum.max()}"

    worder = np.argsort(-wsum, kind="stable")
    core_load = np.zeros(NC, dtype=np.int64)
    core_nwin = np.zeros(NC, dtype=np.int64)
    core_of_win = np.empty(GW, dtype=np.int64)
    for w in worder:
        cands = np.where(core_nwin < NWIN)[0]
        c = cands[np.argmin(core_load[cands])]
        core_of_win[w] = c
        core_load[c] += wsum[w]
        core_nwin[c] += 1

    win_lists = [[] for _ in range(NC)]
    for w in range(GW):
        win_lists[core_of_win[w]].append(w)
    for c in range(NC):
        wl = win_lists[c]
        j = int(np.argmin(wsum[wl]))
        assert wsum[wl[j]] < cap, "no sentinel room"
        wl[j], wl[-1] = wl[-1], wl[j]

    slot_of_node = np.empty(NSLOTS, dtype=np.int64)
    for c in range(NC):
        for wi, w in enumerate(win_lists[c]):
            mem = np.sort(np.array(members_of[w], dtype=np.int64))
            assert len(mem) == WIN
            slot_of_node[mem] = c * SPC + wi * WIN + np.arange(WIN)
    node_of_slot = np.empty(NSLOTS, dtype=np.int64)
    node_of_slot[slot_of_node] = np.arange(NSLOTS)
    cnt_of_slot = cnt_all[node_of_slot]

    qzero = []
    for q in range(4):
        z = np.where(cnt_of_slot[q * QUAD:(q + 1) * QUAD] == 0)[0]
        assert len(z) > 0
        assert z[0] < 32768
        qzero.append(int(z[0]))  # local to quadrant
    czero = []
    for c in range(NC):
        z = np.where(cnt_of_slot[c * SPC:(c + 1) * SPC] == 0)[0]
        assert len(z) > 0
        czero.append(int(z[0]))  # local to core

    dslot = slot_of_node[dst]
    sslot = slot_of_node[src]
    ecore = dslot // SPC
    ewin = (dslot % SPC) // WIN
    key = ecore * (NWIN * WIN) + ewin * WIN + (dslot % WIN)
    eorder = np.argsort(key, kind="stable")
    dsl, ssl = dslot[eorder], sslot[eorder]
    ec, ew = ecore[eorder], ewin[eorder]

    cw = ec * NWIN + ew
    cw_cnt = np.bincount(cw, minlength=NC * NWIN)
    assert cw_cnt.max() <= EPW

    xi_idx = np.zeros((NC, E_PAD), dtype=np.int64)
    xj_idx = np.zeros((NC, E_PAD), dtype=np.int64)
    dstwin = np.full((NC, E_PAD), -1.0, dtype=np.float32)
    valid = np.zeros((NC, E_PAD), dtype=bool)

    ofs = (np.arange(NC * NWIN) % NWIN) * EPW
    start = np.concatenate([[0], np.cumsum(cw_cnt)[:-1]])
    within = np.arange(N_EDGES) - start[cw]
    pos = ofs[cw] + within
    xi_idx[ec, pos] = dsl % SPC
    xj_idx[ec, pos] = ssl
    dstwin[ec, pos] = (dsl % WIN).astype(np.float32)
    valid[ec, pos] = True
    for c in range(NC):
        xi_idx[c, ~valid[c]] = czero[c]
    pad_cnt = (~valid).sum(axis=1).astype(np.float32)
    assert np.all(~valid[:, -1]), "sentinel column must be padding"

    gzero = qzero[0]  # global slot with zero row
    xj_glob = np.where(valid, xj_idx, gzero).astype(np.int32)

    # Chunk-major AllGather table layout: local rows split into 4
    # window-aligned chunks; the full table stores [chunk][core][rows] so
    # each AG chunk output is a contiguous row block.
    sl_ = np.arange(NSLOTS)
    n_, s_ = sl_ // SPC, sl_ % SPC
    c_ = np.searchsorted(np.array(AG_CHB), s_, side="right") - 1
    sizes = np.diff(np.array(AG_CHB))
    base_full = np.concatenate([[0], np.cumsum(sizes * NC)[:-1]])
    row_of_slot = (base_full[c_] + n_ * sizes[c_]
                   + (s_ - np.array(AG_CHB)[c_]))

    inv_cnt = (1.0 / np.maximum(cnt_of_slot.reshape(NC, SPC), 1.0)).astype(np.float32)

    g_of_slot = np.full(NSLOTS, -1, dtype=np.int64)
    real = node_of_slot < N_NODES
    g_of_slot[real] = batch[node_of_slot[real]]
    NGW = 8
    Bg = 0
    pools = [[None] * NGW for _ in range(NC)]
    for c in range(NC):
        gl = g_of_slot[c * SPC:(c + 1) * SPC]
        for gw in range(NGW):
            m = np.where((gl >= gw * 128) & (gl < (gw + 1) * 128))[0]
            pools[c][gw] = m
            Bg = max(Bg, (len(m) + 127) // 128)
    NPG = Bg * 128
    pool_idx = np.zeros((NC, NGW, NPG), dtype=np.int16)
    pool_gwl = np.full((NC, NGW, NPG), -1.0, dtype=np.float32)
    for c in range(NC):
        for gw in range(NGW):
            m = pools[c][gw]
            pool_idx[c, gw, :len(m)] = m.astype(np.int16)
            pool_idx[c, gw, len(m):] = czero[c]
            pool_gwl[c, gw, :len(m)] = (g_of_slot[c * SPC + m] - gw * 128).astype(np.float32)

    gcnt = np.bincount(batch, minlength=N_GRAPHS).astype(np.float32)
    inv_g = np.zeros(1024, dtype=np.float32)
    inv_g[:N_GRAPHS] = 1.0 / np.maximum(gcnt, 1.0)

    return dict(slot_of_node=slot_of_node, node_of_slot=node_of_slot,
                row_of_slot=row_of_slot,
                xj_glob=xj_glob, dstwin=dstwin, pad_cnt=pad_cnt,
                inv_cnt=inv_cnt, valid=valid, eorder=eorder, ec=ec, pos=pos,
                pool_idx=pool_idx, pool_gwl=pool_gwl, inv_g=inv_g, Bg=Bg)


def _wrap_idx(a):
    """[.., n] int -> [.., 128, n//16]: element i -> partition i%16 col i//16,
    replicated to 8 groups of 16 partitions."""
    n = a.shape[-1]
    assert n % 16 == 0
    w = a.reshape(*a.shape[:-1], n // 16, 16)
    w = np.swapaxes(w, -1, -2)
    w = np.broadcast_to(w[..., None, :, :], (*a.shape[:-1], 8, 16, n // 16))
    return np.ascontiguousarray(w).reshape(*a.shape[:-1], 128, n // 16).astype(np.int16)


def _bf(x):
    return np.ascontiguousarray(np.asarray(x, dtype=np.float32)).astype(ml_dtypes.bfloat16)


def _tile_w(w):
    K, M = w.shape
    nk, nm = (K + 127) // 128, (M + 127) // 128
    out = np.zeros((nk, nm, 128, 128), dtype=ml_dtypes.bfloat16)
    for i in range(nk):
        for j in range(nm):
            blk = np.asarray(w, dtype=np.float32)[i * 128:(i + 1) * 128, j * 128:(j + 1) * 128]
            out[i, j, :blk.shape[0], :blk.shape[1]] = _bf(blk)
    return out


# ============================ device kernel ============================

EHALF = E_PAD // 2        # 25088
NSEG_H = EHALF // 512     # 49


def _build(Bg, debug=False, phases=4):
    nc = bacc.Bacc("TRN2", target_bir_lowering=False, debug=False, num_devices=NC)

    def din(name, shape, dt):
        return nc.dram_tensor(name, shape, dt, kind="ExternalInput").ap()

    NIDX = E_PAD // 16
    t_msgT = din("msgT", [48, EHALF], BF16)
    t_xj = din("xj_idx", [128, E_PAD // 128], mybir.dt.int32)
    t_dstwin = din("dstwin", [128, E_PAD // 128], F32)
    t_invcnt = din("invcnt", [128, NWIN], F32)
    t_padcnt = din("padcnt", [128, 1], F32)
    t_iota = din("iota", [128, 128], F32)
    t_ident = din("ident", [128, 128], BF16)
    t_c1w = din("c1w", [3, 128, 128], BF16)
    t_c1a = din("c1a", [2, 128, 1], F32)
    t_c1b = din("c1b", [3, 128, 1], F32)
    t_c1gn = din("c1gn", [3, 3, 128, 1], F32)
    t_c2wa = din("c2wa", [2, 128, 128], BF16)
    t_c2wb = din("c2wb", [2, 128, 128], BF16)
    t_c2w2 = din("c2w2", [2, 2, 128, 128], BF16)
    t_c2b = din("c2b", [2, 2, 128, 1], F32)
    t_c2gn = din("c2gn", [2, 3, 2, 128, 1], F32)
    t_c3wa = din("c3wa", [2, 2, 128, 128], BF16)
    t_c3wb = din("c3wb", [2, 2, 128, 128], BF16)
    t_c3b = din("c3b", [2, 128, 1], F32)
    t_c3gn = din("c3gn", [3, 2, 128, 1], F32)
    t_lw1 = din("lw1", [2, 2, 128, 128], BF16)
    t_lb1 = din("lb1", [2, 128, 1], F32)
    t_lw2 = din("lw2", [2, 128, 2], BF16)
    t_lb2 = din("lb2", [2, 1], F32)
    t_pidx16 = din("pidx16", [128, 8 * Bg * 128 // 16], I16)
    t_pgwl = din("pool_gwl", [128, 8 * Bg], F32)
    t_invg = din("invg", [128, 8], F32)

    o_out = nc.dram_tensor("out", [2, N_GRAPHS], F32, kind="ExternalOutput").ap()
    dbg = {}
    if debug:
        dbg["x1"] = nc.dram_tensor("dbg_x1", [NSLOTS, 128], BF16, kind="ExternalOutput").ap()
        dbg["x2"] = nc.dram_tensor("dbg_x2", [NSLOTS, 256], BF16, kind="ExternalOutput").ap()
        dbg["x3"] = nc.dram_tensor("dbg_x3", [SPC, 256], BF16, kind="ExternalOutput").ap()
        dbg["pool"] = nc.dram_tensor("dbg_pool", [1024, 256], F32, kind="ExternalOutput").ap()

    with tile.TileContext(nc) as tc:
        with tc.tile_pool(name="dram", bufs=1, space="DRAM") as dram, \
             tc.tile_pool(name="cp", bufs=1) as cp:
            z_scr = [dram.tile([2, 128, E_PAD], BF16, tag=f"zscr{i}", name=f"zscr{i}") for i in range(2)]
            tab1_loc = dram.tile([SPC, 128], BF16)
            tab1 = dram.tile([NSLOTS, 128], BF16)
            tab2_loc = dram.tile([SPC, 256], BF16)
            tab2 = dram.tile([NSLOTS, 256], BF16)
            tab3_loc = dram.tile([SPC, 256], BF16)
            st_in = dram.tile([128, 8], F32)
            st_out = dram.tile([128, 8], F32)
            pool_in = dram.tile([1024, 256], F32)
            pool_out = dram.tile([1024, 256], F32)

            ident = cp.tile([128, 128], BF16)
            nc.sync.dma_start(ident[:], t_ident[:])
            iota = cp.tile([128, 128], F32)
            nc.sync.dma_start(iota[:], t_iota[:])
            invcnt = cp.tile([128, NWIN], F32)
            nc.sync.dma_start(invcnt[:], t_invcnt[:])
            dwin = cp.tile([128, E_PAD // 128], F32)
            nc.sync.dma_start(dwin[:], t_dstwin[:])
            padcnt = cp.tile([128, 1], F32)
            nc.sync.dma_start(padcnt[:], t_padcnt[:])

            # ---------- helpers ----------
            def allreduce_stats(s_acc, q_acc, n_mb, sb):
                st = sb.tile([128, 8], F32, tag="st_")
                nc.vector.memset(st[:], 0.0)
                nc.vector.tensor_copy(st[:, 0:n_mb], s_acc[:])
                nc.vector.tensor_copy(st[:, 4:4 + n_mb], q_acc[:])
                nc.sync.dma_start(st_in[:], st[:])
                nc.gpsimd.collective_compute(
                    "AllReduce", AOP.add, replica_groups=[list(range(NC))],
                    ins=[st_in.opt()], outs=[st_out.opt()])
                stg = sb.tile([128, 8], F32, tag="stg_")
                nc.sync.dma_start(stg[:], st_out[:])
                return stg

            def affine_from_stats(stg, n_mb, b_lin, gn, sb):
                A, Cc = [], []
                for mb in range(n_mb):
                    s = stg[:, mb:mb + 1]
                    q = stg[:, 4 + mb:5 + mb]
                    g, bgn, ms = gn[0][mb], gn[1][mb], gn[2][mb]
                    bl = b_lin[mb]
                    m = sb.tile([128, 1], F32, tag="af_m")
                    nc.vector.tensor_scalar(m[:], s, 1.0 / N_EDGES, None, AOP.mult)
                    nc.vector.tensor_tensor(m[:], m[:], bl, op=AOP.add)
                    e2 = sb.tile([128, 1], F32, tag="af_e2")
                    nc.vector.tensor_scalar(e2[:], q, 1.0 / N_EDGES, None, AOP.mult)
                    tmp = sb.tile([128, 1], F32, tag="af_t")
                    nc.vector.tensor_tensor(tmp[:], m[:], bl, op=AOP.mult)
                    nc.vector.tensor_scalar(tmp[:], tmp[:], 2.0, None, AOP.mult)
                    nc.vector.tensor_tensor(e2[:], e2[:], tmp[:], op=AOP.add)
                    nc.vector.tensor_tensor(tmp[:], bl, bl, op=AOP.mult)
                    nc.vector.tensor_tensor(e2[:], e2[:], tmp[:], op=AOP.subtract)
                    msm = sb.tile([128, 1], F32, tag="af_msm")
                    nc.vector.tensor_tensor(msm[:], ms, m[:], op=AOP.mult)
                    var = sb.tile([128, 1], F32, tag="af_v")
                    nc.vector.tensor_tensor(var[:], msm[:], msm[:], op=AOP.mult)
                    nc.vector.tensor_tensor(tmp[:], msm[:], m[:], op=AOP.mult)
                    nc.vector.tensor_scalar(tmp[:], tmp[:], 2.0, None, AOP.mult)
                    nc.vector.tensor_tensor(var[:], var[:], tmp[:], op=AOP.subtract)
                    nc.vector.tensor_tensor(var[:], var[:], e2[:], op=AOP.add)
                    a = sb.tile([128, 1], F32, tag="af_a")
                    nc.vector.tensor_scalar(var[:], var[:], EPS, None, AOP.add)
                    nc.scalar.activation(a[:], var[:], AFT.Sqrt)
                    nc.vector.reciprocal(a[:], a[:])
                    nc.vector.tensor_tensor(a[:], a[:], g, op=AOP.mult)
                    cc = sb.tile([128, 1], F32, tag="af_c")
                    nc.vector.tensor_tensor(cc[:], bl, msm[:], op=AOP.subtract)
                    nc.vector.tensor_tensor(cc[:], cc[:], a[:], op=AOP.mult)
                    nc.vector.tensor_tensor(cc[:], cc[:], bgn, op=AOP.add)
                    A.append(a)
                    Cc.append(cc)
                return A, Cc

            def acc_stats(ps_ap, s_col, q_col, sb):
                t1 = sb.tile([128, 1], F32, tag="rs_t1")
                nc.vector.reduce_sum(out=t1[:], in_=ps_ap, axis=AX.X)
                nc.vector.tensor_tensor(s_col, s_col, t1[:], op=AOP.add)
                n = ps_ap.shape[-1]
                sq = sb.tile([128, 512], BF16, tag="rs_sq")
                qa = sb.tile([128, 1], F32, tag="rs_qa")
                nc.scalar.activation(sq[:, :n], ps_ap, AFT.Square, accum_out=qa[:])
                nc.vector.tensor_tensor(q_col, q_col, qa[:], op=AOP.add)

            def bn_finish(st, s_col, q_col, sb, tag):
                # bn_stats 6-tuples (equal 512-col groups) -> sum / sq-sum
                agg = sb.tile([128, 2], F32, tag=tag + "g")
                nc.vector.bn_aggr(agg[:], st[:])
                nc.vector.tensor_scalar(s_col, agg[:, 0:1], float(E_PAD),
                                        None, AOP.mult)
                t = sb.tile([128, 1], F32, tag=tag + "t")
                nc.vector.tensor_tensor(t[:], agg[:, 0:1], agg[:, 0:1], op=AOP.mult)
                nc.vector.tensor_tensor(t[:], t[:], agg[:, 1:2], op=AOP.add)
                nc.vector.tensor_scalar(q_col, t[:], float(E_PAD), None, AOP.mult)

            def sentinel_correct(s_acc, q_acc, zsent_cols, n_mb, sb):
                for mb in range(n_mb):
                    zs = zsent_cols[mb]
                    t1 = sb.tile([128, 1], F32, tag="sc_t1")
                    nc.vector.tensor_tensor(t1[:], zs, padcnt[:], op=AOP.mult)
                    nc.vector.tensor_tensor(s_acc[:, mb:mb + 1], s_acc[:, mb:mb + 1],
                                            t1[:], op=AOP.subtract)
                    nc.vector.tensor_tensor(t1[:], zs, zs, op=AOP.mult)
                    nc.vector.tensor_tensor(t1[:], t1[:], padcnt[:], op=AOP.mult)
                    nc.vector.tensor_tensor(q_acc[:, mb:mb + 1], q_acc[:, mb:mb + 1],
                                            t1[:], op=AOP.subtract)

            def load_vec(t_ap, sb, tag):
                v = sb.tile([128, 1], F32, tag=tag)
                nc.sync.dma_start(v[:], t_ap)
                return v[:]

            AG_BASE = [0]
            for _c in range(3):
                AG_BASE.append(AG_BASE[-1] + NC * (AG_CHB[_c + 1] - AG_CHB[_c]))

            def fire_ag(tab_loc, tab_full, c):
                # chunk-major table: AG chunk c is a contiguous row block
                lo, hi = AG_CHB[c], AG_CHB[c + 1]
                nc.gpsimd.collective_compute(
                    "AllGather", AOP.bypass, replica_groups=[list(range(NC))],
                    ins=[tab_loc[lo:hi, :].opt()],
                    outs=[tab_full[AG_BASE[c]:AG_BASE[c] + NC * (hi - lo),
                                   :].opt()])

            def scatter_pass(zsrc, n_mb, A, Cc, tab_loc, Cout, ag=None):
                with tc.tile_pool(name="sc_sb", bufs=2) as sb, \
                     tc.tile_pool(name="sc_tp", bufs=2, space="PSUM") as ps_tp, \
                     tc.tile_pool(name="sc_sc", bufs=2, space="PSUM") as ps_sc:
                    for b in range(NBLK):
                        if ag is not None and b in AG_FIRE:
                            fire_ag(tab_loc, ag, AG_FIRE.index(b))
                        hs = []
                        for mb in range(n_mb):
                            z = sb.tile([128, BLK], BF16, tag=f"sp_z{mb}")
                            nc.sync.dma_start(z[:], zsrc[mb, :, b * BLK:(b + 1) * BLK])
                            h = sb.tile([128, BLK], BF16, tag=f"sp_h{mb}")
                            nc.scalar.activation(h[:], z[:], AFT.Relu,
                                                 bias=Cc[mb], scale=A[mb])
                            hs.append(h)
                        hE = sb.tile([128, NCHUNK * Cout], BF16, tag="sp_hE")
                        for ch in range(NCHUNK):
                            for mb in range(n_mb):
                                tp = ps_tp.tile([128, 128], BF16, tag="sp_tp", space="PSUM")
                                nc.tensor.transpose(tp[:], hs[mb][:, ch * 128:(ch + 1) * 128],
                                                    ident[:])
                                nc.vector.tensor_copy(
                                    hE[:, ch * Cout + mb * 128:ch * Cout + (mb + 1) * 128],
                                    tp[:])
                        for w in range(NW_BLK):
                            gw = b * NW_BLK + w
                            sc = ps_sc.tile([128, Cout], F32, tag="sp_sc", space="PSUM")
                            for cb in range(B):
                                ch = w * B + cb
                                col = b * NCHUNK + ch
                                oh = sb.tile([128, 128], BF16, tag="sp_oh")
                                nc.vector.tensor_tensor(
                                    out=oh[:],
                                    in0=dwin[:, col:col + 1].to_broadcast([128, 128]),
                                    in1=iota[:], op=AOP.is_equal)
                                nc.tensor.matmul(sc[:], oh[:],
                                                 hE[:, ch * Cout:(ch + 1) * Cout],
                                                 start=(cb == 0), stop=(cb == B - 1))
                            nt = sb.tile([128, Cout], BF16, tag="sp_nt")
                            nc.vector.tensor_scalar(nt[:], sc[:], invcnt[:, gw:gw + 1],
                                                    None, AOP.mult)
                            nc.sync.dma_start(tab_loc[gw * WIN:(gw + 1) * WIN, :], nt[:])
                    if ag is not None:
                        fire_ag(tab_loc, ag, 3)

            # ======================= CONV 1 =======================
            # SBUF-resident: L1 stats precomputed on host; h kept on-chip,
            # L2 overwrites it in place; L3 fused with the scatter.
            NSEG_T = E_PAD // 512  # 98
            with tc.tile_pool(name="c1sb", bufs=2) as sb:
                c1b = [[load_vec(t_c1b[i], sb, f"c1b{i}")] for i in range(3)]
                c1gn = [[[load_vec(t_c1gn[i, j], sb, f"c1gn{i}{j}")] for j in range(3)]
                        for i in range(3)]
                A1h = load_vec(t_c1a[0], sb, "c1a0")
                C1h = load_vec(t_c1a[1], sb, "c1a1")
                with tc.tile_pool(name="c1h", bufs=1) as hp, \
                     tc.tile_pool(name="c1ps", bufs=2, space="PSUM") as ps, \
                     tc.tile_pool(name="c1p2", bufs=2, space="PSUM") as ps2:
                    c1w = []
                    for i in range(3):
                        w = sb.tile([128, 128], BF16, tag=f"c1w{i}")
                        nc.sync.dma_start(w[:], t_c1w[i])
                        c1w.append(w)
                    msgT = hp.tile([48, EHALF], BF16, tag="msgT")
                    nc.sync.dma_start(msgT[:], t_msgT[:])
                    h_full = hp.tile([128, E_PAD], BF16)

                    def bn_to_sq(st, tag):
                        agg = sb.tile([128, 2], F32, tag=tag + "agg")
                        nc.vector.bn_aggr(agg[:], st[:])
                        s_acc = sb.tile([128, 1], F32, tag=tag + "s")
                        q_acc = sb.tile([128, 1], F32, tag=tag + "q")
                        nc.vector.tensor_scalar(s_acc[:], agg[:, 0:1],
                                                float(E_PAD), None, AOP.mult)
                        nc.vector.tensor_tensor(q_acc[:], agg[:, 0:1], agg[:, 0:1],
                                                op=AOP.mult)
                        nc.vector.tensor_tensor(q_acc[:], q_acc[:], agg[:, 1:2],
                                                op=AOP.add)
                        nc.vector.tensor_scalar(q_acc[:], q_acc[:],
                                                float(E_PAD), None, AOP.mult)
                        return s_acc, q_acc

                    # pass 1: L1 -> h_full; L2 stats
                    st2 = hp.tile([128, NSEG_T * 6], F32, tag="st2")
                    zs2 = sb.tile([128, 1], F32, tag="zs2")
                    for g in range(NSEG_T):
                        hh, shalf = g // NSEG_H, g % NSEG_H
                        zp = ps.tile([128, 512], F32, tag="zp")
                        nc.tensor.matmul(zp[:], c1w[0][32 * hh:32 * hh + 10, :],
                                         msgT[32 * hh:32 * hh + 10,
                                              shalf * 512:(shalf + 1) * 512],
                                         start=True, stop=True)
                        nc.scalar.activation(h_full[:, g * 512:(g + 1) * 512], zp[:],
                                             AFT.Relu, bias=C1h, scale=A1h)
                        zp2 = ps2.tile([128, 512], F32, tag="zp2")
                        nc.tensor.matmul(zp2[:], c1w[1][:],
                                         h_full[:, g * 512:(g + 1) * 512],
                                         start=True, stop=True)
                        nc.vector.bn_stats(st2[:, g * 6:(g + 1) * 6], zp2[:])
                        if g == NSEG_T - 1:
                            nc.vector.tensor_copy(zs2[:], zp2[:, 511:512])
                    s2, q2 = bn_to_sq(st2, "b2")
                    sentinel_correct(s2, q2, [zs2[:]], 1, sb)
                    stg2 = allreduce_stats(s2, q2, 1, sb)
                    A2, C2 = affine_from_stats(stg2, 1, c1b[1], c1gn[1], sb)

                    # pass 2: L2 -> h_full (in place); L3 stats
                    st3 = hp.tile([128, NSEG_T * 6], F32, tag="st3")
                    zs3 = sb.tile([128, 1], F32, tag="zs3")
                    for g in range(NSEG_T):
                        zp = ps.tile([128, 512], F32, tag="zp")
                        nc.tensor.matmul(zp[:], c1w[1][:],
                                         h_full[:, g * 512:(g + 1) * 512],
                                         start=True, stop=True)
                        nc.scalar.activation(h_full[:, g * 512:(g + 1) * 512], zp[:],
                                             AFT.Relu, bias=C2[0], scale=A2[0])
                        zp3 = ps2.tile([128, 512], F32, tag="zp2")
                        nc.tensor.matmul(zp3[:], c1w[2][:],
                                         h_full[:, g * 512:(g + 1) * 512],
                                         start=True, stop=True)
                        nc.vector.bn_stats(st3[:, g * 6:(g + 1) * 6], zp3[:])
                        if g == NSEG_T - 1:
                            nc.vector.tensor_copy(zs3[:], zp3[:, 511:512])
                    s3, q3 = bn_to_sq(st3, "b3")
                    sentinel_correct(s3, q3, [zs3[:]], 1, sb)
                    stg3 = allreduce_stats(s3, q3, 1, sb)
                    A3, C3 = affine_from_stats(stg3, 1, c1b[2], c1gn[2], sb)

                    # pass 3: L3 + fused scatter
                    with tc.tile_pool(name="c1sc", bufs=2) as scb, \
                         tc.tile_pool(name="c1tp", bufs=2, space="PSUM") as ps_tp, \
                         tc.tile_pool(name="c1s2", bufs=2, space="PSUM") as ps_sc:
                        for b in range(NBLK):
                            if b in AG_FIRE:
                                fire_ag(tab1_loc, tab1, AG_FIRE.index(b))
                            h3 = scb.tile([128, BLK], BF16, tag="c1h3")
                            for s in range(NSEG):
                                g = b * NSEG + s
                                zp = ps.tile([128, 512], F32, tag="zp")
                                nc.tensor.matmul(zp[:], c1w[2][:],
                                                 h_full[:, g * 512:(g + 1) * 512],
                                                 start=True, stop=True)
                                nc.scalar.activation(h3[:, s * 512:(s + 1) * 512],
                                                     zp[:], AFT.Relu,
                                                     bias=C3[0], scale=A3[0])
                            hE = scb.tile([128, NCHUNK * 128], BF16, tag="c1hE")
                            for ch in range(NCHUNK):
                                tp = ps_tp.tile([128, 128], BF16, tag="c1tp",
                                                space="PSUM")
                                nc.tensor.transpose(tp[:], h3[:, ch * 128:(ch + 1) * 128],
                                                    ident[:])
                                nc.vector.tensor_copy(hE[:, ch * 128:(ch + 1) * 128],
                                                      tp[:])
                            for w in range(NW_BLK):
                                gw = b * NW_BLK + w
                                sc = ps_sc.tile([128, 128], F32, tag="c1sc",
                                                space="PSUM")
                                for cb in range(B):
                                    ch = w * B + cb
                                    col = b * NCHUNK + ch
                                    oh = scb.tile([128, 128], BF16, tag="c1oh")
                                    nc.vector.tensor_tensor(
                                        out=oh[:],
                                        in0=dwin[:, col:col + 1].to_broadcast([128, 128]),
                                        in1=iota[:], op=AOP.is_equal)
                                    nc.tensor.matmul(sc[:], oh[:],
                                                     hE[:, ch * 128:(ch + 1) * 128],
                                                     start=(cb == 0), stop=(cb == B - 1))
                                nt = scb.tile([128, 128], BF16, tag="c1nt")
                                nc.vector.tensor_scalar(nt[:], sc[:],
                                                        invcnt[:, gw:gw + 1],
                                                        None, AOP.mult)
                                nc.sync.dma_start(tab1_loc[gw * WIN:(gw + 1) * WIN, :],
                                                  nt[:])
                        fire_ag(tab1_loc, tab1, 3)

            if debug:
                nc.sync.dma_start(dbg["x1"][:], tab1[:])

            # ============== gather-based first layer (conv2/conv3) ==============
            def gather_layer(tab_full, tab_loc, Cin, wa_t, wb_t, n_kb, zdst, sb):
                mb_in = Cin // 128
                s_acc = sb.tile([128, 2], F32, tag="gl_s")
                q_acc = sb.tile([128, 2], F32, tag="gl_q")
                sts = [sb.tile([128, (E_PAD // 512) * 6], F32, tag=f"gl_st{mo}",
                               name=f"gl_st{mo}")
                       for mo in range(2)]
                with tc.tile_pool(name="gl_g2", bufs=3) as g2, \
                     tc.tile_pool(name="gl_g1", bufs=2) as g1, \
                     tc.tile_pool(name="gl_zw", bufs=2) as zwp, \
                     tc.tile_pool(name="gl_ps", bufs=2, space="PSUM") as ps, \
                     tc.tile_pool(name="gl_tp", bufs=2, space="PSUM") as ps_tp, \
                     tc.tile_pool(name="gl_xp", bufs=2, space="PSUM") as ps_xp:
                    was, wbs = [], []
                    for ki in range(n_kb):
                        for mo in range(2):
                            wta = sb.tile([128, 128], BF16, tag=f"gl_wa{ki}{mo}")
                            nc.sync.dma_start(wta[:], wa_t[ki, mo] if n_kb > 1 else wa_t[mo])
                            was.append(wta)
                            wtb = sb.tile([128, 128], BF16, tag=f"gl_wb{ki}{mo}")
                            nc.sync.dma_start(wtb[:], wb_t[ki, mo] if n_kb > 1 else wb_t[mo])
                            wbs.append(wtb)
                    for b in range(NBLK):
                        ixj = g2.tile([128, NCHUNK], mybir.dt.int32, tag="gl_ixj")
                        nc.sync.dma_start(ixj[:], t_xj[:, b * NCHUNK:(b + 1) * NCHUNK])
                        gxj = g2.tile([128, NCHUNK * Cin], BF16, tag="gl_gxj")
                        for ch in range(NCHUNK):
                            nc.gpsimd.indirect_dma_start(
                                out=gxj[:, ch * Cin:(ch + 1) * Cin],
                                out_offset=None,
                                in_=tab_full[:],
                                in_offset=bass.IndirectOffsetOnAxis(
                                    ap=ixj[:, ch:ch + 1], axis=0))
                        xjT = g1.tile([128, mb_in * BLK], BF16, tag="gl_xjT")
                        for ch in range(NCHUNK):
                            for kb in range(mb_in):
                                tp2 = ps_tp.tile([128, 128], BF16, tag="gl_ohp",
                                                 space="PSUM")
                                nc.tensor.transpose(
                                    tp2[:],
                                    gxj[:, ch * Cin + kb * 128:ch * Cin + (kb + 1) * 128],
                                    ident[:])
                                nc.vector.tensor_copy(
                                    xjT[:, kb * BLK + ch * 128:kb * BLK + (ch + 1) * 128],
                                    tp2[:])
                        # xi via window expansion
                        xiT = g1.tile([128, mb_in * BLK], BF16, tag="gl_xiT")
                        for w in range(NW_BLK):
                            gw = b * NW_BLK + w
                            twin = g2.tile([128, Cin], BF16, tag="gl_twin")
                            nc.sync.dma_start(twin[:], tab_loc[gw * WIN:(gw + 1) * WIN, :])
                            for cb in range(B):
                                ch = w * B + cb
                                col = b * NCHUNK + ch
                                oh = g2.tile([128, 128], BF16, tag="gl_oh")
                                nc.vector.tensor_tensor(
                                    out=oh[:],
                                    in0=dwin[:, col:col + 1].to_broadcast([128, 128]),
                                    in1=iota[:], op=AOP.is_equal)
                                ohp = ps_tp.tile([128, 128], BF16, tag="gl_ohp", space="PSUM")
                                nc.tensor.transpose(ohp[:], oh[:], ident[:])
                                oh2 = g2.tile([128, 128], BF16, tag="gl_oh2")
                                nc.vector.tensor_copy(oh2[:], ohp[:])
                                for kb in range(mb_in):
                                    xp = ps_xp.tile([128, 128], F32, tag="gl_xp", space="PSUM")
                                    nc.tensor.matmul(xp[:], twin[:, kb * 128:(kb + 1) * 128],
                                                     oh2[:], start=True, stop=True)
                                    nc.vector.tensor_copy(
                                        xiT[:, kb * BLK + ch * 128:kb * BLK + (ch + 1) * 128],
                                        xp[:])
                        for mo in range(2):
                            zw = zwp.tile([128, BLK], BF16, tag=f"gl_z{mo}")
                            for sg in range(NSEG):
                                g6 = (b * NSEG + sg) * 6
                                zp = ps.tile([128, 512], F32, tag="gl_zp")
                                for ki in range(mb_in):
                                    nc.tensor.matmul(
                                        zp[:], was[ki * 2 + mo][:],
                                        xiT[:, ki * BLK + sg * 512:ki * BLK + (sg + 1) * 512],
                                        start=(ki == 0), stop=False)
                                for ki in range(mb_in):
                                    nc.tensor.matmul(
                                        zp[:], wbs[ki * 2 + mo][:],
                                        xjT[:, ki * BLK + sg * 512:ki * BLK + (sg + 1) * 512],
                                        start=False, stop=(ki == mb_in - 1))
                                nc.vector.bn_stats(sts[mo][:, g6:g6 + 6], zp[:])
                                nc.scalar.copy(zw[:, sg * 512:(sg + 1) * 512], zp[:])
                            nc.sync.dma_start(zdst[mo, :, b * BLK:(b + 1) * BLK], zw[:])
                for mo in range(2):
                    bn_finish(sts[mo], s_acc[:, mo:mo + 1], q_acc[:, mo:mo + 1],
                              sb, f"glf{mo}")
                return s_acc, q_acc

            # ======================= CONV 2 =======================
            if phases >= 2:
              with tc.tile_pool(name="c2sb", bufs=2) as sb:
                  c2b = [[load_vec(t_c2b[i, mb], sb, f"c2b{i}{mb}") for mb in range(2)]
                         for i in range(2)]
                  c2gn = [[[load_vec(t_c2gn[i, j, mb], sb, f"c2gn{i}{j}{mb}")
                            for mb in range(2)] for j in range(3)] for i in range(2)]
                  sA, qA = gather_layer(tab1, tab1_loc, 128, t_c2wa, t_c2wb, 1,
                                        z_scr[0], sb)
                  stg = allreduce_stats(sA, qA, 2, sb)
                  A1, C1 = affine_from_stats(stg, 2, c2b[0], c2gn[0], sb)

                  s2 = sb.tile([128, 2], F32, tag="c2s2")
                  q2 = sb.tile([128, 2], F32, tag="c2q2")
                  st2s = [sb.tile([128, (E_PAD // 512) * 6], F32, tag=f"c2st{mo}",
                                  name=f"c2st{mo}")
                          for mo in range(2)]
                  zlast = [None, None]
                  with tc.tile_pool(name="c2mid", bufs=2) as mp, \
                       tc.tile_pool(name="c2ps", bufs=2, space="PSUM") as ps:
                      w2s = []
                      for ki in range(2):
                          for mo in range(2):
                              w = sb.tile([128, 128], BF16, tag=f"c2w2{ki}{mo}")
                              nc.sync.dma_start(w[:], t_c2w2[ki, mo])
                              w2s.append(w)
                      for b in range(NBLK):
                          h1 = []
                          for mb in range(2):
                              z = mp.tile([128, BLK], BF16, tag=f"c2z1r{mb}")
                              nc.sync.dma_start(z[:], z_scr[0][mb, :, b * BLK:(b + 1) * BLK])
                              hh = mp.tile([128, BLK], BF16, tag=f"c2h1{mb}")
                              nc.scalar.activation(hh[:], z[:], AFT.Relu,
                                                   bias=C1[mb], scale=A1[mb])
                              h1.append(hh)
                          for mo in range(2):
                              zw = mp.tile([128, BLK], BF16, tag=f"c2z2w{mo}")
                              for s in range(NSEG):
                                  g6 = (b * NSEG + s) * 6
                                  zp = ps.tile([128, 512], F32, tag="c2zp")
                                  for ki in range(2):
                                      nc.tensor.matmul(zp[:], w2s[ki * 2 + mo][:],
                                                       h1[ki][:, s * 512:(s + 1) * 512],
                                                       start=(ki == 0), stop=(ki == 1))
                                  nc.vector.bn_stats(st2s[mo][:, g6:g6 + 6], zp[:])
                                  nc.scalar.copy(zw[:, s * 512:(s + 1) * 512], zp[:])
                              nc.sync.dma_start(z_scr[1][mo, :, b * BLK:(b + 1) * BLK], zw[:])
                              zlast[mo] = zw
                      zsent = []
                      for mo in range(2):
                          zc = sb.tile([128, 1], F32, tag=f"c2zs{mo}")
                          nc.vector.tensor_copy(zc[:], zlast[mo][:, BLK - 1:BLK])
                          zsent.append(zc[:])
                  for mo in range(2):
                      bn_finish(st2s[mo], s2[:, mo:mo + 1], q2[:, mo:mo + 1],
                                sb, f"c2f{mo}")
                  sentinel_correct(s2, q2, zsent, 2, sb)
                  stg2 = allreduce_stats(s2, q2, 2, sb)
                  A2, C2 = affine_from_stats(stg2, 2, c2b[1], c2gn[1], sb)
                  scatter_pass(z_scr[1], 2, A2, C2, tab2_loc, 256, ag=tab2)

            if debug:
                nc.sync.dma_start(dbg["x2"][:], tab2[:])

            # ======================= CONV 3 =======================
            if phases >= 3:
              with tc.tile_pool(name="c3sb", bufs=2) as sb:
                  c3b = [load_vec(t_c3b[mb], sb, f"c3b{mb}") for mb in range(2)]
                  c3gn = [[load_vec(t_c3gn[j, mb], sb, f"c3gn{j}{mb}") for mb in range(2)]
                          for j in range(3)]
                  sA, qA = gather_layer(tab2, tab2_loc, 256, t_c3wa, t_c3wb, 2,
                                        z_scr[0], sb)
                  stg = allreduce_stats(sA, qA, 2, sb)
                  A1, C1 = affine_from_stats(stg, 2, c3b, c3gn, sb)
                  scatter_pass(z_scr[0], 2, A1, C1, tab3_loc, 256)

            if debug:
                nc.sync.dma_start(dbg["x3"][:], tab3_loc[:])

            # ======================= POOL + HEAD =======================
            if phases >= 4:
              with tc.tile_pool(name="p_sb", bufs=2) as sb, \
                 tc.tile_pool(name="p_ps", bufs=2, space="PSUM") as ps:
                  pgwl = sb.tile([128, 8 * Bg], F32, tag="p_pgwl")
                  nc.sync.dma_start(pgwl[:], t_pgwl[:])
                  NPG = Bg * 128
                  pidxw = sb.tile([128, 8 * NPG // 16], I16, tag="p_idx16")
                  nc.sync.dma_start(pidxw[:], t_pidx16[:])
                  for gw in range(8):
                      gp = sb.tile([128, Bg, 256], BF16, tag="p_gp")
                      nc.gpsimd.dma_gather(
                          out_ap=gp[:], in_ap=tab3_loc[:],
                          idxs_ap=pidxw[:, gw * (NPG // 16):(gw + 1) * (NPG // 16)],
                          num_idxs=NPG, num_idxs_reg=NPG, elem_size=256,
                          transpose=False, single_packet=(NPG <= 896))
                      pp = ps.tile([128, 256], F32, tag="p_pp", space="PSUM")
                      for c in range(Bg):
                          oh = sb.tile([128, 128], BF16, tag="p_oh")
                          nc.vector.tensor_tensor(
                              out=oh[:],
                              in0=pgwl[:, gw * Bg + c:gw * Bg + c + 1].to_broadcast([128, 128]),
                              in1=iota[:], op=AOP.is_equal)
                          nc.tensor.matmul(pp[:], oh[:], gp[:, c, :],
                                           start=(c == 0), stop=(c == Bg - 1))
                      pf = sb.tile([128, 256], F32, tag="p_pf")
                      nc.vector.tensor_copy(pf[:], pp[:])
                      nc.sync.dma_start(pool_in[gw * 128:(gw + 1) * 128, :], pf[:])
                  nc.gpsimd.collective_compute(
                      "AllReduce", AOP.add, replica_groups=[list(range(NC))],
                      ins=[pool_in.opt()], outs=[pool_out.opt()])
                  if debug:
                      nc.sync.dma_start(dbg["pool"][:], pool_out[:])

                  invg = sb.tile([128, 8], F32, tag="p_invg")
                  nc.sync.dma_start(invg[:], t_invg[:])
                  lw1 = []
                  for ki in range(2):
                      for mo in range(2):
                          w = sb.tile([128, 128], BF16, tag=f"p_lw1{ki}{mo}")
                          nc.sync.dma_start(w[:], t_lw1[ki, mo])
                          lw1.append(w)
                  lw2 = []
                  for ki in range(2):
                      w = sb.tile([128, 2], BF16, tag=f"p_lw2{ki}")
                      nc.sync.dma_start(w[:], t_lw2[ki])
                      lw2.append(w)
                  lb1 = [load_vec(t_lb1[mb], sb, f"p_lb1{mb}") for mb in range(2)]
                  lb2 = sb.tile([2, 1], F32, tag="p_lb2")
                  nc.sync.dma_start(lb2[:], t_lb2[:])
                  ofin = sb.tile([2, 1024], F32, tag="p_out")
                  for gw in range(8):
                      g = sb.tile([128, 256], F32, tag="p_g")
                      nc.sync.dma_start(g[:], pool_out[gw * 128:(gw + 1) * 128, :])
                      gm = sb.tile([128, 256], BF16, tag="p_gm")
                      nc.vector.tensor_scalar(gm[:], g[:], invg[:, gw:gw + 1], None, AOP.mult)
                      gT = sb.tile([128, 2 * 128], BF16, tag="p_gT")
                      for kb in range(2):
                          tp = ps.tile([128, 128], BF16, tag="p_tp", space="PSUM")
                          nc.tensor.transpose(tp[:], gm[:, kb * 128:(kb + 1) * 128], ident[:])
                          nc.vector.tensor_copy(gT[:, kb * 128:(kb + 1) * 128], tp[:])
                      hT = sb.tile([128, 2 * 128], BF16, tag="p_hT")
                      for mo in range(2):
                          hp = ps.tile([128, 128], F32, tag="p_hp", space="PSUM")
                          for ki in range(2):
                              nc.tensor.matmul(hp[:], lw1[ki * 2 + mo][:],
                                               gT[:, ki * 128:(ki + 1) * 128],
                                               start=(ki == 0), stop=(ki == 1))
                          nc.scalar.activation(hT[:, mo * 128:(mo + 1) * 128], hp[:],
                                               AFT.Relu, bias=lb1[mo])
                      op_ = ps.tile([2, 128], F32, tag="p_op", space="PSUM")
                      for ki in range(2):
                          nc.tensor.matmul(op_[:], lw2[ki][:],
                                           hT[:, ki * 128:(ki + 1) * 128],
                                           start=(ki == 0), stop=(ki == 1))
                      nc.vector.tensor_scalar(ofin[:, gw * 128:(gw + 1) * 128],
                                              op_[:], lb2[:], None, AOP.add)
                  nc.sync.dma_start(o_out[:], ofin[:, :N_GRAPHS])

    nc.compile()
    return nc


# ============================ entry point ============================


def kernel(**inputs):
    x = np.asarray(inputs["x"], dtype=np.float32)
    edge_index = np.asarray(inputs["edge_index"])
    batch = np.asarray(inputs["batch"])

    meta = _pack(edge_index, batch)
    Bg = meta["Bg"]

    import os as _os
    phases = int(_os.environ.get("KPHASES", "4"))
    key = ("mod", Bg, phases, _DEBUG[0])
    if key not in _cache:
        _cache[key] = _build(Bg, debug=bool(inputs.get("_debug", False)) or _DEBUG[0],
                             phases=phases)
    nc = _cache[key]

    # ---- per-core input arrays ----
    slot_of_node = meta["slot_of_node"]
    src = np.asarray(edge_index[0], dtype=np.int64)
    dst = np.asarray(edge_index[1], dtype=np.int64)

    # conv1 msgT: [core, 20, E_PAD//2] bf16; edge e<EHALF -> rows 0..9 col e,
    # e>=EHALF -> rows 10..19 col e-EHALF
    EHALF = E_PAD // 2
    xi_v = x[dst]
    xj_v = x[src]
    msg = np.concatenate([xi_v, xj_v - xi_v], axis=1)       # [E, 10]

    # exact conv1-L1 GraphNorm stats on host (tiny 10-dim Gram)
    msg64 = msg.astype(np.float64)
    W1 = np.asarray(inputs["c1_w1"], np.float64)            # [10, 128]
    b1 = np.asarray(inputs["c1_b1"], np.float64)            # [128]
    S = msg64.sum(0)
    G = msg64.T @ msg64
    SW = S @ W1
    qz = np.einsum('ij,ik,kj->j', W1, G, W1) + 2 * b1 * SW + N_EDGES * b1 * b1
    m1 = (SW + N_EDGES * b1) / N_EDGES
    e2 = qz / N_EDGES
    gn1 = np.asarray(inputs["c1_gn1"], np.float64)          # [3, 128]
    msm = gn1[2] * m1
    var1 = e2 - 2 * msm * m1 + msm * msm
    A1h = gn1[0] / np.sqrt(var1 + EPS)
    C1h = gn1[1] + A1h * (b1 - msm)
    c1a_in = np.stack([A1h, C1h]).astype(np.float32).reshape(2, 128, 1)

    msg_full = np.zeros((NC, E_PAD, 10), dtype=np.float32)
    ec, pos = meta["ec"], meta["pos"]
    msg_full[ec, pos] = msg[meta["eorder"]]
    msgT = np.zeros((NC, 48, EHALF), dtype=ml_dtypes.bfloat16)
    msgT[:, :10, :] = _bf(msg_full[:, :EHALF].transpose(0, 2, 1))
    msgT[:, 32:42, :] = _bf(msg_full[:, EHALF:].transpose(0, 2, 1))

    dstwin = meta["dstwin"]  # [NC, E_PAD]
    dwin_in = np.ascontiguousarray(
        dstwin.reshape(NC, E_PAD // 128, 128).transpose(0, 2, 1)).astype(np.float32)
    invcnt_in = np.ascontiguousarray(
        meta["inv_cnt"].reshape(NC, NWIN, 128).transpose(0, 2, 1)).astype(np.float32)
    padcnt_in = np.repeat(meta["pad_cnt"][:, None], 128, axis=1)[:, :, None].astype(np.float32)

    iota_in = np.broadcast_to(np.arange(128, dtype=np.float32)[None, :], (128, 128))
    iota_in = np.ascontiguousarray(iota_in)
    ident_in = np.eye(128, dtype=np.float32).astype(ml_dtypes.bfloat16)

    xj_row = meta["row_of_slot"][meta["xj_glob"]]  # [NC, E_PAD] chunk-major rows
    xj_in = np.ascontiguousarray(
        xj_row.reshape(NC, E_PAD // 128, 128).transpose(0, 2, 1)).astype(np.int32)

    # weights
    c1w = np.zeros((3, 128, 128), dtype=ml_dtypes.bfloat16)
    c1w[0, :10, :] = _bf(inputs["c1_w1"])
    c1w[0, 32:42, :] = _bf(inputs["c1_w1"])
    c1w[1] = _bf(inputs["c1_w2"])
    c1w[2] = _bf(inputs["c1_w3"])
    c1b = np.stack([np.asarray(inputs[f"c1_b{i}"], dtype=np.float32).reshape(128, 1)
                    for i in (1, 2, 3)])
    c1gn = np.stack([np.asarray(inputs[f"c1_gn{i}"], dtype=np.float32).reshape(3, 128, 1)
                     for i in (1, 2, 3)])

    w2a = np.asarray(inputs["c2_w1"], dtype=np.float32)   # [256, 256]
    WA2 = w2a[:128] - w2a[128:]
    WB2 = w2a[128:]
    c2wa = _tile_w(WA2)[0]                                # [2, 128, 128]
    c2wb = _tile_w(WB2)[0]
    c2w2 = _tile_w(np.asarray(inputs["c2_w2"], dtype=np.float32))  # [2,2,128,128]
    c2b = np.stack([np.asarray(inputs["c2_b1"], dtype=np.float32).reshape(2, 128, 1),
                    np.asarray(inputs["c2_b2"], dtype=np.float32).reshape(2, 128, 1)])
    c2gn = np.stack([np.asarray(inputs["c2_gn1"], dtype=np.float32).reshape(3, 2, 128, 1),
                     np.asarray(inputs["c2_gn2"], dtype=np.float32).reshape(3, 2, 128, 1)])

    w3a = np.asarray(inputs["c3_w1"], dtype=np.float32)   # [512, 256]
    WA3 = w3a[:256] - w3a[256:]
    WB3 = w3a[256:]
    c3wa = _tile_w(WA3)                                   # [2,2,128,128]
    c3wb = _tile_w(WB3)
    c3b = np.asarray(inputs["c3_b1"], dtype=np.float32).reshape(2, 128, 1)
    c3gn = np.asarray(inputs["c3_gn1"], dtype=np.float32).reshape(3, 2, 128, 1)

    lw1 = _tile_w(np.asarray(inputs["lin_w1"], dtype=np.float32))
    lb1 = np.asarray(inputs["lin_b1"], dtype=np.float32).reshape(2, 128, 1)
    lw2_f = np.asarray(inputs["lin_w2"], dtype=np.float32)  # [256, 2]
    lw2 = np.stack([_bf(lw2_f[:128]), _bf(lw2_f[128:])])    # [2, 128, 2]
    lb2 = np.asarray(inputs["lin_b2"], dtype=np.float32).reshape(2, 1)

    Bg0 = meta["Bg"]
    pidx16_in = _wrap_idx(meta["pool_idx"].reshape(NC, 8 * Bg0 * 128))
    pidx16_in = pidx16_in.reshape(NC, 128, -1)
    pgwl = meta["pool_gwl"]                # [NC, 8, NPG]
    Bg_ = meta["Bg"]
    pgwl_in = np.ascontiguousarray(
        pgwl.reshape(NC, 8, Bg_, 128).transpose(0, 3, 1, 2)).reshape(NC, 128, 8 * Bg_)
    invg_in = np.broadcast_to(
        meta["inv_g"].reshape(8, 128).T[None], (NC, 128, 8)).astype(np.float32)
    invg_in = np.ascontiguousarray(invg_in)

    in_maps = []
    for c in range(NC):
        im = {
            "msgT": msgT[c],
            "xj_idx": xj_in[c],
            "dstwin": dwin_in[c],
            "invcnt": invcnt_in[c],
            "padcnt": padcnt_in[c],
            "iota": iota_in,
            "ident": ident_in,
            "c1w": c1w, "c1a": c1a_in, "c1b": c1b, "c1gn": c1gn,
            "c2wa": c2wa, "c2wb": c2wb, "c2w2": c2w2, "c2b": c2b, "c2gn": c2gn,
            "c3wa": c3wa, "c3wb": c3wb, "c3b": c3b, "c3gn": c3gn,
            "lw1": lw1, "lb1": lb1, "lw2": lw2, "lb2": lb2,
            "pidx16": pidx16_in[c],
            "pool_gwl": pgwl_in[c].astype(np.float32),
            "invg": invg_in[c],
        }
        in_maps.append(im)

    res = run_bass_kernel_spmd(nc, in_maps, core_ids=list(range(NC)),
                               trace=_TRACE[0])
    kernel.last_result = res
    kernel.last_meta = meta
    out = res.results[0]["out"]            # [2, 1000]
    return np.ascontiguousarray(out.T).astype(np.float32)


_DEBUG = [False]
_TRACE = [False]



# revision 54
# speedup vs baseline: 1.2406x; 1.0011x over previous
"""LundNetTagger GNN on 8 Trainium2 NeuronCores (Bass/Tile).

Self-contained: kernel(**inputs) -> np.ndarray [1000, 2] float32.

Strategy: nodes are assigned to 100352 "slots" (8 cores x 98 windows x 128),
packed so each window receives <= 512 edges. Edges live on the core owning
their dst slot, in window-major order padded to 4x128-edge chunks per window.
Per-edge MLPs run in bf16 feature-major layout; EdgeConv cat[xi, xj-xi] is
folded into split weights WA = W[:C]-W[C:], WB = W[C:]. GraphNorm stats are
global AllReduces of per-core sums (conv1 layer-1 stats are computed exactly
on the host from the 10-dim message Gram; deeper layers use vector-engine
bn_stats on PSUM with a sentinel pad column for exact correction).
conv1 keeps h fully SBUF-resident (no z spills): layer 2 overwrites h in
place after its stats AllReduce, and layer 3 fuses into the scatter.
Mean-aggregation is a collision-free one-hot matmul scatter into PSUM per
window. Node tables are AllGathered in bf16 between convs in two chunk-major
halves (each half fires as soon as its windows are written, overlapping the
producing scatter); src-side gathers use per-chunk indirect DMA with
chunk-major global row indices.
"""
import numpy as np
import ml_dtypes

import concourse.bass as bass
import concourse.tile as tile
from concourse import bacc, mybir
from concourse.bass_utils import run_bass_kernel_spmd
from concourse import library_config

BF16 = mybir.dt.bfloat16
F32 = mybir.dt.float32
I16 = mybir.dt.int16
AOP = mybir.AluOpType
AFT = mybir.ActivationFunctionType
AX = mybir.AxisListType

N_NODES = 100000
N_EDGES = 400000
N_GRAPHS = 1000
NC = 8
WIN = 128
NWIN = 98
SPC = WIN * NWIN          # 12544
NSLOTS = SPC * NC         # 100352
QUAD = NSLOTS // 4        # 25088
B = 4                     # chunks per window
EPW = B * WIN             # 512
E_PAD = NWIN * EPW        # 50176
EPS = 1e-5

NW_BLK = 7
BLK = NW_BLK * EPW        # 3584
NBLK = NWIN // NW_BLK     # 14
NCHUNK = BLK // 128       # 28
NSEG = BLK // 512         # 7

# window-aligned AllGather chunk boundaries (local rows) and the scatter
# block index after which each chunk's windows are complete
AG_CHB = [0, 25 * WIN, 50 * WIN, 74 * WIN, SPC]   # 3200/3200/3072/3072 rows
AG_FIRE = [4, 8, 11]     # fire chunk k at top of block AG_FIRE[k]; last at end


_cache = {}


# ============================ host-side packing ============================

def _pack(edge_index, batch):
    src = np.asarray(edge_index[0], dtype=np.int64)
    dst = np.asarray(edge_index[1], dtype=np.int64)
    batch = np.asarray(batch, dtype=np.int64)
    cnt = np.bincount(dst, minlength=N_NODES)

    nvirt = NSLOTS - N_NODES
    cnt_all = np.concatenate([cnt, np.zeros(nvirt, dtype=cnt.dtype)])
    order = np.argsort(-cnt_all, kind="stable")
    GW = NWIN * NC
    rounds = NSLOTS // GW
    win_of_rank = np.empty(NSLOTS, dtype=np.int64)
    for r in range(rounds):
        seg = np.arange(GW) if r % 2 == 0 else np.arange(GW - 1, -1, -1)
        win_of_rank[r * GW:(r + 1) * GW] = seg
    win_of_node = np.empty(NSLOTS, dtype=np.int64)
    win_of_node[order] = win_of_rank
    wsum = np.bincount(win_of_node, weights=cnt_all.astype(np.float64),
                       minlength=GW).astype(np.int64)

    cap = EPW
    members_of = [list(np.where(win_of_node == w)[0]) for w in range(GW)]
    for _ in range(2000):
        over = np.where(wsum > cap)[0]
        if len(over) == 0:
            break
        w = int(over[0])
        # smallest-count >0 node in w
        mem = members_of[w]
        cs = [(int(cnt_all[n]), n) for n in mem if cnt_all[n] > 0]
        cs.sort()
        moved = False
        for c1, n in cs:
            # find target window with a smaller-count node to swap
            worder2 = np.argsort(wsum)
            for tw in worder2[:64]:
                tw = int(tw)
                if tw == w:
                    continue
                tmem = members_of[tw]
                best = None
                for m in tmem:
                    c2 = int(cnt_all[m])
                    if c2 < c1 and wsum[tw] + c1 - c2 <= cap:
                        if best is None or c2 < best[0]:
                            best = (c2, m)
                        if c2 == 0:
                            break
                if best is not None:
                    c2, m = best
                    members_of[tw].remove(m)
                    members_of[tw].append(n)
                    members_of[w].remove(n)
                    members_of[w].append(m)
                    win_of_node[n] = tw
                    win_of_node[m] = w
                    wsum[tw] += c1 - c2
                    wsum[w] -= c1 - c2
                    moved = True
                    break
            if moved:
                break
        if not moved:
            raise RuntimeError("packing fixup stuck")
    assert wsum.max() <= cap, f"window packing failed: max={wsum.max()}"

    worder = np.argsort(-wsum, kind="stable")
    core_load = np.zeros(NC, dtype=np.int64)
    core_nwin = np.zeros(NC, dtype=np.int64)
    core_of_win = np.empty(GW, dtype=np.int64)
    for w in worder:
        cands = np.where(core_nwin < NWIN)[0]
        c = cands[np.argmin(core_load[cands])]
        core_of_win[w] = c
        core_load[c] += wsum[w]
        core_nwin[c] += 1

    win_lists = [[] for _ in range(NC)]
    for w in range(GW):
        win_lists[core_of_win[w]].append(w)
    for c in range(NC):
        wl = win_lists[c]
        j = int(np.argmin(wsum[wl]))
        assert wsum[wl[j]] < cap, "no sentinel room"
        wl[j], wl[-1] = wl[-1], wl[j]

    slot_of_node = np.empty(NSLOTS, dtype=np.int64)
    for c in range(NC):
        for wi, w in enumerate(win_lists[c]):
            mem = np.sort(np.array(members_of[w], dtype=np.int64))
            assert len(mem) == WIN
            slot_of_node[mem] = c * SPC + wi * WIN + np.arange(WIN)
    node_of_slot = np.empty(NSLOTS, dtype=np.int64)
    node_of_slot[slot_of_node] = np.arange(NSLOTS)
    cnt_of_slot = cnt_all[node_of_slot]

    qzero = []
    for q in range(4):
        z = np.where(cnt_of_slot[q * QUAD:(q + 1) * QUAD] == 0)[0]
        assert len(z) > 0
        assert z[0] < 32768
        qzero.append(int(z[0]))  # local to quadrant
    czero = []
    for c in range(NC):
        z = np.where(cnt_of_slot[c * SPC:(c + 1) * SPC] == 0)[0]
        assert len(z) > 0
        czero.append(int(z[0]))  # local to core

    dslot = slot_of_node[dst]
    sslot = slot_of_node[src]
    ecore = dslot // SPC
    ewin = (dslot % SPC) // WIN
    key = ecore * (NWIN * WIN) + ewin * WIN + (dslot % WIN)
    eorder = np.argsort(key, kind="stable")
    dsl, ssl = dslot[eorder], sslot[eorder]
    ec, ew = ecore[eorder], ewin[eorder]

    cw = ec * NWIN + ew
    cw_cnt = np.bincount(cw, minlength=NC * NWIN)
    assert cw_cnt.max() <= EPW

    xi_idx = np.zeros((NC, E_PAD), dtype=np.int64)
    xj_idx = np.zeros((NC, E_PAD), dtype=np.int64)
    dstwin = np.full((NC, E_PAD), -1.0, dtype=np.float32)
    valid = np.zeros((NC, E_PAD), dtype=bool)

    ofs = (np.arange(NC * NWIN) % NWIN) * EPW
    start = np.concatenate([[0], np.cumsum(cw_cnt)[:-1]])
    within = np.arange(N_EDGES) - start[cw]
    pos = ofs[cw] + within
    xi_idx[ec, pos] = dsl % SPC
    xj_idx[ec, pos] = ssl
    dstwin[ec, pos] = (dsl % WIN).astype(np.float32)
    valid[ec, pos] = True
    for c in range(NC):
        xi_idx[c, ~valid[c]] = czero[c]
    pad_cnt = (~valid).sum(axis=1).astype(np.float32)
    assert np.all(~valid[:, -1]), "sentinel column must be padding"

    gzero = qzero[0]  # global slot with zero row
    xj_glob = np.where(valid, xj_idx, gzero).astype(np.int32)

    # Chunk-major AllGather table layout: local rows split into 4
    # window-aligned chunks; the full table stores [chunk][core][rows] so
    # each AG chunk output is a contiguous row block.
    sl_ = np.arange(NSLOTS)
    n_, s_ = sl_ // SPC, sl_ % SPC
    c_ = np.searchsorted(np.array(AG_CHB), s_, side="right") - 1
    sizes = np.diff(np.array(AG_CHB))
    base_full = np.concatenate([[0], np.cumsum(sizes * NC)[:-1]])
    row_of_slot = (base_full[c_] + n_ * sizes[c_]
                   + (s_ - np.array(AG_CHB)[c_]))

    inv_cnt = (1.0 / np.maximum(cnt_of_slot.reshape(NC, SPC), 1.0)).astype(np.float32)

    g_of_slot = np.full(NSLOTS, -1, dtype=np.int64)
    real = node_of_slot < N_NODES
    g_of_slot[real] = batch[node_of_slot[real]]
    NGW = 8
    Bg = 0
    pools = [[None] * NGW for _ in range(NC)]
    for c in range(NC):
        gl = g_of_slot[c * SPC:(c + 1) * SPC]
        for gw in range(NGW):
            m = np.where((gl >= gw * 128) & (gl < (gw + 1) * 128))[0]
            pools[c][gw] = m
            Bg = max(Bg, (len(m) + 127) // 128)
    NPG = Bg * 128
    pool_idx = np.zeros((NC, NGW, NPG), dtype=np.int16)
    pool_gwl = np.full((NC, NGW, NPG), -1.0, dtype=np.float32)
    for c in range(NC):
        for gw in range(NGW):
            m = pools[c][gw]
            pool_idx[c, gw, :len(m)] = m.astype(np.int16)
            pool_idx[c, gw, len(m):] = czero[c]
            pool_gwl[c, gw, :len(m)] = (g_of_slot[c * SPC + m] - gw * 128).astype(np.float32)

    gcnt = np.bincount(batch, minlength=N_GRAPHS).astype(np.float32)
    inv_g = np.zeros(1024, dtype=np.float32)
    inv_g[:N_GRAPHS] = 1.0 / np.maximum(gcnt, 1.0)

    return dict(slot_of_node=slot_of_node, node_of_slot=node_of_slot,
                row_of_slot=row_of_slot,
                xj_glob=xj_glob, dstwin=dstwin, pad_cnt=pad_cnt,
                inv_cnt=inv_cnt, valid=valid, eorder=eorder, ec=ec, pos=pos,
                pool_idx=pool_idx, pool_gwl=pool_gwl, inv_g=inv_g, Bg=Bg)


def _wrap_idx(a):
    """[.., n] int -> [.., 128, n//16]: element i -> partition i%16 col i//16,
    replicated to 8 groups of 16 partitions."""
    n = a.shape[-1]
    assert n % 16 == 0
    w = a.reshape(*a.shape[:-1], n // 16, 16)
    w = np.swapaxes(w, -1, -2)
    w = np.broadcast_to(w[..., None, :, :], (*a.shape[:-1], 8, 16, n // 16))
    return np.ascontiguousarray(w).reshape(*a.shape[:-1], 128, n // 16).astype(np.int16)


def _bf(x):
    return np.ascontiguousarray(np.asarray(x, dtype=np.float32)).astype(ml_dtypes.bfloat16)


def _tile_w(w):
    K, M = w.shape
    nk, nm = (K + 127) // 128, (M + 127) // 128
    out = np.zeros((nk, nm, 128, 128), dtype=ml_dtypes.bfloat16)
    for i in range(nk):
        for j in range(nm):
            blk = np.asarray(w, dtype=np.float32)[i * 128:(i + 1) * 128, j * 128:(j + 1) * 128]
            out[i, j, :blk.shape[0], :blk.shape[1]] = _bf(blk)
    return out


# ============================ device kernel ============================

EHALF = E_PAD // 2        # 25088
NSEG_H = EHALF // 512     # 49


def _build(Bg, debug=False, phases=4):
    nc = bacc.Bacc("TRN2", target_bir_lowering=False, debug=False, num_devices=NC)

    def din(name, shape, dt):
        return nc.dram_tensor(name, shape, dt, kind="ExternalInput").ap()

    NIDX = E_PAD // 16
    t_msgT = din("msgT", [48, EHALF], BF16)
    t_xj = din("xj_idx", [128, E_PAD // 128], mybir.dt.int32)
    t_dstwin = din("dstwin", [128, E_PAD // 128], F32)
    t_invcnt = din("invcnt", [128, NWIN], F32)
    t_padcnt = din("padcnt", [128, 1], F32)
    t_iota = din("iota", [128, 128], F32)
    t_ident = din("ident", [128, 128], BF16)
    t_c1w = din("c1w", [3, 128, 128], BF16)
    t_c1a = din("c1a", [2, 128, 1], F32)
    t_c1b = din("c1b", [3, 128, 1], F32)
    t_c1gn = din("c1gn", [3, 3, 128, 1], F32)
    t_c2wa = din("c2wa", [2, 128, 128], BF16)
    t_c2wb = din("c2wb", [2, 128, 128], BF16)
    t_c2w2 = din("c2w2", [2, 2, 128, 128], BF16)
    t_c2b = din("c2b", [2, 2, 128, 1], F32)
    t_c2gn = din("c2gn", [2, 3, 2, 128, 1], F32)
    t_c3wa = din("c3wa", [2, 2, 128, 128], BF16)
    t_c3wb = din("c3wb", [2, 2, 128, 128], BF16)
    t_c3b = din("c3b", [2, 128, 1], F32)
    t_c3gn = din("c3gn", [3, 2, 128, 1], F32)
    t_lw1 = din("lw1", [2, 2, 128, 128], BF16)
    t_lb1 = din("lb1", [2, 128, 1], F32)
    t_lw2 = din("lw2", [2, 128, 2], BF16)
    t_lb2 = din("lb2", [2, 1], F32)
    t_pidx16 = din("pidx16", [128, 8 * Bg * 128 // 16], I16)
    t_pgwl = din("pool_gwl", [128, 8 * Bg], F32)
    t_invg = din("invg", [128, 8], F32)

    o_out = nc.dram_tensor("out", [2, N_GRAPHS], F32, kind="ExternalOutput").ap()
    dbg = {}
    if debug:
        dbg["x1"] = nc.dram_tensor("dbg_x1", [NSLOTS, 128], BF16, kind="ExternalOutput").ap()
        dbg["x2"] = nc.dram_tensor("dbg_x2", [NSLOTS, 256], BF16, kind="ExternalOutput").ap()
        dbg["x3"] = nc.dram_tensor("dbg_x3", [SPC, 256], BF16, kind="ExternalOutput").ap()
        dbg["pool"] = nc.dram_tensor("dbg_pool", [1024, 256], F32, kind="ExternalOutput").ap()

    with tile.TileContext(nc) as tc:
        with tc.tile_pool(name="dram", bufs=1, space="DRAM") as dram, \
             tc.tile_pool(name="cp", bufs=1) as cp:
            z_scr = [dram.tile([2, 128, E_PAD], BF16, tag=f"zscr{i}", name=f"zscr{i}") for i in range(2)]
            tab1_loc = dram.tile([SPC, 128], BF16)
            tab1 = dram.tile([NSLOTS, 128], BF16)
            tab2_loc = dram.tile([SPC, 256], BF16)
            tab2 = dram.tile([NSLOTS, 256], BF16)
            tab3_loc = dram.tile([SPC, 256], BF16)
            st_in = dram.tile([128, 8], F32)
            st_out = dram.tile([128, 8], F32)
            pool_in = dram.tile([1024, 256], F32)
            pool_out = dram.tile([1024, 256], F32)

            ident = cp.tile([128, 128], BF16)
            nc.sync.dma_start(ident[:], t_ident[:])
            iota = cp.tile([128, 128], F32)
            nc.sync.dma_start(iota[:], t_iota[:])
            invcnt = cp.tile([128, NWIN], F32)
            nc.sync.dma_start(invcnt[:], t_invcnt[:])
            dwin = cp.tile([128, E_PAD // 128], F32)
            nc.sync.dma_start(dwin[:], t_dstwin[:])
            padcnt = cp.tile([128, 1], F32)
            nc.sync.dma_start(padcnt[:], t_padcnt[:])

            # ---------- helpers ----------
            def allreduce_stats(s_acc, q_acc, n_mb, sb):
                st = sb.tile([128, 8], F32, tag="st_")
                nc.vector.memset(st[:], 0.0)
                nc.vector.tensor_copy(st[:, 0:n_mb], s_acc[:])
                nc.vector.tensor_copy(st[:, 4:4 + n_mb], q_acc[:])
                nc.sync.dma_start(st_in[:], st[:])
                nc.gpsimd.collective_compute(
                    "AllReduce", AOP.add, replica_groups=[list(range(NC))],
                    ins=[st_in.opt()], outs=[st_out.opt()])
                stg = sb.tile([128, 8], F32, tag="stg_")
                nc.sync.dma_start(stg[:], st_out[:])
                return stg

            def affine_from_stats(stg, n_mb, b_lin, gn, sb):
                A, Cc = [], []
                for mb in range(n_mb):
                    s = stg[:, mb:mb + 1]
                    q = stg[:, 4 + mb:5 + mb]
                    g, bgn, ms = gn[0][mb], gn[1][mb], gn[2][mb]
                    bl = b_lin[mb]
                    m = sb.tile([128, 1], F32, tag="af_m")
                    nc.vector.tensor_scalar(m[:], s, 1.0 / N_EDGES, None, AOP.mult)
                    nc.vector.tensor_tensor(m[:], m[:], bl, op=AOP.add)
                    e2 = sb.tile([128, 1], F32, tag="af_e2")
                    nc.vector.tensor_scalar(e2[:], q, 1.0 / N_EDGES, None, AOP.mult)
                    tmp = sb.tile([128, 1], F32, tag="af_t")
                    nc.vector.tensor_tensor(tmp[:], m[:], bl, op=AOP.mult)
                    nc.vector.tensor_scalar(tmp[:], tmp[:], 2.0, None, AOP.mult)
                    nc.vector.tensor_tensor(e2[:], e2[:], tmp[:], op=AOP.add)
                    nc.vector.tensor_tensor(tmp[:], bl, bl, op=AOP.mult)
                    nc.vector.tensor_tensor(e2[:], e2[:], tmp[:], op=AOP.subtract)
                    msm = sb.tile([128, 1], F32, tag="af_msm")
                    nc.vector.tensor_tensor(msm[:], ms, m[:], op=AOP.mult)
                    var = sb.tile([128, 1], F32, tag="af_v")
                    nc.vector.tensor_tensor(var[:], msm[:], msm[:], op=AOP.mult)
                    nc.vector.tensor_tensor(tmp[:], msm[:], m[:], op=AOP.mult)
                    nc.vector.tensor_scalar(tmp[:], tmp[:], 2.0, None, AOP.mult)
                    nc.vector.tensor_tensor(var[:], var[:], tmp[:], op=AOP.subtract)
                    nc.vector.tensor_tensor(var[:], var[:], e2[:], op=AOP.add)
                    a = sb.tile([128, 1], F32, tag="af_a")
                    nc.vector.tensor_scalar(var[:], var[:], EPS, None, AOP.add)
                    nc.scalar.activation(a[:], var[:], AFT.Sqrt)
                    nc.vector.reciprocal(a[:], a[:])
                    nc.vector.tensor_tensor(a[:], a[:], g, op=AOP.mult)
                    cc = sb.tile([128, 1], F32, tag="af_c")
                    nc.vector.tensor_tensor(cc[:], bl, msm[:], op=AOP.subtract)
                    nc.vector.tensor_tensor(cc[:], cc[:], a[:], op=AOP.mult)
                    nc.vector.tensor_tensor(cc[:], cc[:], bgn, op=AOP.add)
                    A.append(a)
                    Cc.append(cc)
                return A, Cc

            def acc_stats(ps_ap, s_col, q_col, sb):
                t1 = sb.tile([128, 1], F32, tag="rs_t1")
                nc.vector.reduce_sum(out=t1[:], in_=ps_ap, axis=AX.X)
                nc.vector.tensor_tensor(s_col, s_col, t1[:], op=AOP.add)
                n = ps_ap.shape[-1]
                sq = sb.tile([128, 512], BF16, tag="rs_sq")
                qa = sb.tile([128, 1], F32, tag="rs_qa")
                nc.scalar.activation(sq[:, :n], ps_ap, AFT.Square, accum_out=qa[:])
                nc.vector.tensor_tensor(q_col, q_col, qa[:], op=AOP.add)

            def bn_finish(st, s_col, q_col, sb, tag):
                # bn_stats 6-tuples (equal 512-col groups) -> sum / sq-sum
                agg = sb.tile([128, 2], F32, tag=tag + "g")
                nc.vector.bn_aggr(agg[:], st[:])
                nc.vector.tensor_scalar(s_col, agg[:, 0:1], float(E_PAD),
                                        None, AOP.mult)
                t = sb.tile([128, 1], F32, tag=tag + "t")
                nc.vector.tensor_tensor(t[:], agg[:, 0:1], agg[:, 0:1], op=AOP.mult)
                nc.vector.tensor_tensor(t[:], t[:], agg[:, 1:2], op=AOP.add)
                nc.vector.tensor_scalar(q_col, t[:], float(E_PAD), None, AOP.mult)

            def sentinel_correct(s_acc, q_acc, zsent_cols, n_mb, sb):
                for mb in range(n_mb):
                    zs = zsent_cols[mb]
                    t1 = sb.tile([128, 1], F32, tag="sc_t1")
                    nc.vector.tensor_tensor(t1[:], zs, padcnt[:], op=AOP.mult)
                    nc.vector.tensor_tensor(s_acc[:, mb:mb + 1], s_acc[:, mb:mb + 1],
                                            t1[:], op=AOP.subtract)
                    nc.vector.tensor_tensor(t1[:], zs, zs, op=AOP.mult)
                    nc.vector.tensor_tensor(t1[:], t1[:], padcnt[:], op=AOP.mult)
                    nc.vector.tensor_tensor(q_acc[:, mb:mb + 1], q_acc[:, mb:mb + 1],
                                            t1[:], op=AOP.subtract)

            def load_vec(t_ap, sb, tag):
                v = sb.tile([128, 1], F32, tag=tag)
                nc.sync.dma_start(v[:], t_ap)
                return v[:]

            AG_BASE = [0]
            for _c in range(3):
                AG_BASE.append(AG_BASE[-1] + NC * (AG_CHB[_c + 1] - AG_CHB[_c]))

            def fire_ag(tab_loc, tab_full, c):
                # chunk-major table: AG chunk c is a contiguous row block
                lo, hi = AG_CHB[c], AG_CHB[c + 1]
                nc.gpsimd.collective_compute(
                    "AllGather", AOP.bypass, replica_groups=[list(range(NC))],
                    ins=[tab_loc[lo:hi, :].opt()],
                    outs=[tab_full[AG_BASE[c]:AG_BASE[c] + NC * (hi - lo),
                                   :].opt()])

            def scatter_pass(zsrc, n_mb, A, Cc, tab_loc, Cout, ag=None):
                with tc.tile_pool(name="sc_sb", bufs=2) as sb, \
                     tc.tile_pool(name="sc_tp", bufs=2, space="PSUM") as ps_tp, \
                     tc.tile_pool(name="sc_sc", bufs=2, space="PSUM") as ps_sc:
                    for b in range(NBLK):
                        if ag is not None and b in AG_FIRE:
                            fire_ag(tab_loc, ag, AG_FIRE.index(b))
                        hs = []
                        for mb in range(n_mb):
                            z = sb.tile([128, BLK], BF16, tag=f"sp_z{mb}")
                            nc.sync.dma_start(z[:], zsrc[mb, :, b * BLK:(b + 1) * BLK])
                            h = sb.tile([128, BLK], BF16, tag=f"sp_h{mb}")
                            nc.scalar.activation(h[:], z[:], AFT.Relu,
                                                 bias=Cc[mb], scale=A[mb])
                            hs.append(h)
                        hE = sb.tile([128, NCHUNK * Cout], BF16, tag="sp_hE")
                        for ch in range(NCHUNK):
                            for mb in range(n_mb):
                                tp = ps_tp.tile([128, 128], BF16, tag="sp_tp", space="PSUM")
                                nc.tensor.transpose(tp[:], hs[mb][:, ch * 128:(ch + 1) * 128],
                                                    ident[:])
                                nc.vector.tensor_copy(
                                    hE[:, ch * Cout + mb * 128:ch * Cout + (mb + 1) * 128],
                                    tp[:])
                        for w in range(NW_BLK):
                            gw = b * NW_BLK + w
                            sc = ps_sc.tile([128, Cout], F32, tag="sp_sc", space="PSUM")
                            for cb in range(B):
                                ch = w * B + cb
                                col = b * NCHUNK + ch
                                oh = sb.tile([128, 128], BF16, tag="sp_oh")
                                nc.vector.tensor_tensor(
                                    out=oh[:],
                                    in0=dwin[:, col:col + 1].to_broadcast([128, 128]),
                                    in1=iota[:], op=AOP.is_equal)
                                nc.tensor.matmul(sc[:], oh[:],
                                                 hE[:, ch * Cout:(ch + 1) * Cout],
                                                 start=(cb == 0), stop=(cb == B - 1))
                            nt = sb.tile([128, Cout], BF16, tag="sp_nt")
                            nc.vector.tensor_scalar(nt[:], sc[:], invcnt[:, gw:gw + 1],
                                                    None, AOP.mult)
                            nc.sync.dma_start(tab_loc[gw * WIN:(gw + 1) * WIN, :], nt[:])
                    if ag is not None:
                        fire_ag(tab_loc, ag, 3)

            # ======================= CONV 1 =======================
            # SBUF-resident: L1 stats precomputed on host; h kept on-chip,
            # L2 overwrites it in place; L3 fused with the scatter.
            NSEG_T = E_PAD // 512  # 98
            with tc.tile_pool(name="c1sb", bufs=2) as sb:
                c1b = [[load_vec(t_c1b[i], sb, f"c1b{i}")] for i in range(3)]
                c1gn = [[[load_vec(t_c1gn[i, j], sb, f"c1gn{i}{j}")] for j in range(3)]
                        for i in range(3)]
                A1h = load_vec(t_c1a[0], sb, "c1a0")
                C1h = load_vec(t_c1a[1], sb, "c1a1")
                with tc.tile_pool(name="c1h", bufs=1) as hp, \
                     tc.tile_pool(name="c1ps", bufs=2, space="PSUM") as ps, \
                     tc.tile_pool(name="c1p2", bufs=2, space="PSUM") as ps2:
                    c1w = []
                    for i in range(3):
                        w = sb.tile([128, 128], BF16, tag=f"c1w{i}")
                        nc.sync.dma_start(w[:], t_c1w[i])
                        c1w.append(w)
                    msgT = hp.tile([48, EHALF], BF16, tag="msgT")
                    nc.sync.dma_start(msgT[:], t_msgT[:])
                    h_full = hp.tile([128, E_PAD], BF16)

                    def bn_to_sq(st, tag):
                        agg = sb.tile([128, 2], F32, tag=tag + "agg")
                        nc.vector.bn_aggr(agg[:], st[:])
                        s_acc = sb.tile([128, 1], F32, tag=tag + "s")
                        q_acc = sb.tile([128, 1], F32, tag=tag + "q")
                        nc.vector.tensor_scalar(s_acc[:], agg[:, 0:1],
                                                float(E_PAD), None, AOP.mult)
                        nc.vector.tensor_tensor(q_acc[:], agg[:, 0:1], agg[:, 0:1],
                                                op=AOP.mult)
                        nc.vector.tensor_tensor(q_acc[:], q_acc[:], agg[:, 1:2],
                                                op=AOP.add)
                        nc.vector.tensor_scalar(q_acc[:], q_acc[:],
                                                float(E_PAD), None, AOP.mult)
                        return s_acc, q_acc

                    # pass 1: L1 -> h_full; L2 stats
                    st2 = hp.tile([128, NSEG_T * 6], F32, tag="st2")
                    zs2 = sb.tile([128, 1], F32, tag="zs2")
                    for g in range(NSEG_T):
                        hh, shalf = g // NSEG_H, g % NSEG_H
                        zp = ps.tile([128, 512], F32, tag="zp")
                        nc.tensor.matmul(zp[:], c1w[0][32 * hh:32 * hh + 10, :],
                                         msgT[32 * hh:32 * hh + 10,
                                              shalf * 512:(shalf + 1) * 512],
                                         start=True, stop=True)
                        nc.scalar.activation(h_full[:, g * 512:(g + 1) * 512], zp[:],
                                             AFT.Relu, bias=C1h, scale=A1h)
                        zp2 = ps2.tile([128, 512], F32, tag="zp2")
                        nc.tensor.matmul(zp2[:], c1w[1][:],
                                         h_full[:, g * 512:(g + 1) * 512],
                                         start=True, stop=True)
                        nc.vector.bn_stats(st2[:, g * 6:(g + 1) * 6], zp2[:])
                        if g == NSEG_T - 1:
                            nc.vector.tensor_copy(zs2[:], zp2[:, 511:512])
                    s2, q2 = bn_to_sq(st2, "b2")
                    sentinel_correct(s2, q2, [zs2[:]], 1, sb)
                    stg2 = allreduce_stats(s2, q2, 1, sb)
                    A2, C2 = affine_from_stats(stg2, 1, c1b[1], c1gn[1], sb)

                    # pass 2: L2 -> h_full (in place); L3 stats
                    st3 = hp.tile([128, NSEG_T * 6], F32, tag="st3")
                    zs3 = sb.tile([128, 1], F32, tag="zs3")
                    for g in range(NSEG_T):
                        zp = ps.tile([128, 512], F32, tag="zp")
                        nc.tensor.matmul(zp[:], c1w[1][:],
                                         h_full[:, g * 512:(g + 1) * 512],
                                         start=True, stop=True)
                        nc.scalar.activation(h_full[:, g * 512:(g + 1) * 512], zp[:],
                                             AFT.Relu, bias=C2[0], scale=A2[0])
                        zp3 = ps2.tile([128, 512], F32, tag="zp2")
                        nc.tensor.matmul(zp3[:], c1w[2][:],
                                         h_full[:, g * 512:(g + 1) * 512],
                                         start=True, stop=True)
                        nc.vector.bn_stats(st3[:, g * 6:(g + 1) * 6], zp3[:])
                        if g == NSEG_T - 1:
                            nc.vector.tensor_copy(zs3[:], zp3[:, 511:512])
                    s3, q3 = bn_to_sq(st3, "b3")
                    sentinel_correct(s3, q3, [zs3[:]], 1, sb)
                    stg3 = allreduce_stats(s3, q3, 1, sb)
                    A3, C3 = affine_from_stats(stg3, 1, c1b[2], c1gn[2], sb)

                    # pass 3: L3 + fused scatter
                    with tc.tile_pool(name="c1sc", bufs=2) as scb, \
                         tc.tile_pool(name="c1tp", bufs=2, space="PSUM") as ps_tp, \
                         tc.tile_pool(name="c1s2", bufs=2, space="PSUM") as ps_sc:
                        for b in range(NBLK):
                            if b in AG_FIRE:
                                fire_ag(tab1_loc, tab1, AG_FIRE.index(b))
                            h3 = scb.tile([128, BLK], BF16, tag="c1h3")
                            for s in range(NSEG):
                                g = b * NSEG + s
                                zp = ps.tile([128, 512], F32, tag="zp")
                                nc.tensor.matmul(zp[:], c1w[2][:],
                                                 h_full[:, g * 512:(g + 1) * 512],
                                                 start=True, stop=True)
                                nc.scalar.activation(h3[:, s * 512:(s + 1) * 512],
                                                     zp[:], AFT.Relu,
                                                     bias=C3[0], scale=A3[0])
                            hE = scb.tile([128, NCHUNK * 128], BF16, tag="c1hE")
                            for ch in range(NCHUNK):
                                tp = ps_tp.tile([128, 128], BF16, tag="c1tp",
                                                space="PSUM")
                                nc.tensor.transpose(tp[:], h3[:, ch * 128:(ch + 1) * 128],
                                                    ident[:])
                                nc.vector.tensor_copy(hE[:, ch * 128:(ch + 1) * 128],
                                                      tp[:])
                            for w in range(NW_BLK):
                                gw = b * NW_BLK + w
                                sc = ps_sc.tile([128, 128], F32, tag="c1sc",
                                                space="PSUM")
                                for cb in range(B):
                                    ch = w * B + cb
                                    col = b * NCHUNK + ch
                                    oh = scb.tile([128, 128], BF16, tag="c1oh")
                                    nc.vector.tensor_tensor(
                                        out=oh[:],
                                        in0=dwin[:, col:col + 1].to_broadcast([128, 128]),
                                        in1=iota[:], op=AOP.is_equal)
                                    nc.tensor.matmul(sc[:], oh[:],
                                                     hE[:, ch * 128:(ch + 1) * 128],
                                                     start=(cb == 0), stop=(cb == B - 1))
                                nt = scb.tile([128, 128], BF16, tag="c1nt")
                                nc.vector.tensor_scalar(nt[:], sc[:],
                                                        invcnt[:, gw:gw + 1],
                                                        None, AOP.mult)
                                nc.sync.dma_start(tab1_loc[gw * WIN:(gw + 1) * WIN, :],
                                                  nt[:])
                        fire_ag(tab1_loc, tab1, 3)

            if debug:
                nc.sync.dma_start(dbg["x1"][:], tab1[:])

            # ============== gather-based first layer (conv2/conv3) ==============
            def gather_layer(tab_full, tab_loc, Cin, wa_t, wb_t, n_kb, zdst, sb):
                mb_in = Cin // 128
                s_acc = sb.tile([128, 2], F32, tag="gl_s")
                q_acc = sb.tile([128, 2], F32, tag="gl_q")
                sts = [sb.tile([128, (E_PAD // 512) * 6], F32, tag=f"gl_st{mo}",
                               name=f"gl_st{mo}")
                       for mo in range(2)]
                with tc.tile_pool(name="gl_g2", bufs=3) as g2, \
                     tc.tile_pool(name="gl_g1", bufs=2) as g1, \
                     tc.tile_pool(name="gl_zw", bufs=2) as zwp, \
                     tc.tile_pool(name="gl_ps", bufs=2, space="PSUM") as ps, \
                     tc.tile_pool(name="gl_tp", bufs=2, space="PSUM") as ps_tp, \
                     tc.tile_pool(name="gl_xp", bufs=2, space="PSUM") as ps_xp:
                    was, wbs = [], []
                    for ki in range(n_kb):
                        for mo in range(2):
                            wta = sb.tile([128, 128], BF16, tag=f"gl_wa{ki}{mo}")
                            nc.sync.dma_start(wta[:], wa_t[ki, mo] if n_kb > 1 else wa_t[mo])
                            was.append(wta)
                            wtb = sb.tile([128, 128], BF16, tag=f"gl_wb{ki}{mo}")
                            nc.sync.dma_start(wtb[:], wb_t[ki, mo] if n_kb > 1 else wb_t[mo])
                            wbs.append(wtb)
                    for b in range(NBLK):
                        ixj = g2.tile([128, NCHUNK], mybir.dt.int32, tag="gl_ixj")
                        nc.sync.dma_start(ixj[:], t_xj[:, b * NCHUNK:(b + 1) * NCHUNK])
                        gxj = g2.tile([128, NCHUNK * Cin], BF16, tag="gl_gxj")
                        for ch in range(NCHUNK):
                            nc.gpsimd.indirect_dma_start(
                                out=gxj[:, ch * Cin:(ch + 1) * Cin],
                                out_offset=None,
                                in_=tab_full[:],
                                in_offset=bass.IndirectOffsetOnAxis(
                                    ap=ixj[:, ch:ch + 1], axis=0))
                        xjT = g1.tile([128, mb_in * BLK], BF16, tag="gl_xjT")
                        for ch in range(NCHUNK):
                            for kb in range(mb_in):
                                tp2 = ps_tp.tile([128, 128], BF16, tag="gl_ohp",
                                                 space="PSUM")
                                nc.tensor.transpose(
                                    tp2[:],
                                    gxj[:, ch * Cin + kb * 128:ch * Cin + (kb + 1) * 128],
                                    ident[:])
                                nc.vector.tensor_copy(
                                    xjT[:, kb * BLK + ch * 128:kb * BLK + (ch + 1) * 128],
                                    tp2[:])
                        # xi via window expansion
                        xiT = g1.tile([128, mb_in * BLK], BF16, tag="gl_xiT")
                        for w in range(NW_BLK):
                            gw = b * NW_BLK + w
                            twin = g2.tile([128, Cin], BF16, tag="gl_twin")
                            nc.sync.dma_start(twin[:], tab_loc[gw * WIN:(gw + 1) * WIN, :])
                            for cb in range(B):
                                ch = w * B + cb
                                col = b * NCHUNK + ch
                                oh = g2.tile([128, 128], BF16, tag="gl_oh")
                                nc.vector.tensor_tensor(
                                    out=oh[:],
                                    in0=dwin[:, col:col + 1].to_broadcast([128, 128]),
                                    in1=iota[:], op=AOP.is_equal)
                                ohp = ps_tp.tile([128, 128], BF16, tag="gl_ohp", space="PSUM")
                                nc.tensor.transpose(ohp[:], oh[:], ident[:])
                                oh2 = g2.tile([128, 128], BF16, tag="gl_oh2")
                                nc.vector.tensor_copy(oh2[:], ohp[:])
                                for kb in range(mb_in):
                                    xp = ps_xp.tile([128, 128], F32, tag="gl_xp", space="PSUM")
                                    nc.tensor.matmul(xp[:], twin[:, kb * 128:(kb + 1) * 128],
                                                     oh2[:], start=True, stop=True)
                                    nc.vector.tensor_copy(
                                        xiT[:, kb * BLK + ch * 128:kb * BLK + (ch + 1) * 128],
                                        xp[:])
                        for mo in range(2):
                            zw = zwp.tile([128, BLK], BF16, tag=f"gl_z{mo}")
                            for sg in range(NSEG):
                                g6 = (b * NSEG + sg) * 6
                                zp = ps.tile([128, 512], F32, tag="gl_zp")
                                for ki in range(mb_in):
                                    nc.tensor.matmul(
                                        zp[:], was[ki * 2 + mo][:],
                                        xiT[:, ki * BLK + sg * 512:ki * BLK + (sg + 1) * 512],
                                        start=(ki == 0), stop=False)
                                for ki in range(mb_in):
                                    nc.tensor.matmul(
                                        zp[:], wbs[ki * 2 + mo][:],
                                        xjT[:, ki * BLK + sg * 512:ki * BLK + (sg + 1) * 512],
                                        start=False, stop=(ki == mb_in - 1))
                                nc.vector.bn_stats(sts[mo][:, g6:g6 + 6], zp[:])
                                nc.scalar.copy(zw[:, sg * 512:(sg + 1) * 512], zp[:])
                            nc.sync.dma_start(zdst[mo, :, b * BLK:(b + 1) * BLK], zw[:])
                for mo in range(2):
                    bn_finish(sts[mo], s_acc[:, mo:mo + 1], q_acc[:, mo:mo + 1],
                              sb, f"glf{mo}")
                return s_acc, q_acc

            # ======================= CONV 2 =======================
            if phases >= 2:
              with tc.tile_pool(name="c2sb", bufs=2) as sb:
                  c2b = [[load_vec(t_c2b[i, mb], sb, f"c2b{i}{mb}") for mb in range(2)]
                         for i in range(2)]
                  c2gn = [[[load_vec(t_c2gn[i, j, mb], sb, f"c2gn{i}{j}{mb}")
                            for mb in range(2)] for j in range(3)] for i in range(2)]
                  sA, qA = gather_layer(tab1, tab1_loc, 128, t_c2wa, t_c2wb, 1,
                                        z_scr[0], sb)
                  stg = allreduce_stats(sA, qA, 2, sb)
                  A1, C1 = affine_from_stats(stg, 2, c2b[0], c2gn[0], sb)

                  s2 = sb.tile([128, 2], F32, tag="c2s2")
                  q2 = sb.tile([128, 2], F32, tag="c2q2")
                  st2s = [sb.tile([128, (E_PAD // 512) * 6], F32, tag=f"c2st{mo}",
                                  name=f"c2st{mo}")
                          for mo in range(2)]
                  zlast = [None, None]
                  with tc.tile_pool(name="c2mid", bufs=2) as mp, \
                       tc.tile_pool(name="c2ps", bufs=2, space="PSUM") as ps:
                      w2s = []
                      for ki in range(2):
                          for mo in range(2):
                              w = sb.tile([128, 128], BF16, tag=f"c2w2{ki}{mo}")
                              nc.sync.dma_start(w[:], t_c2w2[ki, mo])
                              w2s.append(w)
                      for b in range(NBLK):
                          h1 = []
                          for mb in range(2):
                              z = mp.tile([128, BLK], BF16, tag=f"c2z1r{mb}")
                              nc.sync.dma_start(z[:], z_scr[0][mb, :, b * BLK:(b + 1) * BLK])
                              hh = mp.tile([128, BLK], BF16, tag=f"c2h1{mb}")
                              nc.scalar.activation(hh[:], z[:], AFT.Relu,
                                                   bias=C1[mb], scale=A1[mb])
                              h1.append(hh)
                          for mo in range(2):
                              zw = mp.tile([128, BLK], BF16, tag=f"c2z2w{mo}")
                              for s in range(NSEG):
                                  g6 = (b * NSEG + s) * 6
                                  zp = ps.tile([128, 512], F32, tag="c2zp")
                                  for ki in range(2):
                                      nc.tensor.matmul(zp[:], w2s[ki * 2 + mo][:],
                                                       h1[ki][:, s * 512:(s + 1) * 512],
                                                       start=(ki == 0), stop=(ki == 1))
                                  nc.vector.bn_stats(st2s[mo][:, g6:g6 + 6], zp[:])
                                  if s % 2 == 0:
                                      nc.scalar.copy(zw[:, s * 512:(s + 1) * 512],
                                                     zp[:])
                                  else:
                                      nc.vector.tensor_copy(
                                          zw[:, s * 512:(s + 1) * 512], zp[:])
                              nc.sync.dma_start(z_scr[1][mo, :, b * BLK:(b + 1) * BLK], zw[:])
                              zlast[mo] = zw
                      zsent = []
                      for mo in range(2):
                          zc = sb.tile([128, 1], F32, tag=f"c2zs{mo}")
                          nc.vector.tensor_copy(zc[:], zlast[mo][:, BLK - 1:BLK])
                          zsent.append(zc[:])
                  for mo in range(2):
                      bn_finish(st2s[mo], s2[:, mo:mo + 1], q2[:, mo:mo + 1],
                                sb, f"c2f{mo}")
                  sentinel_correct(s2, q2, zsent, 2, sb)
                  stg2 = allreduce_stats(s2, q2, 2, sb)
                  A2, C2 = affine_from_stats(stg2, 2, c2b[1], c2gn[1], sb)
                  scatter_pass(z_scr[1], 2, A2, C2, tab2_loc, 256, ag=tab2)

            if debug:
                nc.sync.dma_start(dbg["x2"][:], tab2[:])

            # ======================= CONV 3 =======================
            if phases >= 3:
              with tc.tile_pool(name="c3sb", bufs=2) as sb:
                  c3b = [load_vec(t_c3b[mb], sb, f"c3b{mb}") for mb in range(2)]
                  c3gn = [[load_vec(t_c3gn[j, mb], sb, f"c3gn{j}{mb}") for mb in range(2)]
                          for j in range(3)]
                  sA, qA = gather_layer(tab2, tab2_loc, 256, t_c3wa, t_c3wb, 2,
                                        z_scr[0], sb)
                  stg = allreduce_stats(sA, qA, 2, sb)
                  A1, C1 = affine_from_stats(stg, 2, c3b, c3gn, sb)
                  scatter_pass(z_scr[0], 2, A1, C1, tab3_loc, 256)

            if debug:
                nc.sync.dma_start(dbg["x3"][:], tab3_loc[:])

            # ======================= POOL + HEAD =======================
            if phases >= 4:
              with tc.tile_pool(name="p_sb", bufs=2) as sb, \
                 tc.tile_pool(name="p_ps", bufs=2, space="PSUM") as ps:
                  pgwl = sb.tile([128, 8 * Bg], F32, tag="p_pgwl")
                  nc.sync.dma_start(pgwl[:], t_pgwl[:])
                  NPG = Bg * 128
                  pidxw = sb.tile([128, 8 * NPG // 16], I16, tag="p_idx16")
                  nc.sync.dma_start(pidxw[:], t_pidx16[:])
                  for gw in range(8):
                      gp = sb.tile([128, Bg, 256], BF16, tag="p_gp")
                      nc.gpsimd.dma_gather(
                          out_ap=gp[:], in_ap=tab3_loc[:],
                          idxs_ap=pidxw[:, gw * (NPG // 16):(gw + 1) * (NPG // 16)],
                          num_idxs=NPG, num_idxs_reg=NPG, elem_size=256,
                          transpose=False, single_packet=(NPG <= 896))
                      pp = ps.tile([128, 256], F32, tag="p_pp", space="PSUM")
                      for c in range(Bg):
                          oh = sb.tile([128, 128], BF16, tag="p_oh")
                          nc.vector.tensor_tensor(
                              out=oh[:],
                              in0=pgwl[:, gw * Bg + c:gw * Bg + c + 1].to_broadcast([128, 128]),
                              in1=iota[:], op=AOP.is_equal)
                          nc.tensor.matmul(pp[:], oh[:], gp[:, c, :],
                                           start=(c == 0), stop=(c == Bg - 1))
                      pf = sb.tile([128, 256], F32, tag="p_pf")
                      nc.vector.tensor_copy(pf[:], pp[:])
                      nc.sync.dma_start(pool_in[gw * 128:(gw + 1) * 128, :], pf[:])
                  nc.gpsimd.collective_compute(
                      "AllReduce", AOP.add, replica_groups=[list(range(NC))],
                      ins=[pool_in.opt()], outs=[pool_out.opt()])
                  if debug:
                      nc.sync.dma_start(dbg["pool"][:], pool_out[:])

                  invg = sb.tile([128, 8], F32, tag="p_invg")
                  nc.sync.dma_start(invg[:], t_invg[:])
                  lw1 = []
                  for ki in range(2):
                      for mo in range(2):
                          w = sb.tile([128, 128], BF16, tag=f"p_lw1{ki}{mo}")
                          nc.sync.dma_start(w[:], t_lw1[ki, mo])
                          lw1.append(w)
                  lw2 = []
                  for ki in range(2):
                      w = sb.tile([128, 2], BF16, tag=f"p_lw2{ki}")
                      nc.sync.dma_start(w[:], t_lw2[ki])
                      lw2.append(w)
                  lb1 = [load_vec(t_lb1[mb], sb, f"p_lb1{mb}") for mb in range(2)]
                  lb2 = sb.tile([2, 1], F32, tag="p_lb2")
                  nc.sync.dma_start(lb2[:], t_lb2[:])
                  ofin = sb.tile([2, 1024], F32, tag="p_out")
                  for gw in range(8):
                      g = sb.tile([128, 256], F32, tag="p_g")
                      nc.sync.dma_start(g[:], pool_out[gw * 128:(gw + 1) * 128, :])
                      gm = sb.tile([128, 256], BF16, tag="p_gm")
                      nc.vector.tensor_scalar(gm[:], g[:], invg[:, gw:gw + 1], None, AOP.mult)
                      gT = sb.tile([128, 2 * 128], BF16, tag="p_gT")
                      for kb in range(2):
                          tp = ps.tile([128, 128], BF16, tag="p_tp", space="PSUM")
                          nc.tensor.transpose(tp[:], gm[:, kb * 128:(kb + 1) * 128], ident[:])
                          nc.vector.tensor_copy(gT[:, kb * 128:(kb + 1) * 128], tp[:])
                      hT = sb.tile([128, 2 * 128], BF16, tag="p_hT")
                      for mo in range(2):
                          hp = ps.tile([128, 128], F32, tag="p_hp", space="PSUM")
                          for ki in range(2):
                              nc.tensor.matmul(hp[:], lw1[ki * 2 + mo][:],
                                               gT[:, ki * 128:(ki + 1) * 128],
                                               start=(ki == 0), stop=(ki == 1))
                          nc.scalar.activation(hT[:, mo * 128:(mo + 1) * 128], hp[:],
                                               AFT.Relu, bias=lb1[mo])
                      op_ = ps.tile([2, 128], F32, tag="p_op", space="PSUM")
                      for ki in range(2):
                          nc.tensor.matmul(op_[:], lw2[ki][:],
                                           hT[:, ki * 128:(ki + 1) * 128],
                                           start=(ki == 0), stop=(ki == 1))
                      nc.vector.tensor_scalar(ofin[:, gw * 128:(gw + 1) * 128],
                                              op_[:], lb2[:], None, AOP.add)
                  nc.sync.dma_start(o_out[:], ofin[:, :N_GRAPHS])

    nc.compile()
    return nc


# ============================ entry point ============================


def kernel(**inputs):
    x = np.asarray(inputs["x"], dtype=np.float32)
    edge_index = np.asarray(inputs["edge_index"])
    batch = np.asarray(inputs["batch"])

    meta = _pack(edge_index, batch)
    Bg = meta["Bg"]

    import os as _os
    phases = int(_os.environ.get("KPHASES", "4"))
    key = ("mod", Bg, phases, _DEBUG[0])
    if key not in _cache:
        _cache[key] = _build(Bg, debug=bool(inputs.get("_debug", False)) or _DEBUG[0],
                             phases=phases)
    nc = _cache[key]

    # ---- per-core input arrays ----
    slot_of_node = meta["slot_of_node"]
    src = np.asarray(edge_index[0], dtype=np.int64)
    dst = np.asarray(edge_index[1], dtype=np.int64)

    # conv1 msgT: [core, 20, E_PAD//2] bf16; edge e<EHALF -> rows 0..9 col e,
    # e>=EHALF -> rows 10..19 col e-EHALF
    EHALF = E_PAD // 2
    xi_v = x[dst]
    xj_v = x[src]
    msg = np.concatenate([xi_v, xj_v - xi_v], axis=1)       # [E, 10]

    # exact conv1-L1 GraphNorm stats on host (tiny 10-dim Gram)
    msg64 = msg.astype(np.float64)
    W1 = np.asarray(inputs["c1_w1"], np.float64)            # [10, 128]
    b1 = np.asarray(inputs["c1_b1"], np.float64)            # [128]
    S = msg64.sum(0)
    G = msg64.T @ msg64
    SW = S @ W1
    qz = np.einsum('ij,ik,kj->j', W1, G, W1) + 2 * b1 * SW + N_EDGES * b1 * b1
    m1 = (SW + N_EDGES * b1) / N_EDGES
    e2 = qz / N_EDGES
    gn1 = np.asarray(inputs["c1_gn1"], np.float64)          # [3, 128]
    msm = gn1[2] * m1
    var1 = e2 - 2 * msm * m1 + msm * msm
    A1h = gn1[0] / np.sqrt(var1 + EPS)
    C1h = gn1[1] + A1h * (b1 - msm)
    c1a_in = np.stack([A1h, C1h]).astype(np.float32).reshape(2, 128, 1)

    msg_full = np.zeros((NC, E_PAD, 10), dtype=np.float32)
    ec, pos = meta["ec"], meta["pos"]
    msg_full[ec, pos] = msg[meta["eorder"]]
    msgT = np.zeros((NC, 48, EHALF), dtype=ml_dtypes.bfloat16)
    msgT[:, :10, :] = _bf(msg_full[:, :EHALF].transpose(0, 2, 1))
    msgT[:, 32:42, :] = _bf(msg_full[:, EHALF:].transpose(0, 2, 1))

    dstwin = meta["dstwin"]  # [NC, E_PAD]
    dwin_in = np.ascontiguousarray(
        dstwin.reshape(NC, E_PAD // 128, 128).transpose(0, 2, 1)).astype(np.float32)
    invcnt_in = np.ascontiguousarray(
        meta["inv_cnt"].reshape(NC, NWIN, 128).transpose(0, 2, 1)).astype(np.float32)
    padcnt_in = np.repeat(meta["pad_cnt"][:, None], 128, axis=1)[:, :, None].astype(np.float32)

    iota_in = np.broadcast_to(np.arange(128, dtype=np.float32)[None, :], (128, 128))
    iota_in = np.ascontiguousarray(iota_in)
    ident_in = np.eye(128, dtype=np.float32).astype(ml_dtypes.bfloat16)

    xj_row = meta["row_of_slot"][meta["xj_glob"]]  # [NC, E_PAD] chunk-major rows
    xj_in = np.ascontiguousarray(
        xj_row.reshape(NC, E_PAD // 128, 128).transpose(0, 2, 1)).astype(np.int32)

    # weights
    c1w = np.zeros((3, 128, 128), dtype=ml_dtypes.bfloat16)
    c1w[0, :10, :] = _bf(inputs["c1_w1"])
    c1w[0, 32:42, :] = _bf(inputs["c1_w1"])
    c1w[1] = _bf(inputs["c1_w2"])
    c1w[2] = _bf(inputs["c1_w3"])
    c1b = np.stack([np.asarray(inputs[f"c1_b{i}"], dtype=np.float32).reshape(128, 1)
                    for i in (1, 2, 3)])
    c1gn = np.stack([np.asarray(inputs[f"c1_gn{i}"], dtype=np.float32).reshape(3, 128, 1)
                     for i in (1, 2, 3)])

    w2a = np.asarray(inputs["c2_w1"], dtype=np.float32)   # [256, 256]
    WA2 = w2a[:128] - w2a[128:]
    WB2 = w2a[128:]
    c2wa = _tile_w(WA2)[0]                                # [2, 128, 128]
    c2wb = _tile_w(WB2)[0]
    c2w2 = _tile_w(np.asarray(inputs["c2_w2"], dtype=np.float32))  # [2,2,128,128]
    c2b = np.stack([np.asarray(inputs["c2_b1"], dtype=np.float32).reshape(2, 128, 1),
                    np.asarray(inputs["c2_b2"], dtype=np.float32).reshape(2, 128, 1)])
    c2gn = np.stack([np.asarray(inputs["c2_gn1"], dtype=np.float32).reshape(3, 2, 128, 1),
                     np.asarray(inputs["c2_gn2"], dtype=np.float32).reshape(3, 2, 128, 1)])

    w3a = np.asarray(inputs["c3_w1"], dtype=np.float32)   # [512, 256]
    WA3 = w3a[:256] - w3a[256:]
    WB3 = w3a[256:]
    c3wa = _tile_w(WA3)                                   # [2,2,128,128]
    c3wb = _tile_w(WB3)
    c3b = np.asarray(inputs["c3_b1"], dtype=np.float32).reshape(2, 128, 1)
    c3gn = np.asarray(inputs["c3_gn1"], dtype=np.float32).reshape(3, 2, 128, 1)

    lw1 = _tile_w(np.asarray(inputs["lin_w1"], dtype=np.float32))
    lb1 = np.asarray(inputs["lin_b1"], dtype=np.float32).reshape(2, 128, 1)
    lw2_f = np.asarray(inputs["lin_w2"], dtype=np.float32)  # [256, 2]
    lw2 = np.stack([_bf(lw2_f[:128]), _bf(lw2_f[128:])])    # [2, 128, 2]
    lb2 = np.asarray(inputs["lin_b2"], dtype=np.float32).reshape(2, 1)

    Bg0 = meta["Bg"]
    pidx16_in = _wrap_idx(meta["pool_idx"].reshape(NC, 8 * Bg0 * 128))
    pidx16_in = pidx16_in.reshape(NC, 128, -1)
    pgwl = meta["pool_gwl"]                # [NC, 8, NPG]
    Bg_ = meta["Bg"]
    pgwl_in = np.ascontiguousarray(
        pgwl.reshape(NC, 8, Bg_, 128).transpose(0, 3, 1, 2)).reshape(NC, 128, 8 * Bg_)
    invg_in = np.broadcast_to(
        meta["inv_g"].reshape(8, 128).T[None], (NC, 128, 8)).astype(np.float32)
    invg_in = np.ascontiguousarray(invg_in)

    in_maps = []
    for c in range(NC):
        im = {
            "msgT": msgT[c],
            "xj_idx": xj_in[c],
            "dstwin": dwin_in[c],
            "invcnt": invcnt_in[c],
            "padcnt": padcnt_in[c],
            "iota": iota_in,
            "ident": ident_in,
            "c1w": c1w, "c1a": c1a_in, "c1b": c1b, "c1gn": c1gn,
            "c2wa": c2wa, "c2wb": c2wb, "c2w2": c2w2, "c2b": c2b, "c2gn": c2gn,
            "c3wa": c3wa, "c3wb": c3wb, "c3b": c3b, "c3gn": c3gn,
            "lw1": lw1, "lb1": lb1, "lw2": lw2, "lb2": lb2,
            "pidx16": pidx16_in[c],
            "pool_gwl": pgwl_in[c].astype(np.float32),
            "invg": invg_in[c],
        }
        in_maps.append(im)

    res = run_bass_kernel_spmd(nc, in_maps, core_ids=list(range(NC)),
                               trace=_TRACE[0])
    kernel.last_result = res
    kernel.last_meta = meta
    out = res.results[0]["out"]            # [2, 1000]
    return np.ascontiguousarray(out.T).astype(np.float32)


_DEBUG = [False]
_TRACE = [False]



# revision 60
# speedup vs baseline: 1.2780x; 1.0301x over previous
"""LundNetTagger GNN on 8 Trainium2 NeuronCores (Bass/Tile).

Self-contained: kernel(**inputs) -> np.ndarray [1000, 2] float32.

Strategy: nodes are assigned to 100352 "slots" (8 cores x 98 windows x 128),
packed so each window receives <= 512 edges. Edges live on the core owning
their dst slot, in window-major order padded to 4x128-edge chunks per window.
Per-edge MLPs run in bf16 feature-major layout; EdgeConv cat[xi, xj-xi] is
folded into split weights WA = W[:C]-W[C:], WB = W[C:]. GraphNorm stats are
global AllReduces of per-core sums (conv1 layer-1 stats are computed exactly
on the host from the 10-dim message Gram; deeper layers use vector-engine
bn_stats on PSUM with a sentinel pad column for exact correction).
conv1 keeps h fully SBUF-resident (no z spills): layer 2 overwrites h in
place after its stats AllReduce, and layer 3 fuses into the scatter.
Mean-aggregation is a collision-free one-hot matmul scatter into PSUM per
window. Node tables are AllGathered in bf16 between convs in two chunk-major
halves (each half fires as soon as its windows are written, overlapping the
producing scatter); src-side gathers use per-chunk indirect DMA with
chunk-major global row indices.
"""
import numpy as np
import ml_dtypes

import concourse.bass as bass
import concourse.tile as tile
from concourse import bacc, mybir
from concourse.bass_utils import run_bass_kernel_spmd
from concourse import library_config

BF16 = mybir.dt.bfloat16
F32 = mybir.dt.float32
I16 = mybir.dt.int16
AOP = mybir.AluOpType
AFT = mybir.ActivationFunctionType
AX = mybir.AxisListType

N_NODES = 100000
N_EDGES = 400000
N_GRAPHS = 1000
NC = 8
WIN = 128
NWIN = 98
SPC = WIN * NWIN          # 12544
NSLOTS = SPC * NC         # 100352
QUAD = NSLOTS // 4        # 25088
B = 4                     # chunks per window
EPW = B * WIN             # 512
E_PAD = NWIN * EPW        # 50176
EPS = 1e-5

NW_BLK = 7
BLK = NW_BLK * EPW        # 3584
NBLK = NWIN // NW_BLK     # 14
NCHUNK = BLK // 128       # 28
NSEG = BLK // 512         # 7

# window-aligned AllGather chunk boundaries (local rows) and the scatter
# block index after which each chunk's windows are complete
AG_CHB = [0, 25 * WIN, 50 * WIN, 74 * WIN, SPC]   # 3200/3200/3072/3072 rows
AG_FIRE = [4, 8, 11]     # fire chunk k at top of block AG_FIRE[k]; last at end


_cache = {}


# ============================ host-side packing ============================

def _pack(edge_index, batch):
    src = np.asarray(edge_index[0], dtype=np.int64)
    dst = np.asarray(edge_index[1], dtype=np.int64)
    batch = np.asarray(batch, dtype=np.int64)
    cnt = np.bincount(dst, minlength=N_NODES)

    nvirt = NSLOTS - N_NODES
    cnt_all = np.concatenate([cnt, np.zeros(nvirt, dtype=cnt.dtype)])
    order = np.argsort(-cnt_all, kind="stable")
    GW = NWIN * NC
    rounds = NSLOTS // GW
    win_of_rank = np.empty(NSLOTS, dtype=np.int64)
    for r in range(rounds):
        seg = np.arange(GW) if r % 2 == 0 else np.arange(GW - 1, -1, -1)
        win_of_rank[r * GW:(r + 1) * GW] = seg
    win_of_node = np.empty(NSLOTS, dtype=np.int64)
    win_of_node[order] = win_of_rank
    wsum = np.bincount(win_of_node, weights=cnt_all.astype(np.float64),
                       minlength=GW).astype(np.int64)

    cap = EPW
    members_of = [list(np.where(win_of_node == w)[0]) for w in range(GW)]
    for _ in range(2000):
        over = np.where(wsum > cap)[0]
        if len(over) == 0:
            break
        w = int(over[0])
        # smallest-count >0 node in w
        mem = members_of[w]
        cs = [(int(cnt_all[n]), n) for n in mem if cnt_all[n] > 0]
        cs.sort()
        moved = False
        for c1, n in cs:
            # find target window with a smaller-count node to swap
            worder2 = np.argsort(wsum)
            for tw in worder2[:64]:
                tw = int(tw)
                if tw == w:
                    continue
                tmem = members_of[tw]
                best = None
                for m in tmem:
                    c2 = int(cnt_all[m])
                    if c2 < c1 and wsum[tw] + c1 - c2 <= cap:
                        if best is None or c2 < best[0]:
                            best = (c2, m)
                        if c2 == 0:
                            break
                if best is not None:
                    c2, m = best
                    members_of[tw].remove(m)
                    members_of[tw].append(n)
                    members_of[w].remove(n)
                    members_of[w].append(m)
                    win_of_node[n] = tw
                    win_of_node[m] = w
                    wsum[tw] += c1 - c2
                    wsum[w] -= c1 - c2
                    moved = True
                    break
            if moved:
                break
        if not moved:
            raise RuntimeError("packing fixup stuck")
    assert wsum.max() <= cap, f"window packing failed: max={wsum.max()}"

    worder = np.argsort(-wsum, kind="stable")
    core_load = np.zeros(NC, dtype=np.int64)
    core_nwin = np.zeros(NC, dtype=np.int64)
    core_of_win = np.empty(GW, dtype=np.int64)
    for w in worder:
        cands = np.where(core_nwin < NWIN)[0]
        c = cands[np.argmin(core_load[cands])]
        core_of_win[w] = c
        core_load[c] += wsum[w]
        core_nwin[c] += 1

    win_lists = [[] for _ in range(NC)]
    for w in range(GW):
        win_lists[core_of_win[w]].append(w)
    for c in range(NC):
        wl = win_lists[c]
        j = int(np.argmin(wsum[wl]))
        assert wsum[wl[j]] < cap, "no sentinel room"
        wl[j], wl[-1] = wl[-1], wl[j]

    slot_of_node = np.empty(NSLOTS, dtype=np.int64)
    for c in range(NC):
        for wi, w in enumerate(win_lists[c]):
            mem = np.sort(np.array(members_of[w], dtype=np.int64))
            assert len(mem) == WIN
            slot_of_node[mem] = c * SPC + wi * WIN + np.arange(WIN)
    node_of_slot = np.empty(NSLOTS, dtype=np.int64)
    node_of_slot[slot_of_node] = np.arange(NSLOTS)
    cnt_of_slot = cnt_all[node_of_slot]

    qzero = []
    for q in range(4):
        z = np.where(cnt_of_slot[q * QUAD:(q + 1) * QUAD] == 0)[0]
        assert len(z) > 0
        assert z[0] < 32768
        qzero.append(int(z[0]))  # local to quadrant
    czero = []
    for c in range(NC):
        z = np.where(cnt_of_slot[c * SPC:(c + 1) * SPC] == 0)[0]
        assert len(z) > 0
        czero.append(int(z[0]))  # local to core

    dslot = slot_of_node[dst]
    sslot = slot_of_node[src]
    ecore = dslot // SPC
    ewin = (dslot % SPC) // WIN
    key = ecore * (NWIN * WIN) + ewin * WIN + (dslot % WIN)
    eorder = np.argsort(key, kind="stable")
    dsl, ssl = dslot[eorder], sslot[eorder]
    ec, ew = ecore[eorder], ewin[eorder]

    cw = ec * NWIN + ew
    cw_cnt = np.bincount(cw, minlength=NC * NWIN)
    assert cw_cnt.max() <= EPW

    xi_idx = np.zeros((NC, E_PAD), dtype=np.int64)
    xj_idx = np.zeros((NC, E_PAD), dtype=np.int64)
    dstwin = np.full((NC, E_PAD), -1.0, dtype=np.float32)
    valid = np.zeros((NC, E_PAD), dtype=bool)

    ofs = (np.arange(NC * NWIN) % NWIN) * EPW
    start = np.concatenate([[0], np.cumsum(cw_cnt)[:-1]])
    within = np.arange(N_EDGES) - start[cw]
    pos = ofs[cw] + within
    xi_idx[ec, pos] = dsl % SPC
    xj_idx[ec, pos] = ssl
    dstwin[ec, pos] = (dsl % WIN).astype(np.float32)
    valid[ec, pos] = True
    for c in range(NC):
        xi_idx[c, ~valid[c]] = czero[c]
    pad_cnt = (~valid).sum(axis=1).astype(np.float32)
    assert np.all(~valid[:, -1]), "sentinel column must be padding"

    gzero = qzero[0]  # global slot with zero row
    xj_glob = np.where(valid, xj_idx, gzero).astype(np.int32)

    # Chunk-major AllGather table layout: local rows split into 4
    # window-aligned chunks; the full table stores [chunk][core][rows] so
    # each AG chunk output is a contiguous row block.
    sl_ = np.arange(NSLOTS)
    n_, s_ = sl_ // SPC, sl_ % SPC
    c_ = np.searchsorted(np.array(AG_CHB), s_, side="right") - 1
    sizes = np.diff(np.array(AG_CHB))
    base_full = np.concatenate([[0], np.cumsum(sizes * NC)[:-1]])
    row_of_slot = (base_full[c_] + n_ * sizes[c_]
                   + (s_ - np.array(AG_CHB)[c_]))

    inv_cnt = (1.0 / np.maximum(cnt_of_slot.reshape(NC, SPC), 1.0)).astype(np.float32)

    g_of_slot = np.full(NSLOTS, -1, dtype=np.int64)
    real = node_of_slot < N_NODES
    g_of_slot[real] = batch[node_of_slot[real]]
    NGW = 8
    Bg = 0
    pools = [[None] * NGW for _ in range(NC)]
    for c in range(NC):
        gl = g_of_slot[c * SPC:(c + 1) * SPC]
        for gw in range(NGW):
            m = np.where((gl >= gw * 128) & (gl < (gw + 1) * 128))[0]
            pools[c][gw] = m
            Bg = max(Bg, (len(m) + 127) // 128)
    NPG = Bg * 128
    pool_idx = np.zeros((NC, NGW, NPG), dtype=np.int16)
    pool_gwl = np.full((NC, NGW, NPG), -1.0, dtype=np.float32)
    for c in range(NC):
        for gw in range(NGW):
            m = pools[c][gw]
            pool_idx[c, gw, :len(m)] = m.astype(np.int16)
            pool_idx[c, gw, len(m):] = czero[c]
            pool_gwl[c, gw, :len(m)] = (g_of_slot[c * SPC + m] - gw * 128).astype(np.float32)

    gcnt = np.bincount(batch, minlength=N_GRAPHS).astype(np.float32)
    inv_g = np.zeros(1024, dtype=np.float32)
    inv_g[:N_GRAPHS] = 1.0 / np.maximum(gcnt, 1.0)

    return dict(slot_of_node=slot_of_node, node_of_slot=node_of_slot,
                row_of_slot=row_of_slot,
                xj_glob=xj_glob, dstwin=dstwin, pad_cnt=pad_cnt,
                inv_cnt=inv_cnt, valid=valid, eorder=eorder, ec=ec, pos=pos,
                pool_idx=pool_idx, pool_gwl=pool_gwl, inv_g=inv_g, Bg=Bg)


def _wrap_idx(a):
    """[.., n] int -> [.., 128, n//16]: element i -> partition i%16 col i//16,
    replicated to 8 groups of 16 partitions."""
    n = a.shape[-1]
    assert n % 16 == 0
    w = a.reshape(*a.shape[:-1], n // 16, 16)
    w = np.swapaxes(w, -1, -2)
    w = np.broadcast_to(w[..., None, :, :], (*a.shape[:-1], 8, 16, n // 16))
    return np.ascontiguousarray(w).reshape(*a.shape[:-1], 128, n // 16).astype(np.int16)


def _bf(x):
    return np.ascontiguousarray(np.asarray(x, dtype=np.float32)).astype(ml_dtypes.bfloat16)


def _tile_w(w):
    K, M = w.shape
    nk, nm = (K + 127) // 128, (M + 127) // 128
    out = np.zeros((nk, nm, 128, 128), dtype=ml_dtypes.bfloat16)
    for i in range(nk):
        for j in range(nm):
            blk = np.asarray(w, dtype=np.float32)[i * 128:(i + 1) * 128, j * 128:(j + 1) * 128]
            out[i, j, :blk.shape[0], :blk.shape[1]] = _bf(blk)
    return out


# ============================ device kernel ============================

EHALF = E_PAD // 2        # 25088
NSEG_H = EHALF // 512     # 49


def _build(Bg, debug=False, phases=4):
    nc = bacc.Bacc("TRN2", target_bir_lowering=False, debug=False, num_devices=NC)

    def din(name, shape, dt):
        return nc.dram_tensor(name, shape, dt, kind="ExternalInput").ap()

    NIDX = E_PAD // 16
    t_msgT = din("msgT", [48, EHALF], BF16)
    t_xj = din("xj_idx", [128, E_PAD // 128], mybir.dt.int32)
    t_dstwin = din("dstwin", [128, E_PAD // 128], F32)
    t_dwinR = din("dwinR", [128, E_PAD], BF16)
    t_iotap = din("iotap", [128, 1], F32)
    t_invcnt = din("invcnt", [128, NWIN], F32)
    t_padcnt = din("padcnt", [128, 1], F32)
    t_iota = din("iota", [128, 128], F32)
    t_ident = din("ident", [128, 128], BF16)
    t_c1w = din("c1w", [3, 128, 128], BF16)
    t_c1a = din("c1a", [2, 128, 1], F32)
    t_c1b = din("c1b", [3, 128, 1], F32)
    t_c1gn = din("c1gn", [3, 3, 128, 1], F32)
    t_c2wa = din("c2wa", [2, 128, 128], BF16)
    t_c2wb = din("c2wb", [2, 128, 128], BF16)
    t_c2w2 = din("c2w2", [2, 2, 128, 128], BF16)
    t_c2b = din("c2b", [2, 2, 128, 1], F32)
    t_c2gn = din("c2gn", [2, 3, 2, 128, 1], F32)
    t_c3wa = din("c3wa", [2, 2, 128, 128], BF16)
    t_c3wb = din("c3wb", [2, 2, 128, 128], BF16)
    t_c3b = din("c3b", [2, 128, 1], F32)
    t_c3gn = din("c3gn", [3, 2, 128, 1], F32)
    t_lw1 = din("lw1", [2, 2, 128, 128], BF16)
    t_lb1 = din("lb1", [2, 128, 1], F32)
    t_lw2 = din("lw2", [2, 128, 2], BF16)
    t_lb2 = din("lb2", [2, 1], F32)
    t_pidx16 = din("pidx16", [128, 8 * Bg * 128 // 16], I16)
    t_pgwl = din("pool_gwl", [128, 8 * Bg], F32)
    t_invg = din("invg", [128, 8], F32)

    o_out = nc.dram_tensor("out", [2, N_GRAPHS], F32, kind="ExternalOutput").ap()
    dbg = {}
    if debug:
        dbg["x1"] = nc.dram_tensor("dbg_x1", [NSLOTS, 128], BF16, kind="ExternalOutput").ap()
        dbg["x2"] = nc.dram_tensor("dbg_x2", [NSLOTS, 256], BF16, kind="ExternalOutput").ap()
        dbg["x3"] = nc.dram_tensor("dbg_x3", [SPC, 256], BF16, kind="ExternalOutput").ap()
        dbg["pool"] = nc.dram_tensor("dbg_pool", [1024, 256], F32, kind="ExternalOutput").ap()

    with tile.TileContext(nc) as tc:
        with tc.tile_pool(name="dram", bufs=1, space="DRAM") as dram, \
             tc.tile_pool(name="cp", bufs=1) as cp:
            z_scr = [dram.tile([2, 128, E_PAD], BF16, tag=f"zscr{i}", name=f"zscr{i}") for i in range(2)]
            tab1_loc = dram.tile([SPC, 128], BF16)
            tab1 = dram.tile([NSLOTS, 128], BF16)
            tab2_loc = dram.tile([SPC, 256], BF16)
            tab2 = dram.tile([NSLOTS, 256], BF16)
            tab3_loc = dram.tile([SPC, 256], BF16)
            st_in = dram.tile([128, 8], F32)
            st_out = dram.tile([128, 8], F32)
            pool_in = dram.tile([1024, 256], F32)
            pool_out = dram.tile([1024, 256], F32)

            ident = cp.tile([128, 128], BF16)
            nc.sync.dma_start(ident[:], t_ident[:])
            iota = cp.tile([128, 128], F32)
            nc.sync.dma_start(iota[:], t_iota[:])
            invcnt = cp.tile([128, NWIN], F32)
            nc.sync.dma_start(invcnt[:], t_invcnt[:])
            dwin = cp.tile([128, E_PAD // 128], F32)
            nc.sync.dma_start(dwin[:], t_dstwin[:])
            padcnt = cp.tile([128, 1], F32)
            nc.sync.dma_start(padcnt[:], t_padcnt[:])
            iotap = cp.tile([128, 1], F32)
            nc.sync.dma_start(iotap[:], t_iotap[:])

            # ---------- helpers ----------
            def allreduce_stats(s_acc, q_acc, n_mb, sb):
                st = sb.tile([128, 8], F32, tag="st_")
                nc.vector.memset(st[:], 0.0)
                nc.vector.tensor_copy(st[:, 0:n_mb], s_acc[:])
                nc.vector.tensor_copy(st[:, 4:4 + n_mb], q_acc[:])
                nc.sync.dma_start(st_in[:], st[:])
                nc.gpsimd.collective_compute(
                    "AllReduce", AOP.add, replica_groups=[list(range(NC))],
                    ins=[st_in.opt()], outs=[st_out.opt()])
                stg = sb.tile([128, 8], F32, tag="stg_")
                nc.sync.dma_start(stg[:], st_out[:])
                return stg

            def affine_from_stats(stg, n_mb, b_lin, gn, sb):
                A, Cc = [], []
                for mb in range(n_mb):
                    s = stg[:, mb:mb + 1]
                    q = stg[:, 4 + mb:5 + mb]
                    g, bgn, ms = gn[0][mb], gn[1][mb], gn[2][mb]
                    bl = b_lin[mb]
                    m = sb.tile([128, 1], F32, tag="af_m")
                    nc.vector.tensor_scalar(m[:], s, 1.0 / N_EDGES, None, AOP.mult)
                    nc.vector.tensor_tensor(m[:], m[:], bl, op=AOP.add)
                    e2 = sb.tile([128, 1], F32, tag="af_e2")
                    nc.vector.tensor_scalar(e2[:], q, 1.0 / N_EDGES, None, AOP.mult)
                    tmp = sb.tile([128, 1], F32, tag="af_t")
                    nc.vector.tensor_tensor(tmp[:], m[:], bl, op=AOP.mult)
                    nc.vector.tensor_scalar(tmp[:], tmp[:], 2.0, None, AOP.mult)
                    nc.vector.tensor_tensor(e2[:], e2[:], tmp[:], op=AOP.add)
                    nc.vector.tensor_tensor(tmp[:], bl, bl, op=AOP.mult)
                    nc.vector.tensor_tensor(e2[:], e2[:], tmp[:], op=AOP.subtract)
                    msm = sb.tile([128, 1], F32, tag="af_msm")
                    nc.vector.tensor_tensor(msm[:], ms, m[:], op=AOP.mult)
                    var = sb.tile([128, 1], F32, tag="af_v")
                    nc.vector.tensor_tensor(var[:], msm[:], msm[:], op=AOP.mult)
                    nc.vector.tensor_tensor(tmp[:], msm[:], m[:], op=AOP.mult)
                    nc.vector.tensor_scalar(tmp[:], tmp[:], 2.0, None, AOP.mult)
                    nc.vector.tensor_tensor(var[:], var[:], tmp[:], op=AOP.subtract)
                    nc.vector.tensor_tensor(var[:], var[:], e2[:], op=AOP.add)
                    a = sb.tile([128, 1], F32, tag="af_a")
                    nc.vector.tensor_scalar(var[:], var[:], EPS, None, AOP.add)
                    nc.scalar.activation(a[:], var[:], AFT.Sqrt)
                    nc.vector.reciprocal(a[:], a[:])
                    nc.vector.tensor_tensor(a[:], a[:], g, op=AOP.mult)
                    cc = sb.tile([128, 1], F32, tag="af_c")
                    nc.vector.tensor_tensor(cc[:], bl, msm[:], op=AOP.subtract)
                    nc.vector.tensor_tensor(cc[:], cc[:], a[:], op=AOP.mult)
                    nc.vector.tensor_tensor(cc[:], cc[:], bgn, op=AOP.add)
                    A.append(a)
                    Cc.append(cc)
                return A, Cc

            def acc_stats(ps_ap, s_col, q_col, sb):
                t1 = sb.tile([128, 1], F32, tag="rs_t1")
                nc.vector.reduce_sum(out=t1[:], in_=ps_ap, axis=AX.X)
                nc.vector.tensor_tensor(s_col, s_col, t1[:], op=AOP.add)
                n = ps_ap.shape[-1]
                sq = sb.tile([128, 512], BF16, tag="rs_sq")
                qa = sb.tile([128, 1], F32, tag="rs_qa")
                nc.scalar.activation(sq[:, :n], ps_ap, AFT.Square, accum_out=qa[:])
                nc.vector.tensor_tensor(q_col, q_col, qa[:], op=AOP.add)

            def bn_finish(st, s_col, q_col, sb, tag):
                # bn_stats 6-tuples (equal 512-col groups) -> sum / sq-sum
                agg = sb.tile([128, 2], F32, tag=tag + "g")
                nc.vector.bn_aggr(agg[:], st[:])
                nc.vector.tensor_scalar(s_col, agg[:, 0:1], float(E_PAD),
                                        None, AOP.mult)
                t = sb.tile([128, 1], F32, tag=tag + "t")
                nc.vector.tensor_tensor(t[:], agg[:, 0:1], agg[:, 0:1], op=AOP.mult)
                nc.vector.tensor_tensor(t[:], t[:], agg[:, 1:2], op=AOP.add)
                nc.vector.tensor_scalar(q_col, t[:], float(E_PAD), None, AOP.mult)

            def sentinel_correct(s_acc, q_acc, zsent_cols, n_mb, sb):
                for mb in range(n_mb):
                    zs = zsent_cols[mb]
                    t1 = sb.tile([128, 1], F32, tag="sc_t1")
                    nc.vector.tensor_tensor(t1[:], zs, padcnt[:], op=AOP.mult)
                    nc.vector.tensor_tensor(s_acc[:, mb:mb + 1], s_acc[:, mb:mb + 1],
                                            t1[:], op=AOP.subtract)
                    nc.vector.tensor_tensor(t1[:], zs, zs, op=AOP.mult)
                    nc.vector.tensor_tensor(t1[:], t1[:], padcnt[:], op=AOP.mult)
                    nc.vector.tensor_tensor(q_acc[:, mb:mb + 1], q_acc[:, mb:mb + 1],
                                            t1[:], op=AOP.subtract)

            def load_vec(t_ap, sb, tag):
                v = sb.tile([128, 1], F32, tag=tag)
                nc.sync.dma_start(v[:], t_ap)
                return v[:]

            AG_BASE = [0]
            for _c in range(3):
                AG_BASE.append(AG_BASE[-1] + NC * (AG_CHB[_c + 1] - AG_CHB[_c]))

            def fire_ag(tab_loc, tab_full, c):
                # chunk-major table: AG chunk c is a contiguous row block
                lo, hi = AG_CHB[c], AG_CHB[c + 1]
                nc.gpsimd.collective_compute(
                    "AllGather", AOP.bypass, replica_groups=[list(range(NC))],
                    ins=[tab_loc[lo:hi, :].opt()],
                    outs=[tab_full[AG_BASE[c]:AG_BASE[c] + NC * (hi - lo),
                                   :].opt()])

            def scatter_pass(zsrc, n_mb, A, Cc, tab_loc, Cout, ag=None):
                with tc.tile_pool(name="sc_sb", bufs=2) as sb, \
                     tc.tile_pool(name="sc_tp", bufs=2, space="PSUM") as ps_tp, \
                     tc.tile_pool(name="sc_sc", bufs=2, space="PSUM") as ps_sc:
                    for b in range(NBLK):
                        if ag is not None and b in AG_FIRE:
                            fire_ag(tab_loc, ag, AG_FIRE.index(b))
                        hs = []
                        for mb in range(n_mb):
                            z = sb.tile([128, BLK], BF16, tag=f"sp_z{mb}")
                            nc.sync.dma_start(z[:], zsrc[mb, :, b * BLK:(b + 1) * BLK])
                            h = sb.tile([128, BLK], BF16, tag=f"sp_h{mb}")
                            nc.scalar.activation(h[:], z[:], AFT.Relu,
                                                 bias=Cc[mb], scale=A[mb])
                            hs.append(h)
                        hE = sb.tile([128, NCHUNK * Cout], BF16, tag="sp_hE")
                        for ch in range(NCHUNK):
                            for mb in range(n_mb):
                                tp = ps_tp.tile([128, 128], BF16, tag="sp_tp", space="PSUM")
                                nc.tensor.transpose(tp[:], hs[mb][:, ch * 128:(ch + 1) * 128],
                                                    ident[:])
                                nc.vector.tensor_copy(
                                    hE[:, ch * Cout + mb * 128:ch * Cout + (mb + 1) * 128],
                                    tp[:])
                        for w in range(NW_BLK):
                            gw = b * NW_BLK + w
                            sc = ps_sc.tile([128, Cout], F32, tag="sp_sc", space="PSUM")
                            for cb in range(B):
                                ch = w * B + cb
                                col = b * NCHUNK + ch
                                oh = sb.tile([128, 128], BF16, tag="sp_oh")
                                nc.vector.tensor_tensor(
                                    out=oh[:],
                                    in0=dwin[:, col:col + 1].to_broadcast([128, 128]),
                                    in1=iota[:], op=AOP.is_equal)
                                nc.tensor.matmul(sc[:], oh[:],
                                                 hE[:, ch * Cout:(ch + 1) * Cout],
                                                 start=(cb == 0), stop=(cb == B - 1))
                            nt = sb.tile([128, Cout], BF16, tag="sp_nt")
                            nc.vector.tensor_scalar(nt[:], sc[:], invcnt[:, gw:gw + 1],
                                                    None, AOP.mult)
                            nc.sync.dma_start(tab_loc[gw * WIN:(gw + 1) * WIN, :], nt[:])
                    if ag is not None:
                        fire_ag(tab_loc, ag, 3)

            # ======================= CONV 1 =======================
            # SBUF-resident: L1 stats precomputed on host; h kept on-chip,
            # L2 overwrites it in place; L3 fused with the scatter.
            NSEG_T = E_PAD // 512  # 98
            with tc.tile_pool(name="c1sb", bufs=2) as sb:
                c1b = [[load_vec(t_c1b[i], sb, f"c1b{i}")] for i in range(3)]
                c1gn = [[[load_vec(t_c1gn[i, j], sb, f"c1gn{i}{j}")] for j in range(3)]
                        for i in range(3)]
                A1h = load_vec(t_c1a[0], sb, "c1a0")
                C1h = load_vec(t_c1a[1], sb, "c1a1")
                with tc.tile_pool(name="c1h", bufs=1) as hp, \
                     tc.tile_pool(name="c1ps", bufs=2, space="PSUM") as ps, \
                     tc.tile_pool(name="c1p2", bufs=2, space="PSUM") as ps2:
                    c1w = []
                    for i in range(3):
                        w = sb.tile([128, 128], BF16, tag=f"c1w{i}")
                        nc.sync.dma_start(w[:], t_c1w[i])
                        c1w.append(w)
                    msgT = hp.tile([48, EHALF], BF16, tag="msgT")
                    nc.sync.dma_start(msgT[:], t_msgT[:])
                    h_full = hp.tile([128, E_PAD], BF16)

                    def bn_to_sq(st, tag):
                        agg = sb.tile([128, 2], F32, tag=tag + "agg")
                        nc.vector.bn_aggr(agg[:], st[:])
                        s_acc = sb.tile([128, 1], F32, tag=tag + "s")
                        q_acc = sb.tile([128, 1], F32, tag=tag + "q")
                        nc.vector.tensor_scalar(s_acc[:], agg[:, 0:1],
                                                float(E_PAD), None, AOP.mult)
                        nc.vector.tensor_tensor(q_acc[:], agg[:, 0:1], agg[:, 0:1],
                                                op=AOP.mult)
                        nc.vector.tensor_tensor(q_acc[:], q_acc[:], agg[:, 1:2],
                                                op=AOP.add)
                        nc.vector.tensor_scalar(q_acc[:], q_acc[:],
                                                float(E_PAD), None, AOP.mult)
                        return s_acc, q_acc

                    # pass 1: L1 -> h_full; L2 stats
                    st2 = hp.tile([128, NSEG_T * 6], F32, tag="st2")
                    zs2 = sb.tile([128, 1], F32, tag="zs2")
                    for g in range(NSEG_T):
                        hh, shalf = g // NSEG_H, g % NSEG_H
                        zp = ps.tile([128, 512], F32, tag="zp")
                        nc.tensor.matmul(zp[:], c1w[0][32 * hh:32 * hh + 10, :],
                                         msgT[32 * hh:32 * hh + 10,
                                              shalf * 512:(shalf + 1) * 512],
                                         start=True, stop=True)
                        nc.scalar.activation(h_full[:, g * 512:(g + 1) * 512], zp[:],
                                             AFT.Relu, bias=C1h, scale=A1h)
                        zp2 = ps2.tile([128, 512], F32, tag="zp2")
                        nc.tensor.matmul(zp2[:], c1w[1][:],
                                         h_full[:, g * 512:(g + 1) * 512],
                                         start=True, stop=True)
                        nc.vector.bn_stats(st2[:, g * 6:(g + 1) * 6], zp2[:])
                        if g == NSEG_T - 1:
                            nc.vector.tensor_copy(zs2[:], zp2[:, 511:512])
                    s2, q2 = bn_to_sq(st2, "b2")
                    sentinel_correct(s2, q2, [zs2[:]], 1, sb)
                    stg2 = allreduce_stats(s2, q2, 1, sb)
                    A2, C2 = affine_from_stats(stg2, 1, c1b[1], c1gn[1], sb)

                    # pass 2: L2 -> h_full (in place); L3 stats
                    st3 = hp.tile([128, NSEG_T * 6], F32, tag="st3")
                    zs3 = sb.tile([128, 1], F32, tag="zs3")
                    for g in range(NSEG_T):
                        zp = ps.tile([128, 512], F32, tag="zp")
                        nc.tensor.matmul(zp[:], c1w[1][:],
                                         h_full[:, g * 512:(g + 1) * 512],
                                         start=True, stop=True)
                        nc.scalar.activation(h_full[:, g * 512:(g + 1) * 512], zp[:],
                                             AFT.Relu, bias=C2[0], scale=A2[0])
                        zp3 = ps2.tile([128, 512], F32, tag="zp2")
                        nc.tensor.matmul(zp3[:], c1w[2][:],
                                         h_full[:, g * 512:(g + 1) * 512],
                                         start=True, stop=True)
                        nc.vector.bn_stats(st3[:, g * 6:(g + 1) * 6], zp3[:])
                        if g == NSEG_T - 1:
                            nc.vector.tensor_copy(zs3[:], zp3[:, 511:512])
                    s3, q3 = bn_to_sq(st3, "b3")
                    sentinel_correct(s3, q3, [zs3[:]], 1, sb)
                    stg3 = allreduce_stats(s3, q3, 1, sb)
                    A3, C3 = affine_from_stats(stg3, 1, c1b[2], c1gn[2], sb)

                    # pass 3: L3 + fused scatter
                    with tc.tile_pool(name="c1sc", bufs=2) as scb, \
                         tc.tile_pool(name="c1tp", bufs=2, space="PSUM") as ps_tp, \
                         tc.tile_pool(name="c1s2", bufs=2, space="PSUM") as ps_sc:
                        for b in range(NBLK):
                            if b in AG_FIRE:
                                fire_ag(tab1_loc, tab1, AG_FIRE.index(b))
                            h3 = scb.tile([128, BLK], BF16, tag="c1h3")
                            for s in range(NSEG):
                                g = b * NSEG + s
                                zp = ps.tile([128, 512], F32, tag="zp")
                                nc.tensor.matmul(zp[:], c1w[2][:],
                                                 h_full[:, g * 512:(g + 1) * 512],
                                                 start=True, stop=True)
                                nc.scalar.activation(h3[:, s * 512:(s + 1) * 512],
                                                     zp[:], AFT.Relu,
                                                     bias=C3[0], scale=A3[0])
                            hE = scb.tile([128, NCHUNK * 128], BF16, tag="c1hE")
                            for ch in range(NCHUNK):
                                tp = ps_tp.tile([128, 128], BF16, tag="c1tp",
                                                space="PSUM")
                                nc.tensor.transpose(tp[:], h3[:, ch * 128:(ch + 1) * 128],
                                                    ident[:])
                                nc.vector.tensor_copy(hE[:, ch * 128:(ch + 1) * 128],
                                                      tp[:])
                            for w in range(NW_BLK):
                                gw = b * NW_BLK + w
                                sc = ps_sc.tile([128, 128], F32, tag="c1sc",
                                                space="PSUM")
                                for cb in range(B):
                                    ch = w * B + cb
                                    col = b * NCHUNK + ch
                                    oh = scb.tile([128, 128], BF16, tag="c1oh")
                                    nc.vector.tensor_tensor(
                                        out=oh[:],
                                        in0=dwin[:, col:col + 1].to_broadcast([128, 128]),
                                        in1=iota[:], op=AOP.is_equal)
                                    nc.tensor.matmul(sc[:], oh[:],
                                                     hE[:, ch * 128:(ch + 1) * 128],
                                                     start=(cb == 0), stop=(cb == B - 1))
                                nt = scb.tile([128, 128], BF16, tag="c1nt")
                                nc.vector.tensor_scalar(nt[:], sc[:],
                                                        invcnt[:, gw:gw + 1],
                                                        None, AOP.mult)
                                nc.sync.dma_start(tab1_loc[gw * WIN:(gw + 1) * WIN, :],
                                                  nt[:])
                        fire_ag(tab1_loc, tab1, 3)

            if debug:
                nc.sync.dma_start(dbg["x1"][:], tab1[:])

            # ============== gather-based first layer (conv2/conv3) ==============
            def gather_layer(tab_full, tab_loc, Cin, wa_t, wb_t, n_kb, zdst, sb):
                mb_in = Cin // 128
                s_acc = sb.tile([128, 2], F32, tag="gl_s")
                q_acc = sb.tile([128, 2], F32, tag="gl_q")
                sts = [sb.tile([128, (E_PAD // 512) * 6], F32, tag=f"gl_st{mo}",
                               name=f"gl_st{mo}")
                       for mo in range(2)]
                with tc.tile_pool(name="gl_g2", bufs=3) as g2, \
                     tc.tile_pool(name="gl_g1", bufs=2) as g1, \
                     tc.tile_pool(name="gl_zw", bufs=2) as zwp, \
                     tc.tile_pool(name="gl_ps", bufs=2, space="PSUM") as ps, \
                     tc.tile_pool(name="gl_tp", bufs=2, space="PSUM") as ps_tp, \
                     tc.tile_pool(name="gl_xp", bufs=2, space="PSUM") as ps_xp:
                    was, wbs = [], []
                    for ki in range(n_kb):
                        for mo in range(2):
                            wta = sb.tile([128, 128], BF16, tag=f"gl_wa{ki}{mo}")
                            nc.sync.dma_start(wta[:], wa_t[ki, mo] if n_kb > 1 else wa_t[mo])
                            was.append(wta)
                            wtb = sb.tile([128, 128], BF16, tag=f"gl_wb{ki}{mo}")
                            nc.sync.dma_start(wtb[:], wb_t[ki, mo] if n_kb > 1 else wb_t[mo])
                            wbs.append(wtb)
                    for b in range(NBLK):
                        ixj = g2.tile([128, NCHUNK], mybir.dt.int32, tag="gl_ixj")
                        nc.sync.dma_start(ixj[:], t_xj[:, b * NCHUNK:(b + 1) * NCHUNK])
                        gxj = g2.tile([128, NCHUNK * Cin], BF16, tag="gl_gxj")
                        for ch in range(NCHUNK):
                            nc.gpsimd.indirect_dma_start(
                                out=gxj[:, ch * Cin:(ch + 1) * Cin],
                                out_offset=None,
                                in_=tab_full[:],
                                in_offset=bass.IndirectOffsetOnAxis(
                                    ap=ixj[:, ch:ch + 1], axis=0))
                        xjT = g1.tile([128, mb_in * BLK], BF16, tag="gl_xjT")
                        for ch in range(NCHUNK):
                            for kb in range(mb_in):
                                tp2 = ps_tp.tile([128, 128], BF16, tag="gl_ohp",
                                                 space="PSUM")
                                nc.tensor.transpose(
                                    tp2[:],
                                    gxj[:, ch * Cin + kb * 128:ch * Cin + (kb + 1) * 128],
                                    ident[:])
                                nc.vector.tensor_copy(
                                    xjT[:, kb * BLK + ch * 128:kb * BLK + (ch + 1) * 128],
                                    tp2[:])
                        # xi via window expansion (transposed one-hot built
                        # directly from the replicated dstwin row)
                        dwb = g1.tile([128, BLK], BF16, tag="gl_dwb")
                        nc.sync.dma_start(dwb[:], t_dwinR[:, b * BLK:(b + 1) * BLK])
                        xiT = g1.tile([128, mb_in * BLK], BF16, tag="gl_xiT")
                        for w in range(NW_BLK):
                            gw = b * NW_BLK + w
                            twin = g2.tile([128, Cin], BF16, tag="gl_twin")
                            nc.sync.dma_start(twin[:], tab_loc[gw * WIN:(gw + 1) * WIN, :])
                            for cb in range(B):
                                ch = w * B + cb
                                oh2 = g2.tile([128, 128], BF16, tag="gl_oh2")
                                nc.vector.tensor_scalar(
                                    oh2[:], dwb[:, ch * 128:(ch + 1) * 128],
                                    iotap[:, 0:1], None, AOP.is_equal)
                                for kb in range(mb_in):
                                    xp = ps_xp.tile([128, 128], F32, tag="gl_xp", space="PSUM")
                                    nc.tensor.matmul(xp[:], twin[:, kb * 128:(kb + 1) * 128],
                                                     oh2[:], start=True, stop=True)
                                    nc.vector.tensor_copy(
                                        xiT[:, kb * BLK + ch * 128:kb * BLK + (ch + 1) * 128],
                                        xp[:])
                        for mo in range(2):
                            zw = zwp.tile([128, BLK], BF16, tag=f"gl_z{mo}")
                            for sg in range(NSEG):
                                g6 = (b * NSEG + sg) * 6
                                zp = ps.tile([128, 512], F32, tag="gl_zp")
                                for ki in range(mb_in):
                                    nc.tensor.matmul(
                                        zp[:], was[ki * 2 + mo][:],
                                        xiT[:, ki * BLK + sg * 512:ki * BLK + (sg + 1) * 512],
                                        start=(ki == 0), stop=False)
                                for ki in range(mb_in):
                                    nc.tensor.matmul(
                                        zp[:], wbs[ki * 2 + mo][:],
                                        xjT[:, ki * BLK + sg * 512:ki * BLK + (sg + 1) * 512],
                                        start=False, stop=(ki == mb_in - 1))
                                nc.vector.bn_stats(sts[mo][:, g6:g6 + 6], zp[:])
                                nc.scalar.copy(zw[:, sg * 512:(sg + 1) * 512], zp[:])
                            nc.sync.dma_start(zdst[mo, :, b * BLK:(b + 1) * BLK], zw[:])
                for mo in range(2):
                    bn_finish(sts[mo], s_acc[:, mo:mo + 1], q_acc[:, mo:mo + 1],
                              sb, f"glf{mo}")
                return s_acc, q_acc

            # ======================= CONV 2 =======================
            if phases >= 2:
              with tc.tile_pool(name="c2sb", bufs=2) as sb:
                  c2b = [[load_vec(t_c2b[i, mb], sb, f"c2b{i}{mb}") for mb in range(2)]
                         for i in range(2)]
                  c2gn = [[[load_vec(t_c2gn[i, j, mb], sb, f"c2gn{i}{j}{mb}")
                            for mb in range(2)] for j in range(3)] for i in range(2)]
                  sA, qA = gather_layer(tab1, tab1_loc, 128, t_c2wa, t_c2wb, 1,
                                        z_scr[0], sb)
                  stg = allreduce_stats(sA, qA, 2, sb)
                  A1, C1 = affine_from_stats(stg, 2, c2b[0], c2gn[0], sb)

                  s2 = sb.tile([128, 2], F32, tag="c2s2")
                  q2 = sb.tile([128, 2], F32, tag="c2q2")
                  st2s = [sb.tile([128, (E_PAD // 512) * 6], F32, tag=f"c2st{mo}",
                                  name=f"c2st{mo}")
                          for mo in range(2)]
                  zlast = [None, None]
                  with tc.tile_pool(name="c2mid", bufs=2) as mp, \
                       tc.tile_pool(name="c2ps", bufs=2, space="PSUM") as ps:
                      w2s = []
                      for ki in range(2):
                          for mo in range(2):
                              w = sb.tile([128, 128], BF16, tag=f"c2w2{ki}{mo}")
                              nc.sync.dma_start(w[:], t_c2w2[ki, mo])
                              w2s.append(w)
                      for b in range(NBLK):
                          h1 = []
                          for mb in range(2):
                              z = mp.tile([128, BLK], BF16, tag=f"c2z1r{mb}")
                              nc.sync.dma_start(z[:], z_scr[0][mb, :, b * BLK:(b + 1) * BLK])
                              hh = mp.tile([128, BLK], BF16, tag=f"c2h1{mb}")
                              nc.scalar.activation(hh[:], z[:], AFT.Relu,
                                                   bias=C1[mb], scale=A1[mb])
                              h1.append(hh)
                          for mo in range(2):
                              zw = mp.tile([128, BLK], BF16, tag=f"c2z2w{mo}")
                              for s in range(NSEG):
                                  g6 = (b * NSEG + s) * 6
                                  zp = ps.tile([128, 512], F32, tag="c2zp")
                                  for ki in range(2):
                                      nc.tensor.matmul(zp[:], w2s[ki * 2 + mo][:],
                                                       h1[ki][:, s * 512:(s + 1) * 512],
                                                       start=(ki == 0), stop=(ki == 1))
                                  nc.vector.bn_stats(st2s[mo][:, g6:g6 + 6], zp[:])
                                  if s % 2 == 0:
                                      nc.scalar.copy(zw[:, s * 512:(s + 1) * 512],
                                                     zp[:])
                                  else:
                                      nc.vector.tensor_copy(
                                          zw[:, s * 512:(s + 1) * 512], zp[:])
                              nc.sync.dma_start(z_scr[1][mo, :, b * BLK:(b + 1) * BLK], zw[:])
                              zlast[mo] = zw
                      zsent = []
                      for mo in range(2):
                          zc = sb.tile([128, 1], F32, tag=f"c2zs{mo}")
                          nc.vector.tensor_copy(zc[:], zlast[mo][:, BLK - 1:BLK])
                          zsent.append(zc[:])
                  for mo in range(2):
                      bn_finish(st2s[mo], s2[:, mo:mo + 1], q2[:, mo:mo + 1],
                                sb, f"c2f{mo}")
                  sentinel_correct(s2, q2, zsent, 2, sb)
                  stg2 = allreduce_stats(s2, q2, 2, sb)
                  A2, C2 = affine_from_stats(stg2, 2, c2b[1], c2gn[1], sb)
                  scatter_pass(z_scr[1], 2, A2, C2, tab2_loc, 256, ag=tab2)

            if debug:
                nc.sync.dma_start(dbg["x2"][:], tab2[:])

            # ======================= CONV 3 =======================
            if phases >= 3:
              with tc.tile_pool(name="c3sb", bufs=2) as sb:
                  c3b = [load_vec(t_c3b[mb], sb, f"c3b{mb}") for mb in range(2)]
                  c3gn = [[load_vec(t_c3gn[j, mb], sb, f"c3gn{j}{mb}") for mb in range(2)]
                          for j in range(3)]
                  sA, qA = gather_layer(tab2, tab2_loc, 256, t_c3wa, t_c3wb, 2,
                                        z_scr[0], sb)
                  stg = allreduce_stats(sA, qA, 2, sb)
                  A1, C1 = affine_from_stats(stg, 2, c3b, c3gn, sb)
                  scatter_pass(z_scr[0], 2, A1, C1, tab3_loc, 256)

            if debug:
                nc.sync.dma_start(dbg["x3"][:], tab3_loc[:])

            # ======================= POOL + HEAD =======================
            if phases >= 4:
              with tc.tile_pool(name="p_sb", bufs=2) as sb, \
                 tc.tile_pool(name="p_ps", bufs=2, space="PSUM") as ps:
                  pgwl = sb.tile([128, 8 * Bg], F32, tag="p_pgwl")
                  nc.sync.dma_start(pgwl[:], t_pgwl[:])
                  NPG = Bg * 128
                  pidxw = sb.tile([128, 8 * NPG // 16], I16, tag="p_idx16")
                  nc.sync.dma_start(pidxw[:], t_pidx16[:])
                  for gw in range(8):
                      gp = sb.tile([128, Bg, 256], BF16, tag="p_gp")
                      nc.gpsimd.dma_gather(
                          out_ap=gp[:], in_ap=tab3_loc[:],
                          idxs_ap=pidxw[:, gw * (NPG // 16):(gw + 1) * (NPG // 16)],
                          num_idxs=NPG, num_idxs_reg=NPG, elem_size=256,
                          transpose=False, single_packet=(NPG <= 896))
                      pp = ps.tile([128, 256], F32, tag="p_pp", space="PSUM")
                      for c in range(Bg):
                          oh = sb.tile([128, 128], BF16, tag="p_oh")
                          nc.vector.tensor_tensor(
                              out=oh[:],
                              in0=pgwl[:, gw * Bg + c:gw * Bg + c + 1].to_broadcast([128, 128]),
                              in1=iota[:], op=AOP.is_equal)
                          nc.tensor.matmul(pp[:], oh[:], gp[:, c, :],
                                           start=(c == 0), stop=(c == Bg - 1))
                      pf = sb.tile([128, 256], F32, tag="p_pf")
                      nc.vector.tensor_copy(pf[:], pp[:])
                      nc.sync.dma_start(pool_in[gw * 128:(gw + 1) * 128, :], pf[:])
                  nc.gpsimd.collective_compute(
                      "AllReduce", AOP.add, replica_groups=[list(range(NC))],
                      ins=[pool_in.opt()], outs=[pool_out.opt()])
                  if debug:
                      nc.sync.dma_start(dbg["pool"][:], pool_out[:])

                  invg = sb.tile([128, 8], F32, tag="p_invg")
                  nc.sync.dma_start(invg[:], t_invg[:])
                  lw1 = []
                  for ki in range(2):
                      for mo in range(2):
                          w = sb.tile([128, 128], BF16, tag=f"p_lw1{ki}{mo}")
                          nc.sync.dma_start(w[:], t_lw1[ki, mo])
                          lw1.append(w)
                  lw2 = []
                  for ki in range(2):
                      w = sb.tile([128, 2], BF16, tag=f"p_lw2{ki}")
                      nc.sync.dma_start(w[:], t_lw2[ki])
                      lw2.append(w)
                  lb1 = [load_vec(t_lb1[mb], sb, f"p_lb1{mb}") for mb in range(2)]
                  lb2 = sb.tile([2, 1], F32, tag="p_lb2")
                  nc.sync.dma_start(lb2[:], t_lb2[:])
                  ofin = sb.tile([2, 1024], F32, tag="p_out")
                  for gw in range(8):
                      g = sb.tile([128, 256], F32, tag="p_g")
                      nc.sync.dma_start(g[:], pool_out[gw * 128:(gw + 1) * 128, :])
                      gm = sb.tile([128, 256], BF16, tag="p_gm")
                      nc.vector.tensor_scalar(gm[:], g[:], invg[:, gw:gw + 1], None, AOP.mult)
                      gT = sb.tile([128, 2 * 128], BF16, tag="p_gT")
                      for kb in range(2):
                          tp = ps.tile([128, 128], BF16, tag="p_tp", space="PSUM")
                          nc.tensor.transpose(tp[:], gm[:, kb * 128:(kb + 1) * 128], ident[:])
                          nc.vector.tensor_copy(gT[:, kb * 128:(kb + 1) * 128], tp[:])
                      hT = sb.tile([128, 2 * 128], BF16, tag="p_hT")
                      for mo in range(2):
                          hp = ps.tile([128, 128], F32, tag="p_hp", space="PSUM")
                          for ki in range(2):
                              nc.tensor.matmul(hp[:], lw1[ki * 2 + mo][:],
                                               gT[:, ki * 128:(ki + 1) * 128],
                                               start=(ki == 0), stop=(ki == 1))
                          nc.scalar.activation(hT[:, mo * 128:(mo + 1) * 128], hp[:],
                                               AFT.Relu, bias=lb1[mo])
                      op_ = ps.tile([2, 128], F32, tag="p_op", space="PSUM")
                      for ki in range(2):
                          nc.tensor.matmul(op_[:], lw2[ki][:],
                                           hT[:, ki * 128:(ki + 1) * 128],
                                           start=(ki == 0), stop=(ki == 1))
                      nc.vector.tensor_scalar(ofin[:, gw * 128:(gw + 1) * 128],
                                              op_[:], lb2[:], None, AOP.add)
                  nc.sync.dma_start(o_out[:], ofin[:, :N_GRAPHS])

    nc.compile()
    return nc


# ============================ entry point ============================


def kernel(**inputs):
    x = np.asarray(inputs["x"], dtype=np.float32)
    edge_index = np.asarray(inputs["edge_index"])
    batch = np.asarray(inputs["batch"])

    meta = _pack(edge_index, batch)
    Bg = meta["Bg"]

    import os as _os
    phases = int(_os.environ.get("KPHASES", "4"))
    key = ("mod", Bg, phases, _DEBUG[0])
    if key not in _cache:
        _cache[key] = _build(Bg, debug=bool(inputs.get("_debug", False)) or _DEBUG[0],
                             phases=phases)
    nc = _cache[key]

    # ---- per-core input arrays ----
    slot_of_node = meta["slot_of_node"]
    src = np.asarray(edge_index[0], dtype=np.int64)
    dst = np.asarray(edge_index[1], dtype=np.int64)

    # conv1 msgT: [core, 20, E_PAD//2] bf16; edge e<EHALF -> rows 0..9 col e,
    # e>=EHALF -> rows 10..19 col e-EHALF
    EHALF = E_PAD // 2
    xi_v = x[dst]
    xj_v = x[src]
    msg = np.concatenate([xi_v, xj_v - xi_v], axis=1)       # [E, 10]

    # exact conv1-L1 GraphNorm stats on host (tiny 10-dim Gram)
    msg64 = msg.astype(np.float64)
    W1 = np.asarray(inputs["c1_w1"], np.float64)            # [10, 128]
    b1 = np.asarray(inputs["c1_b1"], np.float64)            # [128]
    S = msg64.sum(0)
    G = msg64.T @ msg64
    SW = S @ W1
    qz = np.einsum('ij,ik,kj->j', W1, G, W1) + 2 * b1 * SW + N_EDGES * b1 * b1
    m1 = (SW + N_EDGES * b1) / N_EDGES
    e2 = qz / N_EDGES
    gn1 = np.asarray(inputs["c1_gn1"], np.float64)          # [3, 128]
    msm = gn1[2] * m1
    var1 = e2 - 2 * msm * m1 + msm * msm
    A1h = gn1[0] / np.sqrt(var1 + EPS)
    C1h = gn1[1] + A1h * (b1 - msm)
    c1a_in = np.stack([A1h, C1h]).astype(np.float32).reshape(2, 128, 1)

    msg_full = np.zeros((NC, E_PAD, 10), dtype=np.float32)
    ec, pos = meta["ec"], meta["pos"]
    msg_full[ec, pos] = msg[meta["eorder"]]
    msgT = np.zeros((NC, 48, EHALF), dtype=ml_dtypes.bfloat16)
    msgT[:, :10, :] = _bf(msg_full[:, :EHALF].transpose(0, 2, 1))
    msgT[:, 32:42, :] = _bf(msg_full[:, EHALF:].transpose(0, 2, 1))

    dstwin = meta["dstwin"]  # [NC, E_PAD]
    dwin_in = np.ascontiguousarray(
        dstwin.reshape(NC, E_PAD // 128, 128).transpose(0, 2, 1)).astype(np.float32)
    invcnt_in = np.ascontiguousarray(
        meta["inv_cnt"].reshape(NC, NWIN, 128).transpose(0, 2, 1)).astype(np.float32)
    padcnt_in = np.repeat(meta["pad_cnt"][:, None], 128, axis=1)[:, :, None].astype(np.float32)

    iota_in = np.broadcast_to(np.arange(128, dtype=np.float32)[None, :], (128, 128))
    iota_in = np.ascontiguousarray(iota_in)
    ident_in = np.eye(128, dtype=np.float32).astype(ml_dtypes.bfloat16)
    iotap_in = np.arange(128, dtype=np.float32).reshape(128, 1)
    dwinR_in = np.ascontiguousarray(np.broadcast_to(
        dstwin[:, None, :], (NC, 128, E_PAD))).astype(ml_dtypes.bfloat16)

    xj_row = meta["row_of_slot"][meta["xj_glob"]]  # [NC, E_PAD] chunk-major rows
    xj_in = np.ascontiguousarray(
        xj_row.reshape(NC, E_PAD // 128, 128).transpose(0, 2, 1)).astype(np.int32)

    # weights
    c1w = np.zeros((3, 128, 128), dtype=ml_dtypes.bfloat16)
    c1w[0, :10, :] = _bf(inputs["c1_w1"])
    c1w[0, 32:42, :] = _bf(inputs["c1_w1"])
    c1w[1] = _bf(inputs["c1_w2"])
    c1w[2] = _bf(inputs["c1_w3"])
    c1b = np.stack([np.asarray(inputs[f"c1_b{i}"], dtype=np.float32).reshape(128, 1)
                    for i in (1, 2, 3)])
    c1gn = np.stack([np.asarray(inputs[f"c1_gn{i}"], dtype=np.float32).reshape(3, 128, 1)
                     for i in (1, 2, 3)])

    w2a = np.asarray(inputs["c2_w1"], dtype=np.float32)   # [256, 256]
    WA2 = w2a[:128] - w2a[128:]
    WB2 = w2a[128:]
    c2wa = _tile_w(WA2)[0]                                # [2, 128, 128]
    c2wb = _tile_w(WB2)[0]
    c2w2 = _tile_w(np.asarray(inputs["c2_w2"], dtype=np.float32))  # [2,2,128,128]
    c2b = np.stack([np.asarray(inputs["c2_b1"], dtype=np.float32).reshape(2, 128, 1),
                    np.asarray(inputs["c2_b2"], dtype=np.float32).reshape(2, 128, 1)])
    c2gn = np.stack([np.asarray(inputs["c2_gn1"], dtype=np.float32).reshape(3, 2, 128, 1),
                     np.asarray(inputs["c2_gn2"], dtype=np.float32).reshape(3, 2, 128, 1)])

    w3a = np.asarray(inputs["c3_w1"], dtype=np.float32)   # [512, 256]
    WA3 = w3a[:256] - w3a[256:]
    WB3 = w3a[256:]
    c3wa = _tile_w(WA3)                                   # [2,2,128,128]
    c3wb = _tile_w(WB3)
    c3b = np.asarray(inputs["c3_b1"], dtype=np.float32).reshape(2, 128, 1)
    c3gn = np.asarray(inputs["c3_gn1"], dtype=np.float32).reshape(3, 2, 128, 1)

    lw1 = _tile_w(np.asarray(inputs["lin_w1"], dtype=np.float32))
    lb1 = np.asarray(inputs["lin_b1"], dtype=np.float32).reshape(2, 128, 1)
    lw2_f = np.asarray(inputs["lin_w2"], dtype=np.float32)  # [256, 2]
    lw2 = np.stack([_bf(lw2_f[:128]), _bf(lw2_f[128:])])    # [2, 128, 2]
    lb2 = np.asarray(inputs["lin_b2"], dtype=np.float32).reshape(2, 1)

    Bg0 = meta["Bg"]
    pidx16_in = _wrap_idx(meta["pool_idx"].reshape(NC, 8 * Bg0 * 128))
    pidx16_in = pidx16_in.reshape(NC, 128, -1)
    pgwl = meta["pool_gwl"]                # [NC, 8, NPG]
    Bg_ = meta["Bg"]
    pgwl_in = np.ascontiguousarray(
        pgwl.reshape(NC, 8, Bg_, 128).transpose(0, 3, 1, 2)).reshape(NC, 128, 8 * Bg_)
    invg_in = np.broadcast_to(
        meta["inv_g"].reshape(8, 128).T[None], (NC, 128, 8)).astype(np.float32)
    invg_in = np.ascontiguousarray(invg_in)

    in_maps = []
    for c in range(NC):
        im = {
            "msgT": msgT[c],
            "xj_idx": xj_in[c],
            "dstwin": dwin_in[c],
            "invcnt": invcnt_in[c],
            "padcnt": padcnt_in[c],
            "iota": iota_in,
            "ident": ident_in,
            "iotap": iotap_in,
            "dwinR": dwinR_in[c],
            "c1w": c1w, "c1a": c1a_in, "c1b": c1b, "c1gn": c1gn,
            "c2wa": c2wa, "c2wb": c2wb, "c2w2": c2w2, "c2b": c2b, "c2gn": c2gn,
            "c3wa": c3wa, "c3wb": c3wb, "c3b": c3b, "c3gn": c3gn,
            "lw1": lw1, "lb1": lb1, "lw2": lw2, "lb2": lb2,
            "pidx16": pidx16_in[c],
            "pool_gwl": pgwl_in[c].astype(np.float32),
            "invg": invg_in[c],
        }
        in_maps.append(im)

    res = run_bass_kernel_spmd(nc, in_maps, core_ids=list(range(NC)),
                               trace=_TRACE[0])
    kernel.last_result = res
    kernel.last_meta = meta
    out = res.results[0]["out"]            # [2, 1000]
    return np.ascontiguousarray(out.T).astype(np.float32)


_DEBUG = [False]
_TRACE = [False]



# revision 61
# speedup vs baseline: 1.3520x; 1.0580x over previous
"""LundNetTagger GNN on 8 Trainium2 NeuronCores (Bass/Tile).

Self-contained: kernel(**inputs) -> np.ndarray [1000, 2] float32.

Strategy: nodes are assigned to 100352 "slots" (8 cores x 98 windows x 128),
packed so each window receives <= 512 edges. Edges live on the core owning
their dst slot, in window-major order padded to 4x128-edge chunks per window.
Per-edge MLPs run in bf16 feature-major layout; EdgeConv cat[xi, xj-xi] is
folded into split weights WA = W[:C]-W[C:], WB = W[C:]. GraphNorm stats are
global AllReduces of per-core sums (conv1 layer-1 stats are computed exactly
on the host from the 10-dim message Gram; deeper layers use vector-engine
bn_stats on PSUM with a sentinel pad column for exact correction).
conv1 keeps h fully SBUF-resident (no z spills): layer 2 overwrites h in
place after its stats AllReduce, and layer 3 fuses into the scatter.
Mean-aggregation is a collision-free one-hot matmul scatter into PSUM per
window. Node tables are AllGathered in bf16 between convs in two chunk-major
halves (each half fires as soon as its windows are written, overlapping the
producing scatter); src-side gathers use per-chunk indirect DMA with
chunk-major global row indices.
"""
import numpy as np
import ml_dtypes

import concourse.bass as bass
import concourse.tile as tile
from concourse import bacc, mybir
from concourse.bass_utils import run_bass_kernel_spmd
from concourse import library_config

BF16 = mybir.dt.bfloat16
F32 = mybir.dt.float32
I16 = mybir.dt.int16
AOP = mybir.AluOpType
AFT = mybir.ActivationFunctionType
AX = mybir.AxisListType

N_NODES = 100000
N_EDGES = 400000
N_GRAPHS = 1000
NC = 8
WIN = 128
NWIN = 98
SPC = WIN * NWIN          # 12544
NSLOTS = SPC * NC         # 100352
QUAD = NSLOTS // 4        # 25088
B = 4                     # chunks per window
EPW = B * WIN             # 512
E_PAD = NWIN * EPW        # 50176
EPS = 1e-5

NW_BLK = 7
BLK = NW_BLK * EPW        # 3584
NBLK = NWIN // NW_BLK     # 14
NCHUNK = BLK // 128       # 28
NSEG = BLK // 512         # 7

# window-aligned AllGather chunk boundaries (local rows) and the scatter
# block index after which each chunk's windows are complete
AG_CHB = [0, 25 * WIN, 50 * WIN, 74 * WIN, SPC]   # 3200/3200/3072/3072 rows
AG_FIRE = [4, 8, 11]     # fire chunk k at top of block AG_FIRE[k]; last at end


_cache = {}


# ============================ host-side packing ============================

def _pack(edge_index, batch):
    src = np.asarray(edge_index[0], dtype=np.int64)
    dst = np.asarray(edge_index[1], dtype=np.int64)
    batch = np.asarray(batch, dtype=np.int64)
    cnt = np.bincount(dst, minlength=N_NODES)

    nvirt = NSLOTS - N_NODES
    cnt_all = np.concatenate([cnt, np.zeros(nvirt, dtype=cnt.dtype)])
    order = np.argsort(-cnt_all, kind="stable")
    GW = NWIN * NC
    rounds = NSLOTS // GW
    win_of_rank = np.empty(NSLOTS, dtype=np.int64)
    for r in range(rounds):
        seg = np.arange(GW) if r % 2 == 0 else np.arange(GW - 1, -1, -1)
        win_of_rank[r * GW:(r + 1) * GW] = seg
    win_of_node = np.empty(NSLOTS, dtype=np.int64)
    win_of_node[order] = win_of_rank
    wsum = np.bincount(win_of_node, weights=cnt_all.astype(np.float64),
                       minlength=GW).astype(np.int64)

    cap = EPW
    members_of = [list(np.where(win_of_node == w)[0]) for w in range(GW)]
    for _ in range(2000):
        over = np.where(wsum > cap)[0]
        if len(over) == 0:
            break
        w = int(over[0])
        # smallest-count >0 node in w
        mem = members_of[w]
        cs = [(int(cnt_all[n]), n) for n in mem if cnt_all[n] > 0]
        cs.sort()
        moved = False
        for c1, n in cs:
            # find target window with a smaller-count node to swap
            worder2 = np.argsort(wsum)
            for tw in worder2[:64]:
                tw = int(tw)
                if tw == w:
                    continue
                tmem = members_of[tw]
                best = None
                for m in tmem:
                    c2 = int(cnt_all[m])
                    if c2 < c1 and wsum[tw] + c1 - c2 <= cap:
                        if best is None or c2 < best[0]:
                            best = (c2, m)
                        if c2 == 0:
                            break
                if best is not None:
                    c2, m = best
                    members_of[tw].remove(m)
                    members_of[tw].append(n)
                    members_of[w].remove(n)
                    members_of[w].append(m)
                    win_of_node[n] = tw
                    win_of_node[m] = w
                    wsum[tw] += c1 - c2
                    wsum[w] -= c1 - c2
                    moved = True
                    break
            if moved:
                break
        if not moved:
            raise RuntimeError("packing fixup stuck")
    assert wsum.max() <= cap, f"window packing failed: max={wsum.max()}"

    worder = np.argsort(-wsum, kind="stable")
    core_load = np.zeros(NC, dtype=np.int64)
    core_nwin = np.zeros(NC, dtype=np.int64)
    core_of_win = np.empty(GW, dtype=np.int64)
    for w in worder:
        cands = np.where(core_nwin < NWIN)[0]
        c = cands[np.argmin(core_load[cands])]
        core_of_win[w] = c
        core_load[c] += wsum[w]
        core_nwin[c] += 1

    win_lists = [[] for _ in range(NC)]
    for w in range(GW):
        win_lists[core_of_win[w]].append(w)
    for c in range(NC):
        wl = win_lists[c]
        j = int(np.argmin(wsum[wl]))
        assert wsum[wl[j]] < cap, "no sentinel room"
        wl[j], wl[-1] = wl[-1], wl[j]

    slot_of_node = np.empty(NSLOTS, dtype=np.int64)
    for c in range(NC):
        for wi, w in enumerate(win_lists[c]):
            mem = np.sort(np.array(members_of[w], dtype=np.int64))
            assert len(mem) == WIN
            slot_of_node[mem] = c * SPC + wi * WIN + np.arange(WIN)
    node_of_slot = np.empty(NSLOTS, dtype=np.int64)
    node_of_slot[slot_of_node] = np.arange(NSLOTS)
    cnt_of_slot = cnt_all[node_of_slot]

    qzero = []
    for q in range(4):
        z = np.where(cnt_of_slot[q * QUAD:(q + 1) * QUAD] == 0)[0]
        assert len(z) > 0
        assert z[0] < 32768
        qzero.append(int(z[0]))  # local to quadrant
    czero = []
    for c in range(NC):
        z = np.where(cnt_of_slot[c * SPC:(c + 1) * SPC] == 0)[0]
        assert len(z) > 0
        czero.append(int(z[0]))  # local to core

    dslot = slot_of_node[dst]
    sslot = slot_of_node[src]
    ecore = dslot // SPC
    ewin = (dslot % SPC) // WIN
    key = ecore * (NWIN * WIN) + ewin * WIN + (dslot % WIN)
    eorder = np.argsort(key, kind="stable")
    dsl, ssl = dslot[eorder], sslot[eorder]
    ec, ew = ecore[eorder], ewin[eorder]

    cw = ec * NWIN + ew
    cw_cnt = np.bincount(cw, minlength=NC * NWIN)
    assert cw_cnt.max() <= EPW

    xi_idx = np.zeros((NC, E_PAD), dtype=np.int64)
    xj_idx = np.zeros((NC, E_PAD), dtype=np.int64)
    dstwin = np.full((NC, E_PAD), -1.0, dtype=np.float32)
    valid = np.zeros((NC, E_PAD), dtype=bool)

    ofs = (np.arange(NC * NWIN) % NWIN) * EPW
    start = np.concatenate([[0], np.cumsum(cw_cnt)[:-1]])
    within = np.arange(N_EDGES) - start[cw]
    pos = ofs[cw] + within
    xi_idx[ec, pos] = dsl % SPC
    xj_idx[ec, pos] = ssl
    dstwin[ec, pos] = (dsl % WIN).astype(np.float32)
    valid[ec, pos] = True
    for c in range(NC):
        xi_idx[c, ~valid[c]] = czero[c]
    pad_cnt = (~valid).sum(axis=1).astype(np.float32)
    assert np.all(~valid[:, -1]), "sentinel column must be padding"

    gzero = qzero[0]  # global slot with zero row
    xj_glob = np.where(valid, xj_idx, gzero).astype(np.int32)

    # Chunk-major AllGather table layout: local rows split into 4
    # window-aligned chunks; the full table stores [chunk][core][rows] so
    # each AG chunk output is a contiguous row block.
    sl_ = np.arange(NSLOTS)
    n_, s_ = sl_ // SPC, sl_ % SPC
    c_ = np.searchsorted(np.array(AG_CHB), s_, side="right") - 1
    sizes = np.diff(np.array(AG_CHB))
    base_full = np.concatenate([[0], np.cumsum(sizes * NC)[:-1]])
    row_of_slot = (base_full[c_] + n_ * sizes[c_]
                   + (s_ - np.array(AG_CHB)[c_]))

    inv_cnt = (1.0 / np.maximum(cnt_of_slot.reshape(NC, SPC), 1.0)).astype(np.float32)

    g_of_slot = np.full(NSLOTS, -1, dtype=np.int64)
    real = node_of_slot < N_NODES
    g_of_slot[real] = batch[node_of_slot[real]]
    NGW = 8
    Bg = 0
    pools = [[None] * NGW for _ in range(NC)]
    for c in range(NC):
        gl = g_of_slot[c * SPC:(c + 1) * SPC]
        for gw in range(NGW):
            m = np.where((gl >= gw * 128) & (gl < (gw + 1) * 128))[0]
            pools[c][gw] = m
            Bg = max(Bg, (len(m) + 127) // 128)
    NPG = Bg * 128
    pool_idx = np.zeros((NC, NGW, NPG), dtype=np.int16)
    pool_gwl = np.full((NC, NGW, NPG), -1.0, dtype=np.float32)
    for c in range(NC):
        for gw in range(NGW):
            m = pools[c][gw]
            pool_idx[c, gw, :len(m)] = m.astype(np.int16)
            pool_idx[c, gw, len(m):] = czero[c]
            pool_gwl[c, gw, :len(m)] = (g_of_slot[c * SPC + m] - gw * 128).astype(np.float32)

    gcnt = np.bincount(batch, minlength=N_GRAPHS).astype(np.float32)
    inv_g = np.zeros(1024, dtype=np.float32)
    inv_g[:N_GRAPHS] = 1.0 / np.maximum(gcnt, 1.0)

    return dict(slot_of_node=slot_of_node, node_of_slot=node_of_slot,
                row_of_slot=row_of_slot,
                xj_glob=xj_glob, dstwin=dstwin, pad_cnt=pad_cnt,
                inv_cnt=inv_cnt, valid=valid, eorder=eorder, ec=ec, pos=pos,
                pool_idx=pool_idx, pool_gwl=pool_gwl, inv_g=inv_g, Bg=Bg)


def _wrap_idx(a):
    """[.., n] int -> [.., 128, n//16]: element i -> partition i%16 col i//16,
    replicated to 8 groups of 16 partitions."""
    n = a.shape[-1]
    assert n % 16 == 0
    w = a.reshape(*a.shape[:-1], n // 16, 16)
    w = np.swapaxes(w, -1, -2)
    w = np.broadcast_to(w[..., None, :, :], (*a.shape[:-1], 8, 16, n // 16))
    return np.ascontiguousarray(w).reshape(*a.shape[:-1], 128, n // 16).astype(np.int16)


def _bf(x):
    return np.ascontiguousarray(np.asarray(x, dtype=np.float32)).astype(ml_dtypes.bfloat16)


def _tile_w(w):
    K, M = w.shape
    nk, nm = (K + 127) // 128, (M + 127) // 128
    out = np.zeros((nk, nm, 128, 128), dtype=ml_dtypes.bfloat16)
    for i in range(nk):
        for j in range(nm):
            blk = np.asarray(w, dtype=np.float32)[i * 128:(i + 1) * 128, j * 128:(j + 1) * 128]
            out[i, j, :blk.shape[0], :blk.shape[1]] = _bf(blk)
    return out


# ============================ device kernel ============================

EHALF = E_PAD // 2        # 25088
NSEG_H = EHALF // 512     # 49


def _build(Bg, debug=False, phases=4):
    nc = bacc.Bacc("TRN2", target_bir_lowering=False, debug=False, num_devices=NC)

    def din(name, shape, dt):
        return nc.dram_tensor(name, shape, dt, kind="ExternalInput").ap()

    NIDX = E_PAD // 16
    t_msgT = din("msgT", [48, EHALF], BF16)
    t_xj = din("xj_idx", [128, E_PAD // 128], mybir.dt.int32)
    t_dstwin = din("dstwin", [128, E_PAD // 128], F32)
    t_dwinR = din("dwinR", [128, E_PAD], BF16)
    t_iotap = din("iotap", [128, 1], F32)
    t_invcnt = din("invcnt", [128, NWIN], F32)
    t_padcnt = din("padcnt", [128, 1], F32)
    t_iota = din("iota", [128, 128], F32)
    t_ident = din("ident", [128, 128], BF16)
    t_c1w = din("c1w", [3, 128, 128], BF16)
    t_c1a = din("c1a", [2, 128, 1], F32)
    t_c1b = din("c1b", [3, 128, 1], F32)
    t_c1gn = din("c1gn", [3, 3, 128, 1], F32)
    t_c2wa = din("c2wa", [2, 128, 128], BF16)
    t_c2wb = din("c2wb", [2, 128, 128], BF16)
    t_c2w2 = din("c2w2", [2, 2, 128, 128], BF16)
    t_c2b = din("c2b", [2, 2, 128, 1], F32)
    t_c2gn = din("c2gn", [2, 3, 2, 128, 1], F32)
    t_c3wa = din("c3wa", [2, 2, 128, 128], BF16)
    t_c3wb = din("c3wb", [2, 2, 128, 128], BF16)
    t_c3b = din("c3b", [2, 128, 1], F32)
    t_c3gn = din("c3gn", [3, 2, 128, 1], F32)
    t_lw1 = din("lw1", [2, 2, 128, 128], BF16)
    t_lb1 = din("lb1", [2, 128, 1], F32)
    t_lw2 = din("lw2", [2, 128, 2], BF16)
    t_lb2 = din("lb2", [2, 1], F32)
    t_pidx16 = din("pidx16", [128, 8 * Bg * 128 // 16], I16)
    t_pgwl = din("pool_gwl", [128, 8 * Bg], F32)
    t_invg = din("invg", [128, 8], F32)

    o_out = nc.dram_tensor("out", [2, N_GRAPHS], F32, kind="ExternalOutput").ap()
    dbg = {}
    if debug:
        dbg["x1"] = nc.dram_tensor("dbg_x1", [NSLOTS, 128], BF16, kind="ExternalOutput").ap()
        dbg["x2"] = nc.dram_tensor("dbg_x2", [NSLOTS, 256], BF16, kind="ExternalOutput").ap()
        dbg["x3"] = nc.dram_tensor("dbg_x3", [SPC, 256], BF16, kind="ExternalOutput").ap()
        dbg["pool"] = nc.dram_tensor("dbg_pool", [1024, 256], F32, kind="ExternalOutput").ap()

    with tile.TileContext(nc) as tc:
        with tc.tile_pool(name="dram", bufs=1, space="DRAM") as dram, \
             tc.tile_pool(name="cp", bufs=1) as cp:
            z_scr = [dram.tile([2, 128, E_PAD], BF16, tag=f"zscr{i}", name=f"zscr{i}") for i in range(2)]
            tab1_loc = dram.tile([SPC, 128], BF16)
            tab1 = dram.tile([NSLOTS, 128], BF16)
            tab2_loc = dram.tile([SPC, 256], BF16)
            tab2 = dram.tile([NSLOTS, 256], BF16)
            tab3_loc = dram.tile([SPC, 256], BF16)
            st_in = dram.tile([128, 8], F32)
            st_out = dram.tile([128, 8], F32)
            pool_in = dram.tile([1024, 256], F32)
            pool_out = dram.tile([1024, 256], F32)

            ident = cp.tile([128, 128], BF16)
            nc.sync.dma_start(ident[:], t_ident[:])
            iota = cp.tile([128, 128], F32)
            nc.sync.dma_start(iota[:], t_iota[:])
            invcnt = cp.tile([128, NWIN], F32)
            nc.sync.dma_start(invcnt[:], t_invcnt[:])
            dwin = cp.tile([128, E_PAD // 128], F32)
            nc.sync.dma_start(dwin[:], t_dstwin[:])
            padcnt = cp.tile([128, 1], F32)
            nc.sync.dma_start(padcnt[:], t_padcnt[:])
            iotap = cp.tile([128, 1], F32)
            nc.sync.dma_start(iotap[:], t_iotap[:])

            # ---------- helpers ----------
            def allreduce_stats(s_acc, q_acc, n_mb, sb):
                st = sb.tile([128, 8], F32, tag="st_")
                nc.vector.memset(st[:], 0.0)
                nc.vector.tensor_copy(st[:, 0:n_mb], s_acc[:])
                nc.vector.tensor_copy(st[:, 4:4 + n_mb], q_acc[:])
                nc.sync.dma_start(st_in[:], st[:])
                nc.gpsimd.collective_compute(
                    "AllReduce", AOP.add, replica_groups=[list(range(NC))],
                    ins=[st_in.opt()], outs=[st_out.opt()])
                stg = sb.tile([128, 8], F32, tag="stg_")
                nc.sync.dma_start(stg[:], st_out[:])
                return stg

            def affine_from_stats(stg, n_mb, b_lin, gn, sb):
                A, Cc = [], []
                for mb in range(n_mb):
                    s = stg[:, mb:mb + 1]
                    q = stg[:, 4 + mb:5 + mb]
                    g, bgn, ms = gn[0][mb], gn[1][mb], gn[2][mb]
                    bl = b_lin[mb]
                    m = sb.tile([128, 1], F32, tag="af_m")
                    nc.vector.tensor_scalar(m[:], s, 1.0 / N_EDGES, None, AOP.mult)
                    nc.vector.tensor_tensor(m[:], m[:], bl, op=AOP.add)
                    e2 = sb.tile([128, 1], F32, tag="af_e2")
                    nc.vector.tensor_scalar(e2[:], q, 1.0 / N_EDGES, None, AOP.mult)
                    tmp = sb.tile([128, 1], F32, tag="af_t")
                    nc.vector.tensor_tensor(tmp[:], m[:], bl, op=AOP.mult)
                    nc.vector.tensor_scalar(tmp[:], tmp[:], 2.0, None, AOP.mult)
                    nc.vector.tensor_tensor(e2[:], e2[:], tmp[:], op=AOP.add)
                    nc.vector.tensor_tensor(tmp[:], bl, bl, op=AOP.mult)
                    nc.vector.tensor_tensor(e2[:], e2[:], tmp[:], op=AOP.subtract)
                    msm = sb.tile([128, 1], F32, tag="af_msm")
                    nc.vector.tensor_tensor(msm[:], ms, m[:], op=AOP.mult)
                    var = sb.tile([128, 1], F32, tag="af_v")
                    nc.vector.tensor_tensor(var[:], msm[:], msm[:], op=AOP.mult)
                    nc.vector.tensor_tensor(tmp[:], msm[:], m[:], op=AOP.mult)
                    nc.vector.tensor_scalar(tmp[:], tmp[:], 2.0, None, AOP.mult)
                    nc.vector.tensor_tensor(var[:], var[:], tmp[:], op=AOP.subtract)
                    nc.vector.tensor_tensor(var[:], var[:], e2[:], op=AOP.add)
                    a = sb.tile([128, 1], F32, tag="af_a")
                    nc.vector.tensor_scalar(var[:], var[:], EPS, None, AOP.add)
                    nc.scalar.activation(a[:], var[:], AFT.Sqrt)
                    nc.vector.reciprocal(a[:], a[:])
                    nc.vector.tensor_tensor(a[:], a[:], g, op=AOP.mult)
                    cc = sb.tile([128, 1], F32, tag="af_c")
                    nc.vector.tensor_tensor(cc[:], bl, msm[:], op=AOP.subtract)
                    nc.vector.tensor_tensor(cc[:], cc[:], a[:], op=AOP.mult)
                    nc.vector.tensor_tensor(cc[:], cc[:], bgn, op=AOP.add)
                    A.append(a)
                    Cc.append(cc)
                return A, Cc

            def acc_stats(ps_ap, s_col, q_col, sb):
                t1 = sb.tile([128, 1], F32, tag="rs_t1")
                nc.vector.reduce_sum(out=t1[:], in_=ps_ap, axis=AX.X)
                nc.vector.tensor_tensor(s_col, s_col, t1[:], op=AOP.add)
                n = ps_ap.shape[-1]
                sq = sb.tile([128, 512], BF16, tag="rs_sq")
                qa = sb.tile([128, 1], F32, tag="rs_qa")
                nc.scalar.activation(sq[:, :n], ps_ap, AFT.Square, accum_out=qa[:])
                nc.vector.tensor_tensor(q_col, q_col, qa[:], op=AOP.add)

            def bn_finish(st, s_col, q_col, sb, tag):
                # bn_stats 6-tuples (equal 512-col groups) -> sum / sq-sum
                agg = sb.tile([128, 2], F32, tag=tag + "g")
                nc.vector.bn_aggr(agg[:], st[:])
                nc.vector.tensor_scalar(s_col, agg[:, 0:1], float(E_PAD),
                                        None, AOP.mult)
                t = sb.tile([128, 1], F32, tag=tag + "t")
                nc.vector.tensor_tensor(t[:], agg[:, 0:1], agg[:, 0:1], op=AOP.mult)
                nc.vector.tensor_tensor(t[:], t[:], agg[:, 1:2], op=AOP.add)
                nc.vector.tensor_scalar(q_col, t[:], float(E_PAD), None, AOP.mult)

            def sentinel_correct(s_acc, q_acc, zsent_cols, n_mb, sb):
                for mb in range(n_mb):
                    zs = zsent_cols[mb]
                    t1 = sb.tile([128, 1], F32, tag="sc_t1")
                    nc.vector.tensor_tensor(t1[:], zs, padcnt[:], op=AOP.mult)
                    nc.vector.tensor_tensor(s_acc[:, mb:mb + 1], s_acc[:, mb:mb + 1],
                                            t1[:], op=AOP.subtract)
                    nc.vector.tensor_tensor(t1[:], zs, zs, op=AOP.mult)
                    nc.vector.tensor_tensor(t1[:], t1[:], padcnt[:], op=AOP.mult)
                    nc.vector.tensor_tensor(q_acc[:, mb:mb + 1], q_acc[:, mb:mb + 1],
                                            t1[:], op=AOP.subtract)

            def load_vec(t_ap, sb, tag):
                v = sb.tile([128, 1], F32, tag=tag)
                nc.sync.dma_start(v[:], t_ap)
                return v[:]

            AG_BASE = [0]
            for _c in range(3):
                AG_BASE.append(AG_BASE[-1] + NC * (AG_CHB[_c + 1] - AG_CHB[_c]))

            def fire_ag(tab_loc, tab_full, c):
                # chunk-major table: AG chunk c is a contiguous row block
                lo, hi = AG_CHB[c], AG_CHB[c + 1]
                nc.gpsimd.collective_compute(
                    "AllGather", AOP.bypass, replica_groups=[list(range(NC))],
                    ins=[tab_loc[lo:hi, :].opt()],
                    outs=[tab_full[AG_BASE[c]:AG_BASE[c] + NC * (hi - lo),
                                   :].opt()])

            def scatter_pass(zsrc, n_mb, A, Cc, tab_loc, Cout, ag=None):
                with tc.tile_pool(name="sc_sb", bufs=2) as sb, \
                     tc.tile_pool(name="sc_tp", bufs=2, space="PSUM") as ps_tp, \
                     tc.tile_pool(name="sc_sc", bufs=2, space="PSUM") as ps_sc:
                    for b in range(NBLK):
                        if ag is not None and b in AG_FIRE:
                            fire_ag(tab_loc, ag, AG_FIRE.index(b))
                        hs = []
                        for mb in range(n_mb):
                            z = sb.tile([128, BLK], BF16, tag=f"sp_z{mb}")
                            nc.sync.dma_start(z[:], zsrc[mb, :, b * BLK:(b + 1) * BLK])
                            h = sb.tile([128, BLK], BF16, tag=f"sp_h{mb}")
                            nc.scalar.activation(h[:], z[:], AFT.Relu,
                                                 bias=Cc[mb], scale=A[mb])
                            hs.append(h)
                        hE = sb.tile([128, NCHUNK * Cout], BF16, tag="sp_hE")
                        for ch in range(NCHUNK):
                            for mb in range(n_mb):
                                tp = ps_tp.tile([128, 128], BF16, tag="sp_tp", space="PSUM")
                                nc.tensor.transpose(tp[:], hs[mb][:, ch * 128:(ch + 1) * 128],
                                                    ident[:])
                                nc.vector.tensor_copy(
                                    hE[:, ch * Cout + mb * 128:ch * Cout + (mb + 1) * 128],
                                    tp[:])
                        for w in range(NW_BLK):
                            gw = b * NW_BLK + w
                            sc = ps_sc.tile([128, Cout], F32, tag="sp_sc", space="PSUM")
                            for cb in range(B):
                                ch = w * B + cb
                                col = b * NCHUNK + ch
                                oh = sb.tile([128, 128], BF16, tag="sp_oh")
                                nc.vector.tensor_tensor(
                                    out=oh[:],
                                    in0=dwin[:, col:col + 1].to_broadcast([128, 128]),
                                    in1=iota[:], op=AOP.is_equal)
                                nc.tensor.matmul(sc[:], oh[:],
                                                 hE[:, ch * Cout:(ch + 1) * Cout],
                                                 start=(cb == 0), stop=(cb == B - 1))
                            nt = sb.tile([128, Cout], BF16, tag="sp_nt")
                            nc.vector.tensor_scalar(nt[:], sc[:], invcnt[:, gw:gw + 1],
                                                    None, AOP.mult)
                            nc.sync.dma_start(tab_loc[gw * WIN:(gw + 1) * WIN, :], nt[:])
                    if ag is not None:
                        fire_ag(tab_loc, ag, 3)

            # ======================= CONV 1 =======================
            # SBUF-resident: L1 stats precomputed on host; h kept on-chip,
            # L2 overwrites it in place; L3 fused with the scatter.
            NSEG_T = E_PAD // 512  # 98
            with tc.tile_pool(name="c1sb", bufs=2) as sb:
                c1b = [[load_vec(t_c1b[i], sb, f"c1b{i}")] for i in range(3)]
                c1gn = [[[load_vec(t_c1gn[i, j], sb, f"c1gn{i}{j}")] for j in range(3)]
                        for i in range(3)]
                A1h = load_vec(t_c1a[0], sb, "c1a0")
                C1h = load_vec(t_c1a[1], sb, "c1a1")
                with tc.tile_pool(name="c1h", bufs=1) as hp, \
                     tc.tile_pool(name="c1ps", bufs=2, space="PSUM") as ps, \
                     tc.tile_pool(name="c1p2", bufs=2, space="PSUM") as ps2:
                    c1w = []
                    for i in range(3):
                        w = sb.tile([128, 128], BF16, tag=f"c1w{i}")
                        nc.sync.dma_start(w[:], t_c1w[i])
                        c1w.append(w)
                    msgT = hp.tile([48, EHALF], BF16, tag="msgT")
                    nc.sync.dma_start(msgT[:], t_msgT[:])
                    h_full = hp.tile([128, E_PAD], BF16)

                    def bn_to_sq(st, tag):
                        agg = sb.tile([128, 2], F32, tag=tag + "agg")
                        nc.vector.bn_aggr(agg[:], st[:])
                        s_acc = sb.tile([128, 1], F32, tag=tag + "s")
                        q_acc = sb.tile([128, 1], F32, tag=tag + "q")
                        nc.vector.tensor_scalar(s_acc[:], agg[:, 0:1],
                                                float(E_PAD), None, AOP.mult)
                        nc.vector.tensor_tensor(q_acc[:], agg[:, 0:1], agg[:, 0:1],
                                                op=AOP.mult)
                        nc.vector.tensor_tensor(q_acc[:], q_acc[:], agg[:, 1:2],
                                                op=AOP.add)
                        nc.vector.tensor_scalar(q_acc[:], q_acc[:],
                                                float(E_PAD), None, AOP.mult)
                        return s_acc, q_acc

                    # pass 1: L1 -> h_full; L2 stats
                    st2 = hp.tile([128, NSEG_T * 6], F32, tag="st2")
                    zs2 = sb.tile([128, 1], F32, tag="zs2")
                    for g in range(NSEG_T):
                        hh, shalf = g // NSEG_H, g % NSEG_H
                        zp = ps.tile([128, 512], F32, tag="zp")
                        nc.tensor.matmul(zp[:], c1w[0][32 * hh:32 * hh + 10, :],
                                         msgT[32 * hh:32 * hh + 10,
                                              shalf * 512:(shalf + 1) * 512],
                                         start=True, stop=True)
                        nc.scalar.activation(h_full[:, g * 512:(g + 1) * 512], zp[:],
                                             AFT.Relu, bias=C1h, scale=A1h)
                        zp2 = ps2.tile([128, 512], F32, tag="zp2")
                        nc.tensor.matmul(zp2[:], c1w[1][:],
                                         h_full[:, g * 512:(g + 1) * 512],
                                         start=True, stop=True)
                        nc.vector.bn_stats(st2[:, g * 6:(g + 1) * 6], zp2[:])
                        if g == NSEG_T - 1:
                            nc.vector.tensor_copy(zs2[:], zp2[:, 511:512])
                    s2, q2 = bn_to_sq(st2, "b2")
                    sentinel_correct(s2, q2, [zs2[:]], 1, sb)
                    stg2 = allreduce_stats(s2, q2, 1, sb)
                    A2, C2 = affine_from_stats(stg2, 1, c1b[1], c1gn[1], sb)

                    # pass 2: L2 -> h_full (in place); L3 stats
                    st3 = hp.tile([128, NSEG_T * 6], F32, tag="st3")
                    zs3 = sb.tile([128, 1], F32, tag="zs3")
                    for g in range(NSEG_T):
                        zp = ps.tile([128, 512], F32, tag="zp")
                        nc.tensor.matmul(zp[:], c1w[1][:],
                                         h_full[:, g * 512:(g + 1) * 512],
                                         start=True, stop=True)
                        nc.scalar.activation(h_full[:, g * 512:(g + 1) * 512], zp[:],
                                             AFT.Relu, bias=C2[0], scale=A2[0])
                        zp3 = ps2.tile([128, 512], F32, tag="zp2")
                        nc.tensor.matmul(zp3[:], c1w[2][:],
                                         h_full[:, g * 512:(g + 1) * 512],
                                         start=True, stop=True)
                        nc.vector.bn_stats(st3[:, g * 6:(g + 1) * 6], zp3[:])
                        if g == NSEG_T - 1:
                            nc.vector.tensor_copy(zs3[:], zp3[:, 511:512])
                    s3, q3 = bn_to_sq(st3, "b3")
                    sentinel_correct(s3, q3, [zs3[:]], 1, sb)
                    stg3 = allreduce_stats(s3, q3, 1, sb)
                    A3, C3 = affine_from_stats(stg3, 1, c1b[2], c1gn[2], sb)

                    # pass 3: L3 + fused scatter
                    with tc.tile_pool(name="c1sc", bufs=2) as scb, \
                         tc.tile_pool(name="c1tp", bufs=2, space="PSUM") as ps_tp, \
                         tc.tile_pool(name="c1s2", bufs=2, space="PSUM") as ps_sc:
                        for b in range(NBLK):
                            if b in AG_FIRE:
                                fire_ag(tab1_loc, tab1, AG_FIRE.index(b))
                            h3 = scb.tile([128, BLK], BF16, tag="c1h3")
                            for s in range(NSEG):
                                g = b * NSEG + s
                                zp = ps.tile([128, 512], F32, tag="zp")
                                nc.tensor.matmul(zp[:], c1w[2][:],
                                                 h_full[:, g * 512:(g + 1) * 512],
                                                 start=True, stop=True)
                                nc.scalar.activation(h3[:, s * 512:(s + 1) * 512],
                                                     zp[:], AFT.Relu,
                                                     bias=C3[0], scale=A3[0])
                            hE = scb.tile([128, NCHUNK * 128], BF16, tag="c1hE")
                            for ch in range(NCHUNK):
                                tp = ps_tp.tile([128, 128], BF16, tag="c1tp",
                                                space="PSUM")
                                nc.tensor.transpose(tp[:], h3[:, ch * 128:(ch + 1) * 128],
                                                    ident[:])
                                nc.vector.tensor_copy(hE[:, ch * 128:(ch + 1) * 128],
                                                      tp[:])
                            for w in range(NW_BLK):
                                gw = b * NW_BLK + w
                                sc = ps_sc.tile([128, 128], F32, tag="c1sc",
                                                space="PSUM")
                                for cb in range(B):
                                    ch = w * B + cb
                                    col = b * NCHUNK + ch
                                    oh = scb.tile([128, 128], BF16, tag="c1oh")
                                    nc.vector.tensor_tensor(
                                        out=oh[:],
                                        in0=dwin[:, col:col + 1].to_broadcast([128, 128]),
                                        in1=iota[:], op=AOP.is_equal)
                                    nc.tensor.matmul(sc[:], oh[:],
                                                     hE[:, ch * 128:(ch + 1) * 128],
                                                     start=(cb == 0), stop=(cb == B - 1))
                                nt = scb.tile([128, 128], BF16, tag="c1nt")
                                nc.vector.tensor_scalar(nt[:], sc[:],
                                                        invcnt[:, gw:gw + 1],
                                                        None, AOP.mult)
                                nc.sync.dma_start(tab1_loc[gw * WIN:(gw + 1) * WIN, :],
                                                  nt[:])
                        fire_ag(tab1_loc, tab1, 3)

            if debug:
                nc.sync.dma_start(dbg["x1"][:], tab1[:])

            # ============== gather-based first layer (conv2/conv3) ==============
            def gather_layer(tab_full, tab_loc, Cin, wa_t, wb_t, n_kb, zdst, sb):
                mb_in = Cin // 128
                s_acc = sb.tile([128, 2], F32, tag="gl_s")
                q_acc = sb.tile([128, 2], F32, tag="gl_q")
                sts = [sb.tile([128, (E_PAD // 512) * 6], F32, tag=f"gl_st{mo}",
                               name=f"gl_st{mo}")
                       for mo in range(2)]
                with tc.tile_pool(name="gl_g2", bufs=3) as g2, \
                     tc.tile_pool(name="gl_g1", bufs=2) as g1, \
                     tc.tile_pool(name="gl_zw", bufs=2) as zwp, \
                     tc.tile_pool(name="gl_ps", bufs=2, space="PSUM") as ps, \
                     tc.tile_pool(name="gl_tp", bufs=2, space="PSUM") as ps_tp, \
                     tc.tile_pool(name="gl_xp", bufs=2, space="PSUM") as ps_xp:
                    was, wbs = [], []
                    for ki in range(n_kb):
                        for mo in range(2):
                            wta = sb.tile([128, 128], BF16, tag=f"gl_wa{ki}{mo}")
                            nc.sync.dma_start(wta[:], wa_t[ki, mo] if n_kb > 1 else wa_t[mo])
                            was.append(wta)
                            wtb = sb.tile([128, 128], BF16, tag=f"gl_wb{ki}{mo}")
                            nc.sync.dma_start(wtb[:], wb_t[ki, mo] if n_kb > 1 else wb_t[mo])
                            wbs.append(wtb)
                    ixj = sb.tile([128, NBLK * NCHUNK], mybir.dt.int32,
                                  tag="gl_ixj")
                    nc.sync.dma_start(ixj[:], t_xj[:])
                    for b in range(NBLK):
                        gxj = g2.tile([128, NCHUNK * Cin], BF16, tag="gl_gxj")
                        for ch in range(NCHUNK):
                            gch = b * NCHUNK + ch
                            nc.gpsimd.indirect_dma_start(
                                out=gxj[:, ch * Cin:(ch + 1) * Cin],
                                out_offset=None,
                                in_=tab_full[:],
                                in_offset=bass.IndirectOffsetOnAxis(
                                    ap=ixj[:, gch:gch + 1], axis=0))
                        xjT = g1.tile([128, mb_in * BLK], BF16, tag="gl_xjT")
                        for ch in range(NCHUNK):
                            for kb in range(mb_in):
                                tp2 = ps_tp.tile([128, 128], BF16, tag="gl_ohp",
                                                 space="PSUM")
                                nc.tensor.transpose(
                                    tp2[:],
                                    gxj[:, ch * Cin + kb * 128:ch * Cin + (kb + 1) * 128],
                                    ident[:])
                                dst = xjT[:, kb * BLK + ch * 128:
                                          kb * BLK + (ch + 1) * 128]
                                if mb_in == 2 and kb == 0:
                                    nc.scalar.copy(dst, tp2[:])
                                else:
                                    nc.vector.tensor_copy(dst, tp2[:])
                        # xi via window expansion (transposed one-hot built
                        # directly from the replicated dstwin row)
                        dwb = g1.tile([128, BLK], BF16, tag="gl_dwb")
                        nc.sync.dma_start(dwb[:], t_dwinR[:, b * BLK:(b + 1) * BLK])
                        xiT = g1.tile([128, mb_in * BLK], BF16, tag="gl_xiT")
                        for w in range(NW_BLK):
                            gw = b * NW_BLK + w
                            twin = g2.tile([128, Cin], BF16, tag="gl_twin")
                            nc.sync.dma_start(twin[:], tab_loc[gw * WIN:(gw + 1) * WIN, :])
                            for cb in range(B):
                                ch = w * B + cb
                                oh2 = g2.tile([128, 128], BF16, tag="gl_oh2")
                                nc.vector.tensor_scalar(
                                    oh2[:], dwb[:, ch * 128:(ch + 1) * 128],
                                    iotap[:, 0:1], None, AOP.is_equal)
                                for kb in range(mb_in):
                                    xp = ps_xp.tile([128, 128], F32, tag="gl_xp", space="PSUM")
                                    nc.tensor.matmul(xp[:], twin[:, kb * 128:(kb + 1) * 128],
                                                     oh2[:], start=True, stop=True)
                                    nc.vector.tensor_copy(
                                        xiT[:, kb * BLK + ch * 128:kb * BLK + (ch + 1) * 128],
                                        xp[:])
                        for mo in range(2):
                            zw = zwp.tile([128, BLK], BF16, tag=f"gl_z{mo}")
                            for sg in range(NSEG):
                                g6 = (b * NSEG + sg) * 6
                                zp = ps.tile([128, 512], F32, tag="gl_zp")
                                for ki in range(mb_in):
                                    nc.tensor.matmul(
                                        zp[:], was[ki * 2 + mo][:],
                                        xiT[:, ki * BLK + sg * 512:ki * BLK + (sg + 1) * 512],
                                        start=(ki == 0), stop=False)
                                for ki in range(mb_in):
                                    nc.tensor.matmul(
                                        zp[:], wbs[ki * 2 + mo][:],
                                        xjT[:, ki * BLK + sg * 512:ki * BLK + (sg + 1) * 512],
                                        start=False, stop=(ki == mb_in - 1))
                                nc.vector.bn_stats(sts[mo][:, g6:g6 + 6], zp[:])
                                nc.scalar.copy(zw[:, sg * 512:(sg + 1) * 512], zp[:])
                            nc.sync.dma_start(zdst[mo, :, b * BLK:(b + 1) * BLK], zw[:])
                for mo in range(2):
                    bn_finish(sts[mo], s_acc[:, mo:mo + 1], q_acc[:, mo:mo + 1],
                              sb, f"glf{mo}")
                return s_acc, q_acc

            # ======================= CONV 2 =======================
            if phases >= 2:
              with tc.tile_pool(name="c2sb", bufs=2) as sb:
                  c2b = [[load_vec(t_c2b[i, mb], sb, f"c2b{i}{mb}") for mb in range(2)]
                         for i in range(2)]
                  c2gn = [[[load_vec(t_c2gn[i, j, mb], sb, f"c2gn{i}{j}{mb}")
                            for mb in range(2)] for j in range(3)] for i in range(2)]
                  sA, qA = gather_layer(tab1, tab1_loc, 128, t_c2wa, t_c2wb, 1,
                                        z_scr[0], sb)
                  stg = allreduce_stats(sA, qA, 2, sb)
                  A1, C1 = affine_from_stats(stg, 2, c2b[0], c2gn[0], sb)

                  s2 = sb.tile([128, 2], F32, tag="c2s2")
                  q2 = sb.tile([128, 2], F32, tag="c2q2")
                  st2s = [sb.tile([128, (E_PAD // 512) * 6], F32, tag=f"c2st{mo}",
                                  name=f"c2st{mo}")
                          for mo in range(2)]
                  zlast = [None, None]
                  with tc.tile_pool(name="c2mid", bufs=2) as mp, \
                       tc.tile_pool(name="c2ps", bufs=2, space="PSUM") as ps:
                      w2s = []
                      for ki in range(2):
                          for mo in range(2):
                              w = sb.tile([128, 128], BF16, tag=f"c2w2{ki}{mo}")
                              nc.sync.dma_start(w[:], t_c2w2[ki, mo])
                              w2s.append(w)
                      for b in range(NBLK):
                          h1 = []
                          for mb in range(2):
                              z = mp.tile([128, BLK], BF16, tag=f"c2z1r{mb}")
                              nc.sync.dma_start(z[:], z_scr[0][mb, :, b * BLK:(b + 1) * BLK])
                              hh = mp.tile([128, BLK], BF16, tag=f"c2h1{mb}")
                              nc.scalar.activation(hh[:], z[:], AFT.Relu,
                                                   bias=C1[mb], scale=A1[mb])
                              h1.append(hh)
                          for mo in range(2):
                              zw = mp.tile([128, BLK], BF16, tag=f"c2z2w{mo}")
                              for s in range(NSEG):
                                  g6 = (b * NSEG + s) * 6
                                  zp = ps.tile([128, 512], F32, tag="c2zp")
                                  for ki in range(2):
                                      nc.tensor.matmul(zp[:], w2s[ki * 2 + mo][:],
                                                       h1[ki][:, s * 512:(s + 1) * 512],
                                                       start=(ki == 0), stop=(ki == 1))
                                  nc.vector.bn_stats(st2s[mo][:, g6:g6 + 6], zp[:])
                                  if s % 2 == 0:
                                      nc.scalar.copy(zw[:, s * 512:(s + 1) * 512],
                                                     zp[:])
                                  else:
                                      nc.vector.tensor_copy(
                                          zw[:, s * 512:(s + 1) * 512], zp[:])
                              nc.sync.dma_start(z_scr[1][mo, :, b * BLK:(b + 1) * BLK], zw[:])
                              zlast[mo] = zw
                      zsent = []
                      for mo in range(2):
                          zc = sb.tile([128, 1], F32, tag=f"c2zs{mo}")
                          nc.vector.tensor_copy(zc[:], zlast[mo][:, BLK - 1:BLK])
                          zsent.append(zc[:])
                  for mo in range(2):
                      bn_finish(st2s[mo], s2[:, mo:mo + 1], q2[:, mo:mo + 1],
                                sb, f"c2f{mo}")
                  sentinel_correct(s2, q2, zsent, 2, sb)
                  stg2 = allreduce_stats(s2, q2, 2, sb)
                  A2, C2 = affine_from_stats(stg2, 2, c2b[1], c2gn[1], sb)
                  scatter_pass(z_scr[1], 2, A2, C2, tab2_loc, 256, ag=tab2)

            if debug:
                nc.sync.dma_start(dbg["x2"][:], tab2[:])

            # ======================= CONV 3 =======================
            if phases >= 3:
              with tc.tile_pool(name="c3sb", bufs=2) as sb:
                  c3b = [load_vec(t_c3b[mb], sb, f"c3b{mb}") for mb in range(2)]
                  c3gn = [[load_vec(t_c3gn[j, mb], sb, f"c3gn{j}{mb}") for mb in range(2)]
                          for j in range(3)]
                  sA, qA = gather_layer(tab2, tab2_loc, 256, t_c3wa, t_c3wb, 2,
                                        z_scr[0], sb)
                  stg = allreduce_stats(sA, qA, 2, sb)
                  A1, C1 = affine_from_stats(stg, 2, c3b, c3gn, sb)
                  scatter_pass(z_scr[0], 2, A1, C1, tab3_loc, 256)

            if debug:
                nc.sync.dma_start(dbg["x3"][:], tab3_loc[:])

            # ======================= POOL + HEAD =======================
            if phases >= 4:
              with tc.tile_pool(name="p_sb", bufs=2) as sb, \
                 tc.tile_pool(name="p_ps", bufs=2, space="PSUM") as ps:
                  pgwl = sb.tile([128, 8 * Bg], F32, tag="p_pgwl")
                  nc.sync.dma_start(pgwl[:], t_pgwl[:])
                  NPG = Bg * 128
                  pidxw = sb.tile([128, 8 * NPG // 16], I16, tag="p_idx16")
                  nc.sync.dma_start(pidxw[:], t_pidx16[:])
                  for gw in range(8):
                      gp = sb.tile([128, Bg, 256], BF16, tag="p_gp")
                      nc.gpsimd.dma_gather(
                          out_ap=gp[:], in_ap=tab3_loc[:],
                          idxs_ap=pidxw[:, gw * (NPG // 16):(gw + 1) * (NPG // 16)],
                          num_idxs=NPG, num_idxs_reg=NPG, elem_size=256,
                          transpose=False, single_packet=(NPG <= 896))
                      pp = ps.tile([128, 256], F32, tag="p_pp", space="PSUM")
                      for c in range(Bg):
                          oh = sb.tile([128, 128], BF16, tag="p_oh")
                          nc.vector.tensor_tensor(
                              out=oh[:],
                              in0=pgwl[:, gw * Bg + c:gw * Bg + c + 1].to_broadcast([128, 128]),
                              in1=iota[:], op=AOP.is_equal)
                          nc.tensor.matmul(pp[:], oh[:], gp[:, c, :],
                                           start=(c == 0), stop=(c == Bg - 1))
                      pf = sb.tile([128, 256], F32, tag="p_pf")
                      nc.vector.tensor_copy(pf[:], pp[:])
                      nc.sync.dma_start(pool_in[gw * 128:(gw + 1) * 128, :], pf[:])
                  nc.gpsimd.collective_compute(
                      "AllReduce", AOP.add, replica_groups=[list(range(NC))],
                      ins=[pool_in.opt()], outs=[pool_out.opt()])
                  if debug:
                      nc.sync.dma_start(dbg["pool"][:], pool_out[:])

                  invg = sb.tile([128, 8], F32, tag="p_invg")
                  nc.sync.dma_start(invg[:], t_invg[:])
                  lw1 = []
                  for ki in range(2):
                      for mo in range(2):
                          w = sb.tile([128, 128], BF16, tag=f"p_lw1{ki}{mo}")
                          nc.sync.dma_start(w[:], t_lw1[ki, mo])
                          lw1.append(w)
                  lw2 = []
                  for ki in range(2):
                      w = sb.tile([128, 2], BF16, tag=f"p_lw2{ki}")
                      nc.sync.dma_start(w[:], t_lw2[ki])
                      lw2.append(w)
                  lb1 = [load_vec(t_lb1[mb], sb, f"p_lb1{mb}") for mb in range(2)]
                  lb2 = sb.tile([2, 1], F32, tag="p_lb2")
                  nc.sync.dma_start(lb2[:], t_lb2[:])
                  ofin = sb.tile([2, 1024], F32, tag="p_out")
                  for gw in range(8):
                      g = sb.tile([128, 256], F32, tag="p_g")
                      nc.sync.dma_start(g[:], pool_out[gw * 128:(gw + 1) * 128, :])
                      gm = sb.tile([128, 256], BF16, tag="p_gm")
                      nc.vector.tensor_scalar(gm[:], g[:], invg[:, gw:gw + 1], None, AOP.mult)
                      gT = sb.tile([128, 2 * 128], BF16, tag="p_gT")
                      for kb in range(2):
                          tp = ps.tile([128, 128], BF16, tag="p_tp", space="PSUM")
                          nc.tensor.transpose(tp[:], gm[:, kb * 128:(kb + 1) * 128], ident[:])
                          nc.vector.tensor_copy(gT[:, kb * 128:(kb + 1) * 128], tp[:])
                      hT = sb.tile([128, 2 * 128], BF16, tag="p_hT")
                      for mo in range(2):
                          hp = ps.tile([128, 128], F32, tag="p_hp", space="PSUM")
                          for ki in range(2):
                              nc.tensor.matmul(hp[:], lw1[ki * 2 + mo][:],
                                               gT[:, ki * 128:(ki + 1) * 128],
                                               start=(ki == 0), stop=(ki == 1))
                          nc.scalar.activation(hT[:, mo * 128:(mo + 1) * 128], hp[:],
                                               AFT.Relu, bias=lb1[mo])
                      op_ = ps.tile([2, 128], F32, tag="p_op", space="PSUM")
                      for ki in range(2):
                          nc.tensor.matmul(op_[:], lw2[ki][:],
                                           hT[:, ki * 128:(ki + 1) * 128],
                                           start=(ki == 0), stop=(ki == 1))
                      nc.vector.tensor_scalar(ofin[:, gw * 128:(gw + 1) * 128],
                                              op_[:], lb2[:], None, AOP.add)
                  nc.sync.dma_start(o_out[:], ofin[:, :N_GRAPHS])

    nc.compile()
    return nc


# ============================ entry point ============================


def kernel(**inputs):
    x = np.asarray(inputs["x"], dtype=np.float32)
    edge_index = np.asarray(inputs["edge_index"])
    batch = np.asarray(inputs["batch"])

    meta = _pack(edge_index, batch)
    Bg = meta["Bg"]

    import os as _os
    phases = int(_os.environ.get("KPHASES", "4"))
    key = ("mod", Bg, phases, _DEBUG[0])
    if key not in _cache:
        _cache[key] = _build(Bg, debug=bool(inputs.get("_debug", False)) or _DEBUG[0],
                             phases=phases)
    nc = _cache[key]

    # ---- per-core input arrays ----
    slot_of_node = meta["slot_of_node"]
    src = np.asarray(edge_index[0], dtype=np.int64)
    dst = np.asarray(edge_index[1], dtype=np.int64)

    # conv1 msgT: [core, 20, E_PAD//2] bf16; edge e<EHALF -> rows 0..9 col e,
    # e>=EHALF -> rows 10..19 col e-EHALF
    EHALF = E_PAD // 2
    xi_v = x[dst]
    xj_v = x[src]
    msg = np.concatenate([xi_v, xj_v - xi_v], axis=1)       # [E, 10]

    # exact conv1-L1 GraphNorm stats on host (tiny 10-dim Gram)
    msg64 = msg.astype(np.float64)
    W1 = np.asarray(inputs["c1_w1"], np.float64)            # [10, 128]
    b1 = np.asarray(inputs["c1_b1"], np.float64)            # [128]
    S = msg64.sum(0)
    G = msg64.T @ msg64
    SW = S @ W1
    qz = np.einsum('ij,ik,kj->j', W1, G, W1) + 2 * b1 * SW + N_EDGES * b1 * b1
    m1 = (SW + N_EDGES * b1) / N_EDGES
    e2 = qz / N_EDGES
    gn1 = np.asarray(inputs["c1_gn1"], np.float64)          # [3, 128]
    msm = gn1[2] * m1
    var1 = e2 - 2 * msm * m1 + msm * msm
    A1h = gn1[0] / np.sqrt(var1 + EPS)
    C1h = gn1[1] + A1h * (b1 - msm)
    c1a_in = np.stack([A1h, C1h]).astype(np.float32).reshape(2, 128, 1)

    msg_full = np.zeros((NC, E_PAD, 10), dtype=np.float32)
    ec, pos = meta["ec"], meta["pos"]
    msg_full[ec, pos] = msg[meta["eorder"]]
    msgT = np.zeros((NC, 48, EHALF), dtype=ml_dtypes.bfloat16)
    msgT[:, :10, :] = _bf(msg_full[:, :EHALF].transpose(0, 2, 1))
    msgT[:, 32:42, :] = _bf(msg_full[:, EHALF:].transpose(0, 2, 1))

    dstwin = meta["dstwin"]  # [NC, E_PAD]
    dwin_in = np.ascontiguousarray(
        dstwin.reshape(NC, E_PAD // 128, 128).transpose(0, 2, 1)).astype(np.float32)
    invcnt_in = np.ascontiguousarray(
        meta["inv_cnt"].reshape(NC, NWIN, 128).transpose(0, 2, 1)).astype(np.float32)
    padcnt_in = np.repeat(meta["pad_cnt"][:, None], 128, axis=1)[:, :, None].astype(np.float32)

    iota_in = np.broadcast_to(np.arange(128, dtype=np.float32)[None, :], (128, 128))
    iota_in = np.ascontiguousarray(iota_in)
    ident_in = np.eye(128, dtype=np.float32).astype(ml_dtypes.bfloat16)
    iotap_in = np.arange(128, dtype=np.float32).reshape(128, 1)
    dwinR_in = np.ascontiguousarray(np.broadcast_to(
        dstwin[:, None, :], (NC, 128, E_PAD))).astype(ml_dtypes.bfloat16)

    xj_row = meta["row_of_slot"][meta["xj_glob"]]  # [NC, E_PAD] chunk-major rows
    xj_in = np.ascontiguousarray(
        xj_row.reshape(NC, E_PAD // 128, 128).transpose(0, 2, 1)).astype(np.int32)

    # weights
    c1w = np.zeros((3, 128, 128), dtype=ml_dtypes.bfloat16)
    c1w[0, :10, :] = _bf(inputs["c1_w1"])
    c1w[0, 32:42, :] = _bf(inputs["c1_w1"])
    c1w[1] = _bf(inputs["c1_w2"])
    c1w[2] = _bf(inputs["c1_w3"])
    c1b = np.stack([np.asarray(inputs[f"c1_b{i}"], dtype=np.float32).reshape(128, 1)
                    for i in (1, 2, 3)])
    c1gn = np.stack([np.asarray(inputs[f"c1_gn{i}"], dtype=np.float32).reshape(3, 128, 1)
                     for i in (1, 2, 3)])

    w2a = np.asarray(inputs["c2_w1"], dtype=np.float32)   # [256, 256]
    WA2 = w2a[:128] - w2a[128:]
    WB2 = w2a[128:]
    c2wa = _tile_w(WA2)[0]                                # [2, 128, 128]
    c2wb = _tile_w(WB2)[0]
    c2w2 = _tile_w(np.asarray(inputs["c2_w2"], dtype=np.float32))  # [2,2,128,128]
    c2b = np.stack([np.asarray(inputs["c2_b1"], dtype=np.float32).reshape(2, 128, 1),
                    np.asarray(inputs["c2_b2"], dtype=np.float32).reshape(2, 128, 1)])
    c2gn = np.stack([np.asarray(inputs["c2_gn1"], dtype=np.float32).reshape(3, 2, 128, 1),
                     np.asarray(inputs["c2_gn2"], dtype=np.float32).reshape(3, 2, 128, 1)])

    w3a = np.asarray(inputs["c3_w1"], dtype=np.float32)   # [512, 256]
    WA3 = w3a[:256] - w3a[256:]
    WB3 = w3a[256:]
    c3wa = _tile_w(WA3)                                   # [2,2,128,128]
    c3wb = _tile_w(WB3)
    c3b = np.asarray(inputs["c3_b1"], dtype=np.float32).reshape(2, 128, 1)
    c3gn = np.asarray(inputs["c3_gn1"], dtype=np.float32).reshape(3, 2, 128, 1)

    lw1 = _tile_w(np.asarray(inputs["lin_w1"], dtype=np.float32))
    lb1 = np.asarray(inputs["lin_b1"], dtype=np.float32).reshape(2, 128, 1)
    lw2_f = np.asarray(inputs["lin_w2"], dtype=np.float32)  # [256, 2]
    lw2 = np.stack([_bf(lw2_f[:128]), _bf(lw2_f[128:])])    # [2, 128, 2]
    lb2 = np.asarray(inputs["lin_b2"], dtype=np.float32).reshape(2, 1)

    Bg0 = meta["Bg"]
    pidx16_in = _wrap_idx(meta["pool_idx"].reshape(NC, 8 * Bg0 * 128))
    pidx16_in = pidx16_in.reshape(NC, 128, -1)
    pgwl = meta["pool_gwl"]                # [NC, 8, NPG]
    Bg_ = meta["Bg"]
    pgwl_in = np.ascontiguousarray(
        pgwl.reshape(NC, 8, Bg_, 128).transpose(0, 3, 1, 2)).reshape(NC, 128, 8 * Bg_)
    invg_in = np.broadcast_to(
        meta["inv_g"].reshape(8, 128).T[None], (NC, 128, 8)).astype(np.float32)
    invg_in = np.ascontiguousarray(invg_in)

    in_maps = []
    for c in range(NC):
        im = {
            "msgT": msgT[c],
            "xj_idx": xj_in[c],
            "dstwin": dwin_in[c],
            "invcnt": invcnt_in[c],
            "padcnt": padcnt_in[c],
            "iota": iota_in,
            "ident": ident_in,
            "iotap": iotap_in,
            "dwinR": dwinR_in[c],
            "c1w": c1w, "c1a": c1a_in, "c1b": c1b, "c1gn": c1gn,
            "c2wa": c2wa, "c2wb": c2wb, "c2w2": c2w2, "c2b": c2b, "c2gn": c2gn,
            "c3wa": c3wa, "c3wb": c3wb, "c3b": c3b, "c3gn": c3gn,
            "lw1": lw1, "lb1": lb1, "lw2": lw2, "lb2": lb2,
            "pidx16": pidx16_in[c],
            "pool_gwl": pgwl_in[c].astype(np.float32),
            "invg": invg_in[c],
        }
        in_maps.append(im)

    res = run_bass_kernel_spmd(nc, in_maps, core_ids=list(range(NC)),
                               trace=_TRACE[0])
    kernel.last_result = res
    kernel.last_meta = meta
    out = res.results[0]["out"]            # [2, 1000]
    return np.ascontiguousarray(out.T).astype(np.float32)


_DEBUG = [False]
_TRACE = [False]



# revision 63
# speedup vs baseline: 1.3565x; 1.0033x over previous
"""LundNetTagger GNN on 8 Trainium2 NeuronCores (Bass/Tile).

Self-contained: kernel(**inputs) -> np.ndarray [1000, 2] float32.

Strategy: nodes are assigned to 100352 "slots" (8 cores x 98 windows x 128),
packed so each window receives <= 512 edges. Edges live on the core owning
their dst slot, in window-major order padded to 4x128-edge chunks per window.
Per-edge MLPs run in bf16 feature-major layout; EdgeConv cat[xi, xj-xi] is
folded into split weights WA = W[:C]-W[C:], WB = W[C:]. GraphNorm stats are
global AllReduces of per-core sums (conv1 layer-1 stats are computed exactly
on the host from the 10-dim message Gram; deeper layers use vector-engine
bn_stats on PSUM with a sentinel pad column for exact correction).
conv1 keeps h fully SBUF-resident (no z spills): layer 2 overwrites h in
place after its stats AllReduce, and layer 3 fuses into the scatter.
Mean-aggregation is a collision-free one-hot matmul scatter into PSUM per
window. Node tables are AllGathered in bf16 between convs in two chunk-major
halves (each half fires as soon as its windows are written, overlapping the
producing scatter); src-side gathers use per-chunk indirect DMA with
chunk-major global row indices.
"""
import numpy as np
import ml_dtypes

import concourse.bass as bass
import concourse.tile as tile
from concourse import bacc, mybir
from concourse.bass_utils import run_bass_kernel_spmd
from concourse import library_config

BF16 = mybir.dt.bfloat16
F32 = mybir.dt.float32
I16 = mybir.dt.int16
AOP = mybir.AluOpType
AFT = mybir.ActivationFunctionType
AX = mybir.AxisListType

N_NODES = 100000
N_EDGES = 400000
N_GRAPHS = 1000
NC = 8
WIN = 128
NWIN = 98
SPC = WIN * NWIN          # 12544
NSLOTS = SPC * NC         # 100352
QUAD = NSLOTS // 4        # 25088
B = 4                     # chunks per window
EPW = B * WIN             # 512
E_PAD = NWIN * EPW        # 50176
EPS = 1e-5

NW_BLK = 7
BLK = NW_BLK * EPW        # 3584
NBLK = NWIN // NW_BLK     # 14
NCHUNK = BLK // 128       # 28
NSEG = BLK // 512         # 7

# window-aligned AllGather chunk boundaries (local rows) and the scatter
# block index after which each chunk's windows are complete
AG_CHB = [0, 25 * WIN, 50 * WIN, 74 * WIN, SPC]   # 3200/3200/3072/3072 rows
AG_FIRE = [4, 8, 11]     # fire chunk k at top of block AG_FIRE[k]; last at end


_cache = {}


# ============================ host-side packing ============================

def _pack(edge_index, batch):
    src = np.asarray(edge_index[0], dtype=np.int64)
    dst = np.asarray(edge_index[1], dtype=np.int64)
    batch = np.asarray(batch, dtype=np.int64)
    cnt = np.bincount(dst, minlength=N_NODES)

    nvirt = NSLOTS - N_NODES
    cnt_all = np.concatenate([cnt, np.zeros(nvirt, dtype=cnt.dtype)])
    order = np.argsort(-cnt_all, kind="stable")
    GW = NWIN * NC
    rounds = NSLOTS // GW
    win_of_rank = np.empty(NSLOTS, dtype=np.int64)
    for r in range(rounds):
        seg = np.arange(GW) if r % 2 == 0 else np.arange(GW - 1, -1, -1)
        win_of_rank[r * GW:(r + 1) * GW] = seg
    win_of_node = np.empty(NSLOTS, dtype=np.int64)
    win_of_node[order] = win_of_rank
    wsum = np.bincount(win_of_node, weights=cnt_all.astype(np.float64),
                       minlength=GW).astype(np.int64)

    cap = EPW
    members_of = [list(np.where(win_of_node == w)[0]) for w in range(GW)]
    for _ in range(2000):
        over = np.where(wsum > cap)[0]
        if len(over) == 0:
            break
        w = int(over[0])
        # smallest-count >0 node in w
        mem = members_of[w]
        cs = [(int(cnt_all[n]), n) for n in mem if cnt_all[n] > 0]
        cs.sort()
        moved = False
        for c1, n in cs:
            # find target window with a smaller-count node to swap
            worder2 = np.argsort(wsum)
            for tw in worder2[:64]:
                tw = int(tw)
                if tw == w:
                    continue
                tmem = members_of[tw]
                best = None
                for m in tmem:
                    c2 = int(cnt_all[m])
                    if c2 < c1 and wsum[tw] + c1 - c2 <= cap:
                        if best is None or c2 < best[0]:
                            best = (c2, m)
                        if c2 == 0:
                            break
                if best is not None:
                    c2, m = best
                    members_of[tw].remove(m)
                    members_of[tw].append(n)
                    members_of[w].remove(n)
                    members_of[w].append(m)
                    win_of_node[n] = tw
                    win_of_node[m] = w
                    wsum[tw] += c1 - c2
                    wsum[w] -= c1 - c2
                    moved = True
                    break
            if moved:
                break
        if not moved:
            raise RuntimeError("packing fixup stuck")
    assert wsum.max() <= cap, f"window packing failed: max={wsum.max()}"

    worder = np.argsort(-wsum, kind="stable")
    core_load = np.zeros(NC, dtype=np.int64)
    core_nwin = np.zeros(NC, dtype=np.int64)
    core_of_win = np.empty(GW, dtype=np.int64)
    for w in worder:
        cands = np.where(core_nwin < NWIN)[0]
        c = cands[np.argmin(core_load[cands])]
        core_of_win[w] = c
        core_load[c] += wsum[w]
        core_nwin[c] += 1

    win_lists = [[] for _ in range(NC)]
    for w in range(GW):
        win_lists[core_of_win[w]].append(w)
    for c in range(NC):
        wl = win_lists[c]
        j = int(np.argmin(wsum[wl]))
        assert wsum[wl[j]] < cap, "no sentinel room"
        wl[j], wl[-1] = wl[-1], wl[j]

    slot_of_node = np.empty(NSLOTS, dtype=np.int64)
    for c in range(NC):
        for wi, w in enumerate(win_lists[c]):
            mem = np.sort(np.array(members_of[w], dtype=np.int64))
            assert len(mem) == WIN
            slot_of_node[mem] = c * SPC + wi * WIN + np.arange(WIN)
    node_of_slot = np.empty(NSLOTS, dtype=np.int64)
    node_of_slot[slot_of_node] = np.arange(NSLOTS)
    cnt_of_slot = cnt_all[node_of_slot]

    qzero = []
    for q in range(4):
        z = np.where(cnt_of_slot[q * QUAD:(q + 1) * QUAD] == 0)[0]
        assert len(z) > 0
        assert z[0] < 32768
        qzero.append(int(z[0]))  # local to quadrant
    czero = []
    for c in range(NC):
        z = np.where(cnt_of_slot[c * SPC:(c + 1) * SPC] == 0)[0]
        assert len(z) > 0
        czero.append(int(z[0]))  # local to core

    dslot = slot_of_node[dst]
    sslot = slot_of_node[src]
    ecore = dslot // SPC
    ewin = (dslot % SPC) // WIN
    key = ecore * (NWIN * WIN) + ewin * WIN + (dslot % WIN)
    eorder = np.argsort(key, kind="stable")
    dsl, ssl = dslot[eorder], sslot[eorder]
    ec, ew = ecore[eorder], ewin[eorder]

    cw = ec * NWIN + ew
    cw_cnt = np.bincount(cw, minlength=NC * NWIN)
    assert cw_cnt.max() <= EPW

    xi_idx = np.zeros((NC, E_PAD), dtype=np.int64)
    xj_idx = np.zeros((NC, E_PAD), dtype=np.int64)
    dstwin = np.full((NC, E_PAD), -1.0, dtype=np.float32)
    valid = np.zeros((NC, E_PAD), dtype=bool)

    ofs = (np.arange(NC * NWIN) % NWIN) * EPW
    start = np.concatenate([[0], np.cumsum(cw_cnt)[:-1]])
    within = np.arange(N_EDGES) - start[cw]
    pos = ofs[cw] + within
    xi_idx[ec, pos] = dsl % SPC
    xj_idx[ec, pos] = ssl
    dstwin[ec, pos] = (dsl % WIN).astype(np.float32)
    valid[ec, pos] = True
    for c in range(NC):
        xi_idx[c, ~valid[c]] = czero[c]
    pad_cnt = (~valid).sum(axis=1).astype(np.float32)
    assert np.all(~valid[:, -1]), "sentinel column must be padding"

    gzero = qzero[0]  # global slot with zero row
    xj_glob = np.where(valid, xj_idx, gzero).astype(np.int32)

    # Chunk-major AllGather table layout: local rows split into 4
    # window-aligned chunks; the full table stores [chunk][core][rows] so
    # each AG chunk output is a contiguous row block.
    sl_ = np.arange(NSLOTS)
    n_, s_ = sl_ // SPC, sl_ % SPC
    c_ = np.searchsorted(np.array(AG_CHB), s_, side="right") - 1
    sizes = np.diff(np.array(AG_CHB))
    base_full = np.concatenate([[0], np.cumsum(sizes * NC)[:-1]])
    row_of_slot = (base_full[c_] + n_ * sizes[c_]
                   + (s_ - np.array(AG_CHB)[c_]))

    inv_cnt = (1.0 / np.maximum(cnt_of_slot.reshape(NC, SPC), 1.0)).astype(np.float32)

    g_of_slot = np.full(NSLOTS, -1, dtype=np.int64)
    real = node_of_slot < N_NODES
    g_of_slot[real] = batch[node_of_slot[real]]
    NGW = 8
    Bg = 0
    pools = [[None] * NGW for _ in range(NC)]
    for c in range(NC):
        gl = g_of_slot[c * SPC:(c + 1) * SPC]
        for gw in range(NGW):
            m = np.where((gl >= gw * 128) & (gl < (gw + 1) * 128))[0]
            pools[c][gw] = m
            Bg = max(Bg, (len(m) + 127) // 128)
    NPG = Bg * 128
    pool_idx = np.zeros((NC, NGW, NPG), dtype=np.int16)
    pool_gwl = np.full((NC, NGW, NPG), -1.0, dtype=np.float32)
    for c in range(NC):
        for gw in range(NGW):
            m = pools[c][gw]
            pool_idx[c, gw, :len(m)] = m.astype(np.int16)
            pool_idx[c, gw, len(m):] = czero[c]
            pool_gwl[c, gw, :len(m)] = (g_of_slot[c * SPC + m] - gw * 128).astype(np.float32)

    gcnt = np.bincount(batch, minlength=N_GRAPHS).astype(np.float32)
    inv_g = np.zeros(1024, dtype=np.float32)
    inv_g[:N_GRAPHS] = 1.0 / np.maximum(gcnt, 1.0)

    return dict(slot_of_node=slot_of_node, node_of_slot=node_of_slot,
                row_of_slot=row_of_slot,
                xj_glob=xj_glob, dstwin=dstwin, pad_cnt=pad_cnt,
                inv_cnt=inv_cnt, valid=valid, eorder=eorder, ec=ec, pos=pos,
                pool_idx=pool_idx, pool_gwl=pool_gwl, inv_g=inv_g, Bg=Bg)


def _wrap_idx(a):
    """[.., n] int -> [.., 128, n//16]: element i -> partition i%16 col i//16,
    replicated to 8 groups of 16 partitions."""
    n = a.shape[-1]
    assert n % 16 == 0
    w = a.reshape(*a.shape[:-1], n // 16, 16)
    w = np.swapaxes(w, -1, -2)
    w = np.broadcast_to(w[..., None, :, :], (*a.shape[:-1], 8, 16, n // 16))
    return np.ascontiguousarray(w).reshape(*a.shape[:-1], 128, n // 16).astype(np.int16)


def _bf(x):
    return np.ascontiguousarray(np.asarray(x, dtype=np.float32)).astype(ml_dtypes.bfloat16)


def _tile_w(w):
    K, M = w.shape
    nk, nm = (K + 127) // 128, (M + 127) // 128
    out = np.zeros((nk, nm, 128, 128), dtype=ml_dtypes.bfloat16)
    for i in range(nk):
        for j in range(nm):
            blk = np.asarray(w, dtype=np.float32)[i * 128:(i + 1) * 128, j * 128:(j + 1) * 128]
            out[i, j, :blk.shape[0], :blk.shape[1]] = _bf(blk)
    return out


# ============================ device kernel ============================

EHALF = E_PAD // 2        # 25088
NSEG_H = EHALF // 512     # 49


def _build(Bg, debug=False, phases=4):
    nc = bacc.Bacc("TRN2", target_bir_lowering=False, debug=False, num_devices=NC)

    def din(name, shape, dt):
        return nc.dram_tensor(name, shape, dt, kind="ExternalInput").ap()

    NIDX = E_PAD // 16
    t_msgT = din("msgT", [48, EHALF], BF16)
    t_xj = din("xj_idx", [128, E_PAD // 128], mybir.dt.int32)
    t_dstwin = din("dstwin", [128, E_PAD // 128], F32)
    t_dwinR = din("dwinR", [128, E_PAD], BF16)
    t_iotap = din("iotap", [128, 1], F32)
    t_invcnt = din("invcnt", [128, NWIN], F32)
    t_padcnt = din("padcnt", [128, 1], F32)
    t_iota = din("iota", [128, 128], F32)
    t_ident = din("ident", [128, 128], BF16)
    t_c1w = din("c1w", [3, 128, 128], BF16)
    t_c1a = din("c1a", [2, 128, 1], F32)
    t_c1b = din("c1b", [3, 128, 1], F32)
    t_c1gn = din("c1gn", [3, 3, 128, 1], F32)
    t_c2wa = din("c2wa", [2, 128, 128], BF16)
    t_c2wb = din("c2wb", [2, 128, 128], BF16)
    t_c2w2 = din("c2w2", [2, 2, 128, 128], BF16)
    t_c2b = din("c2b", [2, 2, 128, 1], F32)
    t_c2gn = din("c2gn", [2, 3, 2, 128, 1], F32)
    t_c3wa = din("c3wa", [2, 2, 128, 128], BF16)
    t_c3wb = din("c3wb", [2, 2, 128, 128], BF16)
    t_c3b = din("c3b", [2, 128, 1], F32)
    t_c3gn = din("c3gn", [3, 2, 128, 1], F32)
    t_lw1 = din("lw1", [2, 2, 128, 128], BF16)
    t_lb1 = din("lb1", [2, 128, 1], F32)
    t_lw2 = din("lw2", [2, 128, 2], BF16)
    t_lb2 = din("lb2", [2, 1], F32)
    t_pidx16 = din("pidx16", [128, 8 * Bg * 128 // 16], I16)
    t_pgwl = din("pool_gwl", [128, 8 * Bg], F32)
    t_invg = din("invg", [128, 8], F32)

    o_out = nc.dram_tensor("out", [2, N_GRAPHS], F32, kind="ExternalOutput").ap()
    dbg = {}
    if debug:
        dbg["x1"] = nc.dram_tensor("dbg_x1", [NSLOTS, 128], BF16, kind="ExternalOutput").ap()
        dbg["x2"] = nc.dram_tensor("dbg_x2", [NSLOTS, 256], BF16, kind="ExternalOutput").ap()
        dbg["x3"] = nc.dram_tensor("dbg_x3", [SPC, 256], BF16, kind="ExternalOutput").ap()
        dbg["pool"] = nc.dram_tensor("dbg_pool", [1024, 256], F32, kind="ExternalOutput").ap()

    with tile.TileContext(nc) as tc:
        with tc.tile_pool(name="dram", bufs=1, space="DRAM") as dram, \
             tc.tile_pool(name="cp", bufs=1) as cp:
            z_scr = [dram.tile([2, 128, E_PAD], BF16, tag=f"zscr{i}", name=f"zscr{i}") for i in range(2)]
            tab1_loc = dram.tile([SPC, 128], BF16)
            tab1 = dram.tile([NSLOTS, 128], BF16)
            tab2_loc = dram.tile([SPC, 256], BF16)
            tab2 = dram.tile([NSLOTS, 256], BF16)
            tab3_loc = dram.tile([SPC, 256], BF16)
            st_in = dram.tile([128, 8], F32)
            st_out = dram.tile([128, 8], F32)
            pool_in = dram.tile([1024, 256], F32)
            pool_out = dram.tile([1024, 256], F32)

            ident = cp.tile([128, 128], BF16)
            nc.sync.dma_start(ident[:], t_ident[:])
            iota = cp.tile([128, 128], F32)
            nc.sync.dma_start(iota[:], t_iota[:])
            invcnt = cp.tile([128, NWIN], F32)
            nc.sync.dma_start(invcnt[:], t_invcnt[:])
            dwin = cp.tile([128, E_PAD // 128], F32)
            nc.sync.dma_start(dwin[:], t_dstwin[:])
            padcnt = cp.tile([128, 1], F32)
            nc.sync.dma_start(padcnt[:], t_padcnt[:])
            iotap = cp.tile([128, 1], F32)
            nc.sync.dma_start(iotap[:], t_iotap[:])

            # ---------- helpers ----------
            def allreduce_stats(s_acc, q_acc, n_mb, sb):
                st = sb.tile([128, 8], F32, tag="st_")
                nc.vector.memset(st[:], 0.0)
                nc.vector.tensor_copy(st[:, 0:n_mb], s_acc[:])
                nc.vector.tensor_copy(st[:, 4:4 + n_mb], q_acc[:])
                nc.sync.dma_start(st_in[:], st[:])
                nc.gpsimd.collective_compute(
                    "AllReduce", AOP.add, replica_groups=[list(range(NC))],
                    ins=[st_in.opt()], outs=[st_out.opt()])
                stg = sb.tile([128, 8], F32, tag="stg_")
                nc.sync.dma_start(stg[:], st_out[:])
                return stg

            def affine_from_stats(stg, n_mb, b_lin, gn, sb):
                A, Cc = [], []
                for mb in range(n_mb):
                    s = stg[:, mb:mb + 1]
                    q = stg[:, 4 + mb:5 + mb]
                    g, bgn, ms = gn[0][mb], gn[1][mb], gn[2][mb]
                    bl = b_lin[mb]
                    m = sb.tile([128, 1], F32, tag="af_m")
                    nc.vector.tensor_scalar(m[:], s, 1.0 / N_EDGES, None, AOP.mult)
                    nc.vector.tensor_tensor(m[:], m[:], bl, op=AOP.add)
                    e2 = sb.tile([128, 1], F32, tag="af_e2")
                    nc.vector.tensor_scalar(e2[:], q, 1.0 / N_EDGES, None, AOP.mult)
                    tmp = sb.tile([128, 1], F32, tag="af_t")
                    nc.vector.tensor_tensor(tmp[:], m[:], bl, op=AOP.mult)
                    nc.vector.tensor_scalar(tmp[:], tmp[:], 2.0, None, AOP.mult)
                    nc.vector.tensor_tensor(e2[:], e2[:], tmp[:], op=AOP.add)
                    nc.vector.tensor_tensor(tmp[:], bl, bl, op=AOP.mult)
                    nc.vector.tensor_tensor(e2[:], e2[:], tmp[:], op=AOP.subtract)
                    msm = sb.tile([128, 1], F32, tag="af_msm")
                    nc.vector.tensor_tensor(msm[:], ms, m[:], op=AOP.mult)
                    var = sb.tile([128, 1], F32, tag="af_v")
                    nc.vector.tensor_tensor(var[:], msm[:], msm[:], op=AOP.mult)
                    nc.vector.tensor_tensor(tmp[:], msm[:], m[:], op=AOP.mult)
                    nc.vector.tensor_scalar(tmp[:], tmp[:], 2.0, None, AOP.mult)
                    nc.vector.tensor_tensor(var[:], var[:], tmp[:], op=AOP.subtract)
                    nc.vector.tensor_tensor(var[:], var[:], e2[:], op=AOP.add)
                    a = sb.tile([128, 1], F32, tag="af_a")
                    nc.vector.tensor_scalar(var[:], var[:], EPS, None, AOP.add)
                    nc.scalar.activation(a[:], var[:], AFT.Sqrt)
                    nc.vector.reciprocal(a[:], a[:])
                    nc.vector.tensor_tensor(a[:], a[:], g, op=AOP.mult)
                    cc = sb.tile([128, 1], F32, tag="af_c")
                    nc.vector.tensor_tensor(cc[:], bl, msm[:], op=AOP.subtract)
                    nc.vector.tensor_tensor(cc[:], cc[:], a[:], op=AOP.mult)
                    nc.vector.tensor_tensor(cc[:], cc[:], bgn, op=AOP.add)
                    A.append(a)
                    Cc.append(cc)
                return A, Cc

            def acc_stats(ps_ap, s_col, q_col, sb):
                t1 = sb.tile([128, 1], F32, tag="rs_t1")
                nc.vector.reduce_sum(out=t1[:], in_=ps_ap, axis=AX.X)
                nc.vector.tensor_tensor(s_col, s_col, t1[:], op=AOP.add)
                n = ps_ap.shape[-1]
                sq = sb.tile([128, 512], BF16, tag="rs_sq")
                qa = sb.tile([128, 1], F32, tag="rs_qa")
                nc.scalar.activation(sq[:, :n], ps_ap, AFT.Square, accum_out=qa[:])
                nc.vector.tensor_tensor(q_col, q_col, qa[:], op=AOP.add)

            def bn_finish(st, s_col, q_col, sb, tag):
                # bn_stats 6-tuples (equal 512-col groups) -> sum / sq-sum
                agg = sb.tile([128, 2], F32, tag=tag + "g")
                nc.vector.bn_aggr(agg[:], st[:])
                nc.vector.tensor_scalar(s_col, agg[:, 0:1], float(E_PAD),
                                        None, AOP.mult)
                t = sb.tile([128, 1], F32, tag=tag + "t")
                nc.vector.tensor_tensor(t[:], agg[:, 0:1], agg[:, 0:1], op=AOP.mult)
                nc.vector.tensor_tensor(t[:], t[:], agg[:, 1:2], op=AOP.add)
                nc.vector.tensor_scalar(q_col, t[:], float(E_PAD), None, AOP.mult)

            def sentinel_correct(s_acc, q_acc, zsent_cols, n_mb, sb):
                for mb in range(n_mb):
                    zs = zsent_cols[mb]
                    t1 = sb.tile([128, 1], F32, tag="sc_t1")
                    nc.vector.tensor_tensor(t1[:], zs, padcnt[:], op=AOP.mult)
                    nc.vector.tensor_tensor(s_acc[:, mb:mb + 1], s_acc[:, mb:mb + 1],
                                            t1[:], op=AOP.subtract)
                    nc.vector.tensor_tensor(t1[:], zs, zs, op=AOP.mult)
                    nc.vector.tensor_tensor(t1[:], t1[:], padcnt[:], op=AOP.mult)
                    nc.vector.tensor_tensor(q_acc[:, mb:mb + 1], q_acc[:, mb:mb + 1],
                                            t1[:], op=AOP.subtract)

            def load_vec(t_ap, sb, tag):
                v = sb.tile([128, 1], F32, tag=tag)
                nc.sync.dma_start(v[:], t_ap)
                return v[:]

            AG_BASE = [0]
            for _c in range(3):
                AG_BASE.append(AG_BASE[-1] + NC * (AG_CHB[_c + 1] - AG_CHB[_c]))

            def fire_ag(tab_loc, tab_full, c):
                # chunk-major table: AG chunk c is a contiguous row block
                lo, hi = AG_CHB[c], AG_CHB[c + 1]
                nc.gpsimd.collective_compute(
                    "AllGather", AOP.bypass, replica_groups=[list(range(NC))],
                    ins=[tab_loc[lo:hi, :].opt()],
                    outs=[tab_full[AG_BASE[c]:AG_BASE[c] + NC * (hi - lo),
                                   :].opt()])

            def scatter_pass(zsrc, n_mb, A, Cc, tab_loc, Cout, ag=None):
                with tc.tile_pool(name="sc_sb", bufs=2) as sb, \
                     tc.tile_pool(name="sc_tp", bufs=2, space="PSUM") as ps_tp, \
                     tc.tile_pool(name="sc_sc", bufs=2, space="PSUM") as ps_sc:
                    for b in range(NBLK):
                        if ag is not None and b in AG_FIRE:
                            fire_ag(tab_loc, ag, AG_FIRE.index(b))
                        hs = []
                        for mb in range(n_mb):
                            z = sb.tile([128, BLK], BF16, tag=f"sp_z{mb}")
                            nc.sync.dma_start(z[:], zsrc[mb, :, b * BLK:(b + 1) * BLK])
                            h = sb.tile([128, BLK], BF16, tag=f"sp_h{mb}")
                            nc.scalar.activation(h[:], z[:], AFT.Relu,
                                                 bias=Cc[mb], scale=A[mb])
                            hs.append(h)
                        hE = sb.tile([128, NCHUNK * Cout], BF16, tag="sp_hE")
                        for ch in range(NCHUNK):
                            for mb in range(n_mb):
                                tp = ps_tp.tile([128, 128], BF16, tag="sp_tp", space="PSUM")
                                nc.tensor.transpose(tp[:], hs[mb][:, ch * 128:(ch + 1) * 128],
                                                    ident[:])
                                nc.vector.tensor_copy(
                                    hE[:, ch * Cout + mb * 128:ch * Cout + (mb + 1) * 128],
                                    tp[:])
                        for w in range(NW_BLK):
                            gw = b * NW_BLK + w
                            sc = ps_sc.tile([128, Cout], F32, tag="sp_sc", space="PSUM")
                            for cb in range(B):
                                ch = w * B + cb
                                col = b * NCHUNK + ch
                                oh = sb.tile([128, 128], BF16, tag="sp_oh")
                                nc.vector.tensor_tensor(
                                    out=oh[:],
                                    in0=dwin[:, col:col + 1].to_broadcast([128, 128]),
                                    in1=iota[:], op=AOP.is_equal)
                                nc.tensor.matmul(sc[:], oh[:],
                                                 hE[:, ch * Cout:(ch + 1) * Cout],
                                                 start=(cb == 0), stop=(cb == B - 1))
                            nt = sb.tile([128, Cout], BF16, tag="sp_nt")
                            nc.vector.tensor_scalar(nt[:], sc[:], invcnt[:, gw:gw + 1],
                                                    None, AOP.mult)
                            nc.sync.dma_start(tab_loc[gw * WIN:(gw + 1) * WIN, :], nt[:])
                    if ag is not None:
                        fire_ag(tab_loc, ag, 3)

            # ======================= CONV 1 =======================
            # SBUF-resident: L1 stats precomputed on host; h kept on-chip,
            # L2 overwrites it in place; L3 fused with the scatter.
            NSEG_T = E_PAD // 512  # 98
            with tc.tile_pool(name="c1sb", bufs=2) as sb:
                c1b = [[load_vec(t_c1b[i], sb, f"c1b{i}")] for i in range(3)]
                c1gn = [[[load_vec(t_c1gn[i, j], sb, f"c1gn{i}{j}")] for j in range(3)]
                        for i in range(3)]
                A1h = load_vec(t_c1a[0], sb, "c1a0")
                C1h = load_vec(t_c1a[1], sb, "c1a1")
                with tc.tile_pool(name="c1h", bufs=1) as hp, \
                     tc.tile_pool(name="c1ps", bufs=2, space="PSUM") as ps, \
                     tc.tile_pool(name="c1p2", bufs=2, space="PSUM") as ps2:
                    c1w = []
                    for i in range(3):
                        w = sb.tile([128, 128], BF16, tag=f"c1w{i}")
                        nc.sync.dma_start(w[:], t_c1w[i])
                        c1w.append(w)
                    msgT = hp.tile([48, EHALF], BF16, tag="msgT")
                    nc.sync.dma_start(msgT[:], t_msgT[:])
                    h_full = hp.tile([128, E_PAD], BF16)

                    def bn_to_sq(st, tag):
                        agg = sb.tile([128, 2], F32, tag=tag + "agg")
                        nc.vector.bn_aggr(agg[:], st[:])
                        s_acc = sb.tile([128, 1], F32, tag=tag + "s")
                        q_acc = sb.tile([128, 1], F32, tag=tag + "q")
                        nc.vector.tensor_scalar(s_acc[:], agg[:, 0:1],
                                                float(E_PAD), None, AOP.mult)
                        nc.vector.tensor_tensor(q_acc[:], agg[:, 0:1], agg[:, 0:1],
                                                op=AOP.mult)
                        nc.vector.tensor_tensor(q_acc[:], q_acc[:], agg[:, 1:2],
                                                op=AOP.add)
                        nc.vector.tensor_scalar(q_acc[:], q_acc[:],
                                                float(E_PAD), None, AOP.mult)
                        return s_acc, q_acc

                    # pass 1: L1 -> h_full; L2 stats
                    st2 = hp.tile([128, NSEG_T * 6], F32, tag="st2")
                    zs2 = sb.tile([128, 1], F32, tag="zs2")
                    for g in range(NSEG_T):
                        hh, shalf = g // NSEG_H, g % NSEG_H
                        zp = ps.tile([128, 512], F32, tag="zp")
                        nc.tensor.matmul(zp[:], c1w[0][32 * hh:32 * hh + 10, :],
                                         msgT[32 * hh:32 * hh + 10,
                                              shalf * 512:(shalf + 1) * 512],
                                         start=True, stop=True)
                        nc.scalar.activation(h_full[:, g * 512:(g + 1) * 512], zp[:],
                                             AFT.Relu, bias=C1h, scale=A1h)
                        zp2 = ps2.tile([128, 512], F32, tag="zp2")
                        nc.tensor.matmul(zp2[:], c1w[1][:],
                                         h_full[:, g * 512:(g + 1) * 512],
                                         start=True, stop=True)
                        nc.vector.bn_stats(st2[:, g * 6:(g + 1) * 6], zp2[:])
                        if g == NSEG_T - 1:
                            nc.vector.tensor_copy(zs2[:], zp2[:, 511:512])
                    s2, q2 = bn_to_sq(st2, "b2")
                    sentinel_correct(s2, q2, [zs2[:]], 1, sb)
                    stg2 = allreduce_stats(s2, q2, 1, sb)
                    A2, C2 = affine_from_stats(stg2, 1, c1b[1], c1gn[1], sb)

                    # pass 2: L2 -> h_full (in place); L3 stats
                    st3 = hp.tile([128, NSEG_T * 6], F32, tag="st3")
                    zs3 = sb.tile([128, 1], F32, tag="zs3")
                    for g in range(NSEG_T):
                        zp = ps.tile([128, 512], F32, tag="zp")
                        nc.tensor.matmul(zp[:], c1w[1][:],
                                         h_full[:, g * 512:(g + 1) * 512],
                                         start=True, stop=True)
                        nc.scalar.activation(h_full[:, g * 512:(g + 1) * 512], zp[:],
                                             AFT.Relu, bias=C2[0], scale=A2[0])
                        zp3 = ps2.tile([128, 512], F32, tag="zp2")
                        nc.tensor.matmul(zp3[:], c1w[2][:],
                                         h_full[:, g * 512:(g + 1) * 512],
                                         start=True, stop=True)
                        nc.vector.bn_stats(st3[:, g * 6:(g + 1) * 6], zp3[:])
                        if g == NSEG_T - 1:
                            nc.vector.tensor_copy(zs3[:], zp3[:, 511:512])
                    s3, q3 = bn_to_sq(st3, "b3")
                    sentinel_correct(s3, q3, [zs3[:]], 1, sb)
                    stg3 = allreduce_stats(s3, q3, 1, sb)
                    A3, C3 = affine_from_stats(stg3, 1, c1b[2], c1gn[2], sb)

                    # pass 3: L3 + fused scatter
                    with tc.tile_pool(name="c1sc", bufs=2) as scb, \
                         tc.tile_pool(name="c1tp", bufs=2, space="PSUM") as ps_tp, \
                         tc.tile_pool(name="c1s2", bufs=2, space="PSUM") as ps_sc:
                        for b in range(NBLK):
                            if b in AG_FIRE:
                                fire_ag(tab1_loc, tab1, AG_FIRE.index(b))
                            h3 = scb.tile([128, BLK], BF16, tag="c1h3")
                            for s in range(NSEG):
                                g = b * NSEG + s
                                zp = ps.tile([128, 512], F32, tag="zp")
                                nc.tensor.matmul(zp[:], c1w[2][:],
                                                 h_full[:, g * 512:(g + 1) * 512],
                                                 start=True, stop=True)
                                nc.scalar.activation(h3[:, s * 512:(s + 1) * 512],
                                                     zp[:], AFT.Relu,
                                                     bias=C3[0], scale=A3[0])
                            hE = scb.tile([128, NCHUNK * 128], BF16, tag="c1hE")
                            for ch in range(NCHUNK):
                                tp = ps_tp.tile([128, 128], BF16, tag="c1tp",
                                                space="PSUM")
                                nc.tensor.transpose(tp[:], h3[:, ch * 128:(ch + 1) * 128],
                                                    ident[:])
                                nc.vector.tensor_copy(hE[:, ch * 128:(ch + 1) * 128],
                                                      tp[:])
                            for w in range(NW_BLK):
                                gw = b * NW_BLK + w
                                sc = ps_sc.tile([128, 128], F32, tag="c1sc",
                                                space="PSUM")
                                for cb in range(B):
                                    ch = w * B + cb
                                    col = b * NCHUNK + ch
                                    oh = scb.tile([128, 128], BF16, tag="c1oh")
                                    nc.vector.tensor_tensor(
                                        out=oh[:],
                                        in0=dwin[:, col:col + 1].to_broadcast([128, 128]),
                                        in1=iota[:], op=AOP.is_equal)
                                    nc.tensor.matmul(sc[:], oh[:],
                                                     hE[:, ch * 128:(ch + 1) * 128],
                                                     start=(cb == 0), stop=(cb == B - 1))
                                nt = scb.tile([128, 128], BF16, tag="c1nt")
                                nc.vector.tensor_scalar(nt[:], sc[:],
                                                        invcnt[:, gw:gw + 1],
                                                        None, AOP.mult)
                                nc.sync.dma_start(tab1_loc[gw * WIN:(gw + 1) * WIN, :],
                                                  nt[:])
                        fire_ag(tab1_loc, tab1, 3)

            if debug:
                nc.sync.dma_start(dbg["x1"][:], tab1[:])

            # ============== gather-based first layer (conv2/conv3) ==============
            def gather_layer(tab_full, tab_loc, Cin, wa_t, wb_t, n_kb, zdst, sb):
                mb_in = Cin // 128
                s_acc = sb.tile([128, 2], F32, tag="gl_s")
                q_acc = sb.tile([128, 2], F32, tag="gl_q")
                sts = [sb.tile([128, (E_PAD // 512) * 6], F32, tag=f"gl_st{mo}",
                               name=f"gl_st{mo}")
                       for mo in range(2)]
                with tc.tile_pool(name="gl_g2", bufs=3) as g2, \
                     tc.tile_pool(name="gl_g1", bufs=2) as g1, \
                     tc.tile_pool(name="gl_zw", bufs=2) as zwp, \
                     tc.tile_pool(name="gl_ps", bufs=2, space="PSUM") as ps, \
                     tc.tile_pool(name="gl_tp", bufs=2, space="PSUM") as ps_tp, \
                     tc.tile_pool(name="gl_xp", bufs=2, space="PSUM") as ps_xp:
                    was, wbs = [], []
                    for ki in range(n_kb):
                        for mo in range(2):
                            wta = sb.tile([128, 128], BF16, tag=f"gl_wa{ki}{mo}")
                            nc.sync.dma_start(wta[:], wa_t[ki, mo] if n_kb > 1 else wa_t[mo])
                            was.append(wta)
                            wtb = sb.tile([128, 128], BF16, tag=f"gl_wb{ki}{mo}")
                            nc.sync.dma_start(wtb[:], wb_t[ki, mo] if n_kb > 1 else wb_t[mo])
                            wbs.append(wtb)
                    ixj = sb.tile([128, NBLK * NCHUNK], mybir.dt.int32,
                                  tag="gl_ixj")
                    nc.sync.dma_start(ixj[:], t_xj[:])
                    for b in range(NBLK):
                        gxj = g2.tile([128, NCHUNK * Cin], BF16, tag="gl_gxj")
                        for ch in range(NCHUNK):
                            gch = b * NCHUNK + ch
                            nc.gpsimd.indirect_dma_start(
                                out=gxj[:, ch * Cin:(ch + 1) * Cin],
                                out_offset=None,
                                in_=tab_full[:],
                                in_offset=bass.IndirectOffsetOnAxis(
                                    ap=ixj[:, gch:gch + 1], axis=0))
                        xjT = g1.tile([128, mb_in * BLK], BF16, tag="gl_xjT")
                        for ch in range(NCHUNK):
                            for kb in range(mb_in):
                                tp2 = ps_tp.tile([128, 128], BF16, tag="gl_ohp",
                                                 space="PSUM")
                                nc.tensor.transpose(
                                    tp2[:],
                                    gxj[:, ch * Cin + kb * 128:ch * Cin + (kb + 1) * 128],
                                    ident[:])
                                dst = xjT[:, kb * BLK + ch * 128:
                                          kb * BLK + (ch + 1) * 128]
                                if kb == 0 and (mb_in == 2 or ch % 2 == 0):
                                    nc.scalar.copy(dst, tp2[:])
                                else:
                                    nc.vector.tensor_copy(dst, tp2[:])
                        # xi via window expansion (transposed one-hot built
                        # directly from the replicated dstwin row)
                        dwb = g1.tile([128, BLK], BF16, tag="gl_dwb")
                        nc.sync.dma_start(dwb[:], t_dwinR[:, b * BLK:(b + 1) * BLK])
                        xiT = g1.tile([128, mb_in * BLK], BF16, tag="gl_xiT")
                        for w in range(NW_BLK):
                            gw = b * NW_BLK + w
                            twin = g2.tile([128, Cin], BF16, tag="gl_twin")
                            nc.sync.dma_start(twin[:], tab_loc[gw * WIN:(gw + 1) * WIN, :])
                            for cb in range(B):
                                ch = w * B + cb
                                oh2 = g2.tile([128, 128], BF16, tag="gl_oh2")
                                nc.vector.tensor_scalar(
                                    oh2[:], dwb[:, ch * 128:(ch + 1) * 128],
                                    iotap[:, 0:1], None, AOP.is_equal)
                                for kb in range(mb_in):
                                    xp = ps_xp.tile([128, 128], F32, tag="gl_xp", space="PSUM")
                                    nc.tensor.matmul(xp[:], twin[:, kb * 128:(kb + 1) * 128],
                                                     oh2[:], start=True, stop=True)
                                    nc.vector.tensor_copy(
                                        xiT[:, kb * BLK + ch * 128:kb * BLK + (ch + 1) * 128],
                                        xp[:])
                        for mo in range(2):
                            zw = zwp.tile([128, BLK], BF16, tag=f"gl_z{mo}")
                            for sg in range(NSEG):
                                g6 = (b * NSEG + sg) * 6
                                zp = ps.tile([128, 512], F32, tag="gl_zp")
                                for ki in range(mb_in):
                                    nc.tensor.matmul(
                                        zp[:], was[ki * 2 + mo][:],
                                        xiT[:, ki * BLK + sg * 512:ki * BLK + (sg + 1) * 512],
                                        start=(ki == 0), stop=False)
                                for ki in range(mb_in):
                                    nc.tensor.matmul(
                                        zp[:], wbs[ki * 2 + mo][:],
                                        xjT[:, ki * BLK + sg * 512:ki * BLK + (sg + 1) * 512],
                                        start=False, stop=(ki == mb_in - 1))
                                nc.vector.bn_stats(sts[mo][:, g6:g6 + 6], zp[:])
                                nc.scalar.copy(zw[:, sg * 512:(sg + 1) * 512], zp[:])
                            nc.sync.dma_start(zdst[mo, :, b * BLK:(b + 1) * BLK], zw[:])
                for mo in range(2):
                    bn_finish(sts[mo], s_acc[:, mo:mo + 1], q_acc[:, mo:mo + 1],
                              sb, f"glf{mo}")
                return s_acc, q_acc

            # ======================= CONV 2 =======================
            if phases >= 2:
              with tc.tile_pool(name="c2sb", bufs=2) as sb:
                  c2b = [[load_vec(t_c2b[i, mb], sb, f"c2b{i}{mb}") for mb in range(2)]
                         for i in range(2)]
                  c2gn = [[[load_vec(t_c2gn[i, j, mb], sb, f"c2gn{i}{j}{mb}")
                            for mb in range(2)] for j in range(3)] for i in range(2)]
                  sA, qA = gather_layer(tab1, tab1_loc, 128, t_c2wa, t_c2wb, 1,
                                        z_scr[0], sb)
                  stg = allreduce_stats(sA, qA, 2, sb)
                  A1, C1 = affine_from_stats(stg, 2, c2b[0], c2gn[0], sb)

                  s2 = sb.tile([128, 2], F32, tag="c2s2")
                  q2 = sb.tile([128, 2], F32, tag="c2q2")
                  st2s = [sb.tile([128, (E_PAD // 512) * 6], F32, tag=f"c2st{mo}",
                                  name=f"c2st{mo}")
                          for mo in range(2)]
                  zlast = [None, None]
                  with tc.tile_pool(name="c2mid", bufs=2) as mp, \
                       tc.tile_pool(name="c2ps", bufs=2, space="PSUM") as ps:
                      w2s = []
                      for ki in range(2):
                          for mo in range(2):
                              w = sb.tile([128, 128], BF16, tag=f"c2w2{ki}{mo}")
                              nc.sync.dma_start(w[:], t_c2w2[ki, mo])
                              w2s.append(w)
                      for b in range(NBLK):
                          h1 = []
                          for mb in range(2):
                              z = mp.tile([128, BLK], BF16, tag=f"c2z1r{mb}")
                              nc.sync.dma_start(z[:], z_scr[0][mb, :, b * BLK:(b + 1) * BLK])
                              hh = mp.tile([128, BLK], BF16, tag=f"c2h1{mb}")
                              nc.scalar.activation(hh[:], z[:], AFT.Relu,
                                                   bias=C1[mb], scale=A1[mb])
                              h1.append(hh)
                          for mo in range(2):
                              zw = mp.tile([128, BLK], BF16, tag=f"c2z2w{mo}")
                              for s in range(NSEG):
                                  g6 = (b * NSEG + s) * 6
                                  zp = ps.tile([128, 512], F32, tag="c2zp")
                                  for ki in range(2):
                                      nc.tensor.matmul(zp[:], w2s[ki * 2 + mo][:],
                                                       h1[ki][:, s * 512:(s + 1) * 512],
                                                       start=(ki == 0), stop=(ki == 1))
                                  nc.vector.bn_stats(st2s[mo][:, g6:g6 + 6], zp[:])
                                  if s % 2 == 0:
                                      nc.scalar.copy(zw[:, s * 512:(s + 1) * 512],
                                                     zp[:])
                                  else:
                                      nc.vector.tensor_copy(
                                          zw[:, s * 512:(s + 1) * 512], zp[:])
                              nc.sync.dma_start(z_scr[1][mo, :, b * BLK:(b + 1) * BLK], zw[:])
                              zlast[mo] = zw
                      zsent = []
                      for mo in range(2):
                          zc = sb.tile([128, 1], F32, tag=f"c2zs{mo}")
                          nc.vector.tensor_copy(zc[:], zlast[mo][:, BLK - 1:BLK])
                          zsent.append(zc[:])
                  for mo in range(2):
                      bn_finish(st2s[mo], s2[:, mo:mo + 1], q2[:, mo:mo + 1],
                                sb, f"c2f{mo}")
                  sentinel_correct(s2, q2, zsent, 2, sb)
                  stg2 = allreduce_stats(s2, q2, 2, sb)
                  A2, C2 = affine_from_stats(stg2, 2, c2b[1], c2gn[1], sb)
                  scatter_pass(z_scr[1], 2, A2, C2, tab2_loc, 256, ag=tab2)

            if debug:
                nc.sync.dma_start(dbg["x2"][:], tab2[:])

            # ======================= CONV 3 =======================
            if phases >= 3:
              with tc.tile_pool(name="c3sb", bufs=2) as sb:
                  c3b = [load_vec(t_c3b[mb], sb, f"c3b{mb}") for mb in range(2)]
                  c3gn = [[load_vec(t_c3gn[j, mb], sb, f"c3gn{j}{mb}") for mb in range(2)]
                          for j in range(3)]
                  sA, qA = gather_layer(tab2, tab2_loc, 256, t_c3wa, t_c3wb, 2,
                                        z_scr[0], sb)
                  stg = allreduce_stats(sA, qA, 2, sb)
                  A1, C1 = affine_from_stats(stg, 2, c3b, c3gn, sb)
                  scatter_pass(z_scr[0], 2, A1, C1, tab3_loc, 256)

            if debug:
                nc.sync.dma_start(dbg["x3"][:], tab3_loc[:])

            # ======================= POOL + HEAD =======================
            if phases >= 4:
              with tc.tile_pool(name="p_sb", bufs=2) as sb, \
                 tc.tile_pool(name="p_ps", bufs=2, space="PSUM") as ps:
                  pgwl = sb.tile([128, 8 * Bg], F32, tag="p_pgwl")
                  nc.sync.dma_start(pgwl[:], t_pgwl[:])
                  NPG = Bg * 128
                  pidxw = sb.tile([128, 8 * NPG // 16], I16, tag="p_idx16")
                  nc.sync.dma_start(pidxw[:], t_pidx16[:])
                  for gw in range(8):
                      gp = sb.tile([128, Bg, 256], BF16, tag="p_gp")
                      nc.gpsimd.dma_gather(
                          out_ap=gp[:], in_ap=tab3_loc[:],
                          idxs_ap=pidxw[:, gw * (NPG // 16):(gw + 1) * (NPG // 16)],
                          num_idxs=NPG, num_idxs_reg=NPG, elem_size=256,
                          transpose=False, single_packet=(NPG <= 896))
                      pp = ps.tile([128, 256], F32, tag="p_pp", space="PSUM")
                      for c in range(Bg):
                          oh = sb.tile([128, 128], BF16, tag="p_oh")
                          nc.vector.tensor_tensor(
                              out=oh[:],
                              in0=pgwl[:, gw * Bg + c:gw * Bg + c + 1].to_broadcast([128, 128]),
                              in1=iota[:], op=AOP.is_equal)
                          nc.tensor.matmul(pp[:], oh[:], gp[:, c, :],
                                           start=(c == 0), stop=(c == Bg - 1))
                      pf = sb.tile([128, 256], F32, tag="p_pf")
                      nc.vector.tensor_copy(pf[:], pp[:])
                      nc.sync.dma_start(pool_in[gw * 128:(gw + 1) * 128, :], pf[:])
                  nc.gpsimd.collective_compute(
                      "AllReduce", AOP.add, replica_groups=[list(range(NC))],
                      ins=[pool_in.opt()], outs=[pool_out.opt()])
                  if debug:
                      nc.sync.dma_start(dbg["pool"][:], pool_out[:])

                  invg = sb.tile([128, 8], F32, tag="p_invg")
                  nc.sync.dma_start(invg[:], t_invg[:])
                  lw1 = []
                  for ki in range(2):
                      for mo in range(2):
                          w = sb.tile([128, 128], BF16, tag=f"p_lw1{ki}{mo}")
                          nc.sync.dma_start(w[:], t_lw1[ki, mo])
                          lw1.append(w)
                  lw2 = []
                  for ki in range(2):
                      w = sb.tile([128, 2], BF16, tag=f"p_lw2{ki}")
                      nc.sync.dma_start(w[:], t_lw2[ki])
                      lw2.append(w)
                  lb1 = [load_vec(t_lb1[mb], sb, f"p_lb1{mb}") for mb in range(2)]
                  lb2 = sb.tile([2, 1], F32, tag="p_lb2")
                  nc.sync.dma_start(lb2[:], t_lb2[:])
                  ofin = sb.tile([2, 1024], F32, tag="p_out")
                  for gw in range(8):
                      g = sb.tile([128, 256], F32, tag="p_g")
                      nc.sync.dma_start(g[:], pool_out[gw * 128:(gw + 1) * 128, :])
                      gm = sb.tile([128, 256], BF16, tag="p_gm")
                      nc.vector.tensor_scalar(gm[:], g[:], invg[:, gw:gw + 1], None, AOP.mult)
                      gT = sb.tile([128, 2 * 128], BF16, tag="p_gT")
                      for kb in range(2):
                          tp = ps.tile([128, 128], BF16, tag="p_tp", space="PSUM")
                          nc.tensor.transpose(tp[:], gm[:, kb * 128:(kb + 1) * 128], ident[:])
                          nc.vector.tensor_copy(gT[:, kb * 128:(kb + 1) * 128], tp[:])
                      hT = sb.tile([128, 2 * 128], BF16, tag="p_hT")
                      for mo in range(2):
                          hp = ps.tile([128, 128], F32, tag="p_hp", space="PSUM")
                          for ki in range(2):
                              nc.tensor.matmul(hp[:], lw1[ki * 2 + mo][:],
                                               gT[:, ki * 128:(ki + 1) * 128],
                                               start=(ki == 0), stop=(ki == 1))
                          nc.scalar.activation(hT[:, mo * 128:(mo + 1) * 128], hp[:],
                                               AFT.Relu, bias=lb1[mo])
                      op_ = ps.tile([2, 128], F32, tag="p_op", space="PSUM")
                      for ki in range(2):
                          nc.tensor.matmul(op_[:], lw2[ki][:],
                                           hT[:, ki * 128:(ki + 1) * 128],
                                           start=(ki == 0), stop=(ki == 1))
                      nc.vector.tensor_scalar(ofin[:, gw * 128:(gw + 1) * 128],
                                              op_[:], lb2[:], None, AOP.add)
                  nc.sync.dma_start(o_out[:], ofin[:, :N_GRAPHS])

    nc.compile()
    return nc


# ============================ entry point ============================


def kernel(**inputs):
    x = np.asarray(inputs["x"], dtype=np.float32)
    edge_index = np.asarray(inputs["edge_index"])
    batch = np.asarray(inputs["batch"])

    meta = _pack(edge_index, batch)
    Bg = meta["Bg"]

    import os as _os
    phases = int(_os.environ.get("KPHASES", "4"))
    key = ("mod", Bg, phases, _DEBUG[0])
    if key not in _cache:
        _cache[key] = _build(Bg, debug=bool(inputs.get("_debug", False)) or _DEBUG[0],
                             phases=phases)
    nc = _cache[key]

    # ---- per-core input arrays ----
    slot_of_node = meta["slot_of_node"]
    src = np.asarray(edge_index[0], dtype=np.int64)
    dst = np.asarray(edge_index[1], dtype=np.int64)

    # conv1 msgT: [core, 20, E_PAD//2] bf16; edge e<EHALF -> rows 0..9 col e,
    # e>=EHALF -> rows 10..19 col e-EHALF
    EHALF = E_PAD // 2
    xi_v = x[dst]
    xj_v = x[src]
    msg = np.concatenate([xi_v, xj_v - xi_v], axis=1)       # [E, 10]

    # exact conv1-L1 GraphNorm stats on host (tiny 10-dim Gram)
    msg64 = msg.astype(np.float64)
    W1 = np.asarray(inputs["c1_w1"], np.float64)            # [10, 128]
    b1 = np.asarray(inputs["c1_b1"], np.float64)            # [128]
    S = msg64.sum(0)
    G = msg64.T @ msg64
    SW = S @ W1
    qz = np.einsum('ij,ik,kj->j', W1, G, W1) + 2 * b1 * SW + N_EDGES * b1 * b1
    m1 = (SW + N_EDGES * b1) / N_EDGES
    e2 = qz / N_EDGES
    gn1 = np.asarray(inputs["c1_gn1"], np.float64)          # [3, 128]
    msm = gn1[2] * m1
    var1 = e2 - 2 * msm * m1 + msm * msm
    A1h = gn1[0] / np.sqrt(var1 + EPS)
    C1h = gn1[1] + A1h * (b1 - msm)
    c1a_in = np.stack([A1h, C1h]).astype(np.float32).reshape(2, 128, 1)

    msg_full = np.zeros((NC, E_PAD, 10), dtype=np.float32)
    ec, pos = meta["ec"], meta["pos"]
    msg_full[ec, pos] = msg[meta["eorder"]]
    msgT = np.zeros((NC, 48, EHALF), dtype=ml_dtypes.bfloat16)
    msgT[:, :10, :] = _bf(msg_full[:, :EHALF].transpose(0, 2, 1))
    msgT[:, 32:42, :] = _bf(msg_full[:, EHALF:].transpose(0, 2, 1))

    dstwin = meta["dstwin"]  # [NC, E_PAD]
    dwin_in = np.ascontiguousarray(
        dstwin.reshape(NC, E_PAD // 128, 128).transpose(0, 2, 1)).astype(np.float32)
    invcnt_in = np.ascontiguousarray(
        meta["inv_cnt"].reshape(NC, NWIN, 128).transpose(0, 2, 1)).astype(np.float32)
    padcnt_in = np.repeat(meta["pad_cnt"][:, None], 128, axis=1)[:, :, None].astype(np.float32)

    iota_in = np.broadcast_to(np.arange(128, dtype=np.float32)[None, :], (128, 128))
    iota_in = np.ascontiguousarray(iota_in)
    ident_in = np.eye(128, dtype=np.float32).astype(ml_dtypes.bfloat16)
    iotap_in = np.arange(128, dtype=np.float32).reshape(128, 1)
    dwinR_in = np.ascontiguousarray(np.broadcast_to(
        dstwin[:, None, :], (NC, 128, E_PAD))).astype(ml_dtypes.bfloat16)

    xj_row = meta["row_of_slot"][meta["xj_glob"]]  # [NC, E_PAD] chunk-major rows
    xj_in = np.ascontiguousarray(
        xj_row.reshape(NC, E_PAD // 128, 128).transpose(0, 2, 1)).astype(np.int32)

    # weights
    c1w = np.zeros((3, 128, 128), dtype=ml_dtypes.bfloat16)
    c1w[0, :10, :] = _bf(inputs["c1_w1"])
    c1w[0, 32:42, :] = _bf(inputs["c1_w1"])
    c1w[1] = _bf(inputs["c1_w2"])
    c1w[2] = _bf(inputs["c1_w3"])
    c1b = np.stack([np.asarray(inputs[f"c1_b{i}"], dtype=np.float32).reshape(128, 1)
                    for i in (1, 2, 3)])
    c1gn = np.stack([np.asarray(inputs[f"c1_gn{i}"], dtype=np.float32).reshape(3, 128, 1)
                     for i in (1, 2, 3)])

    w2a = np.asarray(inputs["c2_w1"], dtype=np.float32)   # [256, 256]
    WA2 = w2a[:128] - w2a[128:]
    WB2 = w2a[128:]
    c2wa = _tile_w(WA2)[0]                                # [2, 128, 128]
    c2wb = _tile_w(WB2)[0]
    c2w2 = _tile_w(np.asarray(inputs["c2_w2"], dtype=np.float32))  # [2,2,128,128]
    c2b = np.stack([np.asarray(inputs["c2_b1"], dtype=np.float32).reshape(2, 128, 1),
                    np.asarray(inputs["c2_b2"], dtype=np.float32).reshape(2, 128, 1)])
    c2gn = np.stack([np.asarray(inputs["c2_gn1"], dtype=np.float32).reshape(3, 2, 128, 1),
                     np.asarray(inputs["c2_gn2"], dtype=np.float32).reshape(3, 2, 128, 1)])

    w3a = np.asarray(inputs["c3_w1"], dtype=np.float32)   # [512, 256]
    WA3 = w3a[:256] - w3a[256:]
    WB3 = w3a[256:]
    c3wa = _tile_w(WA3)                                   # [2,2,128,128]
    c3wb = _tile_w(WB3)
    c3b = np.asarray(inputs["c3_b1"], dtype=np.float32).reshape(2, 128, 1)
    c3gn = np.asarray(inputs["c3_gn1"], dtype=np.float32).reshape(3, 2, 128, 1)

    lw1 = _tile_w(np.asarray(inputs["lin_w1"], dtype=np.float32))
    lb1 = np.asarray(inputs["lin_b1"], dtype=np.float32).reshape(2, 128, 1)
    lw2_f = np.asarray(inputs["lin_w2"], dtype=np.float32)  # [256, 2]
    lw2 = np.stack([_bf(lw2_f[:128]), _bf(lw2_f[128:])])    # [2, 128, 2]
    lb2 = np.asarray(inputs["lin_b2"], dtype=np.float32).reshape(2, 1)

    Bg0 = meta["Bg"]
    pidx16_in = _wrap_idx(meta["pool_idx"].reshape(NC, 8 * Bg0 * 128))
    pidx16_in = pidx16_in.reshape(NC, 128, -1)
    pgwl = meta["pool_gwl"]                # [NC, 8, NPG]
    Bg_ = meta["Bg"]
    pgwl_in = np.ascontiguousarray(
        pgwl.reshape(NC, 8, Bg_, 128).transpose(0, 3, 1, 2)).reshape(NC, 128, 8 * Bg_)
    invg_in = np.broadcast_to(
        meta["inv_g"].reshape(8, 128).T[None], (NC, 128, 8)).astype(np.float32)
    invg_in = np.ascontiguousarray(invg_in)

    in_maps = []
    for c in range(NC):
        im = {
            "msgT": msgT[c],
            "xj_idx": xj_in[c],
            "dstwin": dwin_in[c],
            "invcnt": invcnt_in[c],
            "padcnt": padcnt_in[c],
            "iota": iota_in,
            "ident": ident_in,
            "iotap": iotap_in,
            "dwinR": dwinR_in[c],
            "c1w": c1w, "c1a": c1a_in, "c1b": c1b, "c1gn": c1gn,
            "c2wa": c2wa, "c2wb": c2wb, "c2w2": c2w2, "c2b": c2b, "c2gn": c2gn,
            "c3wa": c3wa, "c3wb": c3wb, "c3b": c3b, "c3gn": c3gn,
            "lw1": lw1, "lb1": lb1, "lw2": lw2, "lb2": lb2,
            "pidx16": pidx16_in[c],
            "pool_gwl": pgwl_in[c].astype(np.float32),
            "invg": invg_in[c],
        }
        in_maps.append(im)

    res = run_bass_kernel_spmd(nc, in_maps, core_ids=list(range(NC)),
                               trace=_TRACE[0])
    kernel.last_result = res
    kernel.last_meta = meta
    out = res.results[0]["out"]            # [2, 1000]
    return np.ascontiguousarray(out.T).astype(np.float32)


_DEBUG = [False]
_TRACE = [False]



# revision 64
# speedup vs baseline: 1.3607x; 1.0032x over previous
"""LundNetTagger GNN on 8 Trainium2 NeuronCores (Bass/Tile).

Self-contained: kernel(**inputs) -> np.ndarray [1000, 2] float32.

Strategy: nodes are assigned to 100352 "slots" (8 cores x 98 windows x 128),
packed so each window receives <= 512 edges. Edges live on the core owning
their dst slot, in window-major order padded to 4x128-edge chunks per window.
Per-edge MLPs run in bf16 feature-major layout; EdgeConv cat[xi, xj-xi] is
folded into split weights WA = W[:C]-W[C:], WB = W[C:]. GraphNorm stats are
global AllReduces of per-core sums (conv1 layer-1 stats are computed exactly
on the host from the 10-dim message Gram; deeper layers use vector-engine
bn_stats on PSUM with a sentinel pad column for exact correction).
conv1 keeps h fully SBUF-resident (no z spills): layer 2 overwrites h in
place after its stats AllReduce, and layer 3 fuses into the scatter.
Mean-aggregation is a collision-free one-hot matmul scatter into PSUM per
window. Node tables are AllGathered in bf16 between convs in two chunk-major
halves (each half fires as soon as its windows are written, overlapping the
producing scatter); src-side gathers use per-chunk indirect DMA with
chunk-major global row indices.
"""
import numpy as np
import ml_dtypes

import concourse.bass as bass
import concourse.tile as tile
from concourse import bacc, mybir
from concourse.bass_utils import run_bass_kernel_spmd
from concourse import library_config

BF16 = mybir.dt.bfloat16
F32 = mybir.dt.float32
I16 = mybir.dt.int16
AOP = mybir.AluOpType
AFT = mybir.ActivationFunctionType
AX = mybir.AxisListType

N_NODES = 100000
N_EDGES = 400000
N_GRAPHS = 1000
NC = 8
WIN = 128
NWIN = 98
SPC = WIN * NWIN          # 12544
NSLOTS = SPC * NC         # 100352
QUAD = NSLOTS // 4        # 25088
B = 4                     # chunks per window
EPW = B * WIN             # 512
E_PAD = NWIN * EPW        # 50176
EPS = 1e-5

NW_BLK = 7
BLK = NW_BLK * EPW        # 3584
NBLK = NWIN // NW_BLK     # 14
NCHUNK = BLK // 128       # 28
NSEG = BLK // 512         # 7

# window-aligned AllGather chunk boundaries (local rows) and the scatter
# block index after which each chunk's windows are complete
AG_CHB = [0, 25 * WIN, 50 * WIN, 74 * WIN, SPC]   # 3200/3200/3072/3072 rows
AG_FIRE = [4, 8, 11]     # fire chunk k at top of block AG_FIRE[k]; last at end


_cache = {}


# ============================ host-side packing ============================

def _pack(edge_index, batch):
    src = np.asarray(edge_index[0], dtype=np.int64)
    dst = np.asarray(edge_index[1], dtype=np.int64)
    batch = np.asarray(batch, dtype=np.int64)
    cnt = np.bincount(dst, minlength=N_NODES)

    nvirt = NSLOTS - N_NODES
    cnt_all = np.concatenate([cnt, np.zeros(nvirt, dtype=cnt.dtype)])
    order = np.argsort(-cnt_all, kind="stable")
    GW = NWIN * NC
    rounds = NSLOTS // GW
    win_of_rank = np.empty(NSLOTS, dtype=np.int64)
    for r in range(rounds):
        seg = np.arange(GW) if r % 2 == 0 else np.arange(GW - 1, -1, -1)
        win_of_rank[r * GW:(r + 1) * GW] = seg
    win_of_node = np.empty(NSLOTS, dtype=np.int64)
    win_of_node[order] = win_of_rank
    wsum = np.bincount(win_of_node, weights=cnt_all.astype(np.float64),
                       minlength=GW).astype(np.int64)

    cap = EPW
    members_of = [list(np.where(win_of_node == w)[0]) for w in range(GW)]
    for _ in range(2000):
        over = np.where(wsum > cap)[0]
        if len(over) == 0:
            break
        w = int(over[0])
        # smallest-count >0 node in w
        mem = members_of[w]
        cs = [(int(cnt_all[n]), n) for n in mem if cnt_all[n] > 0]
        cs.sort()
        moved = False
        for c1, n in cs:
            # find target window with a smaller-count node to swap
            worder2 = np.argsort(wsum)
            for tw in worder2[:64]:
                tw = int(tw)
                if tw == w:
                    continue
                tmem = members_of[tw]
                best = None
                for m in tmem:
                    c2 = int(cnt_all[m])
                    if c2 < c1 and wsum[tw] + c1 - c2 <= cap:
                        if best is None or c2 < best[0]:
                            best = (c2, m)
                        if c2 == 0:
                            break
                if best is not None:
                    c2, m = best
                    members_of[tw].remove(m)
                    members_of[tw].append(n)
                    members_of[w].remove(n)
                    members_of[w].append(m)
                    win_of_node[n] = tw
                    win_of_node[m] = w
                    wsum[tw] += c1 - c2
                    wsum[w] -= c1 - c2
                    moved = True
                    break
            if moved:
                break
        if not moved:
            raise RuntimeError("packing fixup stuck")
    assert wsum.max() <= cap, f"window packing failed: max={wsum.max()}"

    worder = np.argsort(-wsum, kind="stable")
    core_load = np.zeros(NC, dtype=np.int64)
    core_nwin = np.zeros(NC, dtype=np.int64)
    core_of_win = np.empty(GW, dtype=np.int64)
    for w in worder:
        cands = np.where(core_nwin < NWIN)[0]
        c = cands[np.argmin(core_load[cands])]
        core_of_win[w] = c
        core_load[c] += wsum[w]
        core_nwin[c] += 1

    win_lists = [[] for _ in range(NC)]
    for w in range(GW):
        win_lists[core_of_win[w]].append(w)
    for c in range(NC):
        wl = win_lists[c]
        j = int(np.argmin(wsum[wl]))
        assert wsum[wl[j]] < cap, "no sentinel room"
        wl[j], wl[-1] = wl[-1], wl[j]

    slot_of_node = np.empty(NSLOTS, dtype=np.int64)
    for c in range(NC):
        for wi, w in enumerate(win_lists[c]):
            mem = np.sort(np.array(members_of[w], dtype=np.int64))
            assert len(mem) == WIN
            slot_of_node[mem] = c * SPC + wi * WIN + np.arange(WIN)
    node_of_slot = np.empty(NSLOTS, dtype=np.int64)
    node_of_slot[slot_of_node] = np.arange(NSLOTS)
    cnt_of_slot = cnt_all[node_of_slot]

    qzero = []
    for q in range(4):
        z = np.where(cnt_of_slot[q * QUAD:(q + 1) * QUAD] == 0)[0]
        assert len(z) > 0
        assert z[0] < 32768
        qzero.append(int(z[0]))  # local to quadrant
    czero = []
    for c in range(NC):
        z = np.where(cnt_of_slot[c * SPC:(c + 1) * SPC] == 0)[0]
        assert len(z) > 0
        czero.append(int(z[0]))  # local to core

    dslot = slot_of_node[dst]
    sslot = slot_of_node[src]
    ecore = dslot // SPC
    ewin = (dslot % SPC) // WIN
    key = ecore * (NWIN * WIN) + ewin * WIN + (dslot % WIN)
    eorder = np.argsort(key, kind="stable")
    dsl, ssl = dslot[eorder], sslot[eorder]
    ec, ew = ecore[eorder], ewin[eorder]

    cw = ec * NWIN + ew
    cw_cnt = np.bincount(cw, minlength=NC * NWIN)
    assert cw_cnt.max() <= EPW

    xi_idx = np.zeros((NC, E_PAD), dtype=np.int64)
    xj_idx = np.zeros((NC, E_PAD), dtype=np.int64)
    dstwin = np.full((NC, E_PAD), -1.0, dtype=np.float32)
    valid = np.zeros((NC, E_PAD), dtype=bool)

    ofs = (np.arange(NC * NWIN) % NWIN) * EPW
    start = np.concatenate([[0], np.cumsum(cw_cnt)[:-1]])
    within = np.arange(N_EDGES) - start[cw]
    pos = ofs[cw] + within
    xi_idx[ec, pos] = dsl % SPC
    xj_idx[ec, pos] = ssl
    dstwin[ec, pos] = (dsl % WIN).astype(np.float32)
    valid[ec, pos] = True
    for c in range(NC):
        xi_idx[c, ~valid[c]] = czero[c]
    pad_cnt = (~valid).sum(axis=1).astype(np.float32)
    assert np.all(~valid[:, -1]), "sentinel column must be padding"

    gzero = qzero[0]  # global slot with zero row
    xj_glob = np.where(valid, xj_idx, gzero).astype(np.int32)

    # Chunk-major AllGather table layout: local rows split into 4
    # window-aligned chunks; the full table stores [chunk][core][rows] so
    # each AG chunk output is a contiguous row block.
    sl_ = np.arange(NSLOTS)
    n_, s_ = sl_ // SPC, sl_ % SPC
    c_ = np.searchsorted(np.array(AG_CHB), s_, side="right") - 1
    sizes = np.diff(np.array(AG_CHB))
    base_full = np.concatenate([[0], np.cumsum(sizes * NC)[:-1]])
    row_of_slot = (base_full[c_] + n_ * sizes[c_]
                   + (s_ - np.array(AG_CHB)[c_]))

    inv_cnt = (1.0 / np.maximum(cnt_of_slot.reshape(NC, SPC), 1.0)).astype(np.float32)

    g_of_slot = np.full(NSLOTS, -1, dtype=np.int64)
    real = node_of_slot < N_NODES
    g_of_slot[real] = batch[node_of_slot[real]]
    NGW = 8
    Bg = 0
    pools = [[None] * NGW for _ in range(NC)]
    for c in range(NC):
        gl = g_of_slot[c * SPC:(c + 1) * SPC]
        for gw in range(NGW):
            m = np.where((gl >= gw * 128) & (gl < (gw + 1) * 128))[0]
            pools[c][gw] = m
            Bg = max(Bg, (len(m) + 127) // 128)
    NPG = Bg * 128
    pool_idx = np.zeros((NC, NGW, NPG), dtype=np.int16)
    pool_gwl = np.full((NC, NGW, NPG), -1.0, dtype=np.float32)
    for c in range(NC):
        for gw in range(NGW):
            m = pools[c][gw]
            pool_idx[c, gw, :len(m)] = m.astype(np.int16)
            pool_idx[c, gw, len(m):] = czero[c]
            pool_gwl[c, gw, :len(m)] = (g_of_slot[c * SPC + m] - gw * 128).astype(np.float32)

    gcnt = np.bincount(batch, minlength=N_GRAPHS).astype(np.float32)
    inv_g = np.zeros(1024, dtype=np.float32)
    inv_g[:N_GRAPHS] = 1.0 / np.maximum(gcnt, 1.0)

    return dict(slot_of_node=slot_of_node, node_of_slot=node_of_slot,
                row_of_slot=row_of_slot,
                xj_glob=xj_glob, dstwin=dstwin, pad_cnt=pad_cnt,
                inv_cnt=inv_cnt, valid=valid, eorder=eorder, ec=ec, pos=pos,
                pool_idx=pool_idx, pool_gwl=pool_gwl, inv_g=inv_g, Bg=Bg)


def _wrap_idx(a):
    """[.., n] int -> [.., 128, n//16]: element i -> partition i%16 col i//16,
    replicated to 8 groups of 16 partitions."""
    n = a.shape[-1]
    assert n % 16 == 0
    w = a.reshape(*a.shape[:-1], n // 16, 16)
    w = np.swapaxes(w, -1, -2)
    w = np.broadcast_to(w[..., None, :, :], (*a.shape[:-1], 8, 16, n // 16))
    return np.ascontiguousarray(w).reshape(*a.shape[:-1], 128, n // 16).astype(np.int16)


def _bf(x):
    return np.ascontiguousarray(np.asarray(x, dtype=np.float32)).astype(ml_dtypes.bfloat16)


def _tile_w(w):
    K, M = w.shape
    nk, nm = (K + 127) // 128, (M + 127) // 128
    out = np.zeros((nk, nm, 128, 128), dtype=ml_dtypes.bfloat16)
    for i in range(nk):
        for j in range(nm):
            blk = np.asarray(w, dtype=np.float32)[i * 128:(i + 1) * 128, j * 128:(j + 1) * 128]
            out[i, j, :blk.shape[0], :blk.shape[1]] = _bf(blk)
    return out


# ============================ device kernel ============================

EHALF = E_PAD // 2        # 25088
NSEG_H = EHALF // 512     # 49


def _build(Bg, debug=False, phases=4):
    nc = bacc.Bacc("TRN2", target_bir_lowering=False, debug=False, num_devices=NC)

    def din(name, shape, dt):
        return nc.dram_tensor(name, shape, dt, kind="ExternalInput").ap()

    NIDX = E_PAD // 16
    t_msgT = din("msgT", [48, EHALF], BF16)
    t_xj = din("xj_idx", [128, E_PAD // 128], mybir.dt.int32)
    t_dstwin = din("dstwin", [128, E_PAD // 128], F32)
    t_dwinR = din("dwinR", [128, E_PAD], BF16)
    t_iotap = din("iotap", [128, 1], F32)
    t_invcnt = din("invcnt", [128, NWIN], F32)
    t_padcnt = din("padcnt", [128, 1], F32)
    t_iota = din("iota", [128, 128], F32)
    t_ident = din("ident", [128, 128], BF16)
    t_c1w = din("c1w", [3, 128, 128], BF16)
    t_c1a = din("c1a", [2, 128, 1], F32)
    t_c1b = din("c1b", [3, 128, 1], F32)
    t_c1gn = din("c1gn", [3, 3, 128, 1], F32)
    t_c2wa = din("c2wa", [2, 128, 128], BF16)
    t_c2wb = din("c2wb", [2, 128, 128], BF16)
    t_c2w2 = din("c2w2", [2, 2, 128, 128], BF16)
    t_c2b = din("c2b", [2, 2, 128, 1], F32)
    t_c2gn = din("c2gn", [2, 3, 2, 128, 1], F32)
    t_c3wa = din("c3wa", [2, 2, 128, 128], BF16)
    t_c3wb = din("c3wb", [2, 2, 128, 128], BF16)
    t_c3b = din("c3b", [2, 128, 1], F32)
    t_c3gn = din("c3gn", [3, 2, 128, 1], F32)
    t_lw1 = din("lw1", [2, 2, 128, 128], BF16)
    t_lb1 = din("lb1", [2, 128, 1], F32)
    t_lw2 = din("lw2", [2, 128, 2], BF16)
    t_lb2 = din("lb2", [2, 1], F32)
    t_pidx16 = din("pidx16", [128, 8 * Bg * 128 // 16], I16)
    t_pgwl = din("pool_gwl", [128, 8 * Bg], F32)
    t_invg = din("invg", [128, 8], F32)

    o_out = nc.dram_tensor("out", [2, N_GRAPHS], F32, kind="ExternalOutput").ap()
    dbg = {}
    if debug:
        dbg["x1"] = nc.dram_tensor("dbg_x1", [NSLOTS, 128], BF16, kind="ExternalOutput").ap()
        dbg["x2"] = nc.dram_tensor("dbg_x2", [NSLOTS, 256], BF16, kind="ExternalOutput").ap()
        dbg["x3"] = nc.dram_tensor("dbg_x3", [SPC, 256], BF16, kind="ExternalOutput").ap()
        dbg["pool"] = nc.dram_tensor("dbg_pool", [1024, 256], F32, kind="ExternalOutput").ap()

    with tile.TileContext(nc) as tc:
        with tc.tile_pool(name="dram", bufs=1, space="DRAM") as dram, \
             tc.tile_pool(name="cp", bufs=1) as cp:
            z_scr = [dram.tile([2, 128, E_PAD], BF16, tag=f"zscr{i}", name=f"zscr{i}") for i in range(2)]
            tab1_loc = dram.tile([SPC, 128], BF16)
            tab1 = dram.tile([NSLOTS, 128], BF16)
            tab2_loc = dram.tile([SPC, 256], BF16)
            tab2 = dram.tile([NSLOTS, 256], BF16)
            tab3_loc = dram.tile([SPC, 256], BF16)
            st_in = dram.tile([128, 8], F32)
            st_out = dram.tile([128, 8], F32)
            pool_in = dram.tile([1024, 256], F32)
            pool_out = dram.tile([1024, 256], F32)

            ident = cp.tile([128, 128], BF16)
            nc.sync.dma_start(ident[:], t_ident[:])
            iota = cp.tile([128, 128], F32)
            nc.sync.dma_start(iota[:], t_iota[:])
            invcnt = cp.tile([128, NWIN], F32)
            nc.sync.dma_start(invcnt[:], t_invcnt[:])
            dwin = cp.tile([128, E_PAD // 128], F32)
            nc.sync.dma_start(dwin[:], t_dstwin[:])
            padcnt = cp.tile([128, 1], F32)
            nc.sync.dma_start(padcnt[:], t_padcnt[:])
            iotap = cp.tile([128, 1], F32)
            nc.sync.dma_start(iotap[:], t_iotap[:])

            # ---------- helpers ----------
            def allreduce_stats(s_acc, q_acc, n_mb, sb):
                st = sb.tile([128, 8], F32, tag="st_")
                nc.vector.memset(st[:], 0.0)
                nc.vector.tensor_copy(st[:, 0:n_mb], s_acc[:])
                nc.vector.tensor_copy(st[:, 4:4 + n_mb], q_acc[:])
                nc.sync.dma_start(st_in[:], st[:])
                nc.gpsimd.collective_compute(
                    "AllReduce", AOP.add, replica_groups=[list(range(NC))],
                    ins=[st_in.opt()], outs=[st_out.opt()])
                stg = sb.tile([128, 8], F32, tag="stg_")
                nc.sync.dma_start(stg[:], st_out[:])
                return stg

            def affine_from_stats(stg, n_mb, b_lin, gn, sb):
                A, Cc = [], []
                for mb in range(n_mb):
                    s = stg[:, mb:mb + 1]
                    q = stg[:, 4 + mb:5 + mb]
                    g, bgn, ms = gn[0][mb], gn[1][mb], gn[2][mb]
                    bl = b_lin[mb]
                    m = sb.tile([128, 1], F32, tag="af_m")
                    nc.vector.tensor_scalar(m[:], s, 1.0 / N_EDGES, None, AOP.mult)
                    nc.vector.tensor_tensor(m[:], m[:], bl, op=AOP.add)
                    e2 = sb.tile([128, 1], F32, tag="af_e2")
                    nc.vector.tensor_scalar(e2[:], q, 1.0 / N_EDGES, None, AOP.mult)
                    tmp = sb.tile([128, 1], F32, tag="af_t")
                    nc.vector.tensor_tensor(tmp[:], m[:], bl, op=AOP.mult)
                    nc.vector.tensor_scalar(tmp[:], tmp[:], 2.0, None, AOP.mult)
                    nc.vector.tensor_tensor(e2[:], e2[:], tmp[:], op=AOP.add)
                    nc.vector.tensor_tensor(tmp[:], bl, bl, op=AOP.mult)
                    nc.vector.tensor_tensor(e2[:], e2[:], tmp[:], op=AOP.subtract)
                    msm = sb.tile([128, 1], F32, tag="af_msm")
                    nc.vector.tensor_tensor(msm[:], ms, m[:], op=AOP.mult)
                    var = sb.tile([128, 1], F32, tag="af_v")
                    nc.vector.tensor_tensor(var[:], msm[:], msm[:], op=AOP.mult)
                    nc.vector.tensor_tensor(tmp[:], msm[:], m[:], op=AOP.mult)
                    nc.vector.tensor_scalar(tmp[:], tmp[:], 2.0, None, AOP.mult)
                    nc.vector.tensor_tensor(var[:], var[:], tmp[:], op=AOP.subtract)
                    nc.vector.tensor_tensor(var[:], var[:], e2[:], op=AOP.add)
                    a = sb.tile([128, 1], F32, tag="af_a")
                    nc.vector.tensor_scalar(var[:], var[:], EPS, None, AOP.add)
                    nc.scalar.activation(a[:], var[:], AFT.Sqrt)
                    nc.vector.reciprocal(a[:], a[:])
                    nc.vector.tensor_tensor(a[:], a[:], g, op=AOP.mult)
                    cc = sb.tile([128, 1], F32, tag="af_c")
                    nc.vector.tensor_tensor(cc[:], bl, msm[:], op=AOP.subtract)
                    nc.vector.tensor_tensor(cc[:], cc[:], a[:], op=AOP.mult)
                    nc.vector.tensor_tensor(cc[:], cc[:], bgn, op=AOP.add)
                    A.append(a)
                    Cc.append(cc)
                return A, Cc

            def acc_stats(ps_ap, s_col, q_col, sb):
                t1 = sb.tile([128, 1], F32, tag="rs_t1")
                nc.vector.reduce_sum(out=t1[:], in_=ps_ap, axis=AX.X)
                nc.vector.tensor_tensor(s_col, s_col, t1[:], op=AOP.add)
                n = ps_ap.shape[-1]
                sq = sb.tile([128, 512], BF16, tag="rs_sq")
                qa = sb.tile([128, 1], F32, tag="rs_qa")
                nc.scalar.activation(sq[:, :n], ps_ap, AFT.Square, accum_out=qa[:])
                nc.vector.tensor_tensor(q_col, q_col, qa[:], op=AOP.add)

            def bn_finish(st, s_col, q_col, sb, tag):
                # bn_stats 6-tuples (equal 512-col groups) -> sum / sq-sum
                agg = sb.tile([128, 2], F32, tag=tag + "g")
                nc.vector.bn_aggr(agg[:], st[:])
                nc.vector.tensor_scalar(s_col, agg[:, 0:1], float(E_PAD),
                                        None, AOP.mult)
                t = sb.tile([128, 1], F32, tag=tag + "t")
                nc.vector.tensor_tensor(t[:], agg[:, 0:1], agg[:, 0:1], op=AOP.mult)
                nc.vector.tensor_tensor(t[:], t[:], agg[:, 1:2], op=AOP.add)
                nc.vector.tensor_scalar(q_col, t[:], float(E_PAD), None, AOP.mult)

            def sentinel_correct(s_acc, q_acc, zsent_cols, n_mb, sb):
                for mb in range(n_mb):
                    zs = zsent_cols[mb]
                    t1 = sb.tile([128, 1], F32, tag="sc_t1")
                    nc.vector.tensor_tensor(t1[:], zs, padcnt[:], op=AOP.mult)
                    nc.vector.tensor_tensor(s_acc[:, mb:mb + 1], s_acc[:, mb:mb + 1],
                                            t1[:], op=AOP.subtract)
                    nc.vector.tensor_tensor(t1[:], zs, zs, op=AOP.mult)
                    nc.vector.tensor_tensor(t1[:], t1[:], padcnt[:], op=AOP.mult)
                    nc.vector.tensor_tensor(q_acc[:, mb:mb + 1], q_acc[:, mb:mb + 1],
                                            t1[:], op=AOP.subtract)

            def load_vec(t_ap, sb, tag):
                v = sb.tile([128, 1], F32, tag=tag)
                nc.sync.dma_start(v[:], t_ap)
                return v[:]

            AG_BASE = [0]
            for _c in range(3):
                AG_BASE.append(AG_BASE[-1] + NC * (AG_CHB[_c + 1] - AG_CHB[_c]))

            def fire_ag(tab_loc, tab_full, c):
                # chunk-major table: AG chunk c is a contiguous row block
                lo, hi = AG_CHB[c], AG_CHB[c + 1]
                nc.gpsimd.collective_compute(
                    "AllGather", AOP.bypass, replica_groups=[list(range(NC))],
                    ins=[tab_loc[lo:hi, :].opt()],
                    outs=[tab_full[AG_BASE[c]:AG_BASE[c] + NC * (hi - lo),
                                   :].opt()])

            def scatter_pass(zsrc, n_mb, A, Cc, tab_loc, Cout, ag=None):
                with tc.tile_pool(name="sc_sb", bufs=3) as sb, \
                     tc.tile_pool(name="sc_tp", bufs=3, space="PSUM") as ps_tp, \
                     tc.tile_pool(name="sc_sc", bufs=2, space="PSUM") as ps_sc:
                    for b in range(NBLK):
                        if ag is not None and b in AG_FIRE:
                            fire_ag(tab_loc, ag, AG_FIRE.index(b))
                        hs = []
                        for mb in range(n_mb):
                            z = sb.tile([128, BLK], BF16, tag=f"sp_z{mb}")
                            nc.sync.dma_start(z[:], zsrc[mb, :, b * BLK:(b + 1) * BLK])
                            h = sb.tile([128, BLK], BF16, tag=f"sp_h{mb}")
                            nc.scalar.activation(h[:], z[:], AFT.Relu,
                                                 bias=Cc[mb], scale=A[mb])
                            hs.append(h)
                        hE = sb.tile([128, NCHUNK * Cout], BF16, tag="sp_hE")
                        for ch in range(NCHUNK):
                            for mb in range(n_mb):
                                tp = ps_tp.tile([128, 128], BF16, tag="sp_tp", space="PSUM")
                                nc.tensor.transpose(tp[:], hs[mb][:, ch * 128:(ch + 1) * 128],
                                                    ident[:])
                                nc.vector.tensor_copy(
                                    hE[:, ch * Cout + mb * 128:ch * Cout + (mb + 1) * 128],
                                    tp[:])
                        for w in range(NW_BLK):
                            gw = b * NW_BLK + w
                            sc = ps_sc.tile([128, Cout], F32, tag="sp_sc", space="PSUM")
                            for cb in range(B):
                                ch = w * B + cb
                                col = b * NCHUNK + ch
                                oh = sb.tile([128, 128], BF16, tag="sp_oh")
                                nc.vector.tensor_tensor(
                                    out=oh[:],
                                    in0=dwin[:, col:col + 1].to_broadcast([128, 128]),
                                    in1=iota[:], op=AOP.is_equal)
                                nc.tensor.matmul(sc[:], oh[:],
                                                 hE[:, ch * Cout:(ch + 1) * Cout],
                                                 start=(cb == 0), stop=(cb == B - 1))
                            nt = sb.tile([128, Cout], BF16, tag="sp_nt")
                            nc.vector.tensor_scalar(nt[:], sc[:], invcnt[:, gw:gw + 1],
                                                    None, AOP.mult)
                            nc.sync.dma_start(tab_loc[gw * WIN:(gw + 1) * WIN, :], nt[:])
                    if ag is not None:
                        fire_ag(tab_loc, ag, 3)

            # ======================= CONV 1 =======================
            # SBUF-resident: L1 stats precomputed on host; h kept on-chip,
            # L2 overwrites it in place; L3 fused with the scatter.
            NSEG_T = E_PAD // 512  # 98
            with tc.tile_pool(name="c1sb", bufs=2) as sb:
                c1b = [[load_vec(t_c1b[i], sb, f"c1b{i}")] for i in range(3)]
                c1gn = [[[load_vec(t_c1gn[i, j], sb, f"c1gn{i}{j}")] for j in range(3)]
                        for i in range(3)]
                A1h = load_vec(t_c1a[0], sb, "c1a0")
                C1h = load_vec(t_c1a[1], sb, "c1a1")
                with tc.tile_pool(name="c1h", bufs=1) as hp, \
                     tc.tile_pool(name="c1ps", bufs=2, space="PSUM") as ps, \
                     tc.tile_pool(name="c1p2", bufs=2, space="PSUM") as ps2:
                    c1w = []
                    for i in range(3):
                        w = sb.tile([128, 128], BF16, tag=f"c1w{i}")
                        nc.sync.dma_start(w[:], t_c1w[i])
                        c1w.append(w)
                    msgT = hp.tile([48, EHALF], BF16, tag="msgT")
                    nc.sync.dma_start(msgT[:], t_msgT[:])
                    h_full = hp.tile([128, E_PAD], BF16)

                    def bn_to_sq(st, tag):
                        agg = sb.tile([128, 2], F32, tag=tag + "agg")
                        nc.vector.bn_aggr(agg[:], st[:])
                        s_acc = sb.tile([128, 1], F32, tag=tag + "s")
                        q_acc = sb.tile([128, 1], F32, tag=tag + "q")
                        nc.vector.tensor_scalar(s_acc[:], agg[:, 0:1],
                                                float(E_PAD), None, AOP.mult)
                        nc.vector.tensor_tensor(q_acc[:], agg[:, 0:1], agg[:, 0:1],
                                                op=AOP.mult)
                        nc.vector.tensor_tensor(q_acc[:], q_acc[:], agg[:, 1:2],
                                                op=AOP.add)
                        nc.vector.tensor_scalar(q_acc[:], q_acc[:],
                                                float(E_PAD), None, AOP.mult)
                        return s_acc, q_acc

                    # pass 1: L1 -> h_full; L2 stats
                    st2 = hp.tile([128, NSEG_T * 6], F32, tag="st2")
                    zs2 = sb.tile([128, 1], F32, tag="zs2")
                    for g in range(NSEG_T):
                        hh, shalf = g // NSEG_H, g % NSEG_H
                        zp = ps.tile([128, 512], F32, tag="zp")
                        nc.tensor.matmul(zp[:], c1w[0][32 * hh:32 * hh + 10, :],
                                         msgT[32 * hh:32 * hh + 10,
                                              shalf * 512:(shalf + 1) * 512],
                                         start=True, stop=True)
                        nc.scalar.activation(h_full[:, g * 512:(g + 1) * 512], zp[:],
                                             AFT.Relu, bias=C1h, scale=A1h)
                        zp2 = ps2.tile([128, 512], F32, tag="zp2")
                        nc.tensor.matmul(zp2[:], c1w[1][:],
                                         h_full[:, g * 512:(g + 1) * 512],
                                         start=True, stop=True)
                        nc.vector.bn_stats(st2[:, g * 6:(g + 1) * 6], zp2[:])
                        if g == NSEG_T - 1:
                            nc.vector.tensor_copy(zs2[:], zp2[:, 511:512])
                    s2, q2 = bn_to_sq(st2, "b2")
                    sentinel_correct(s2, q2, [zs2[:]], 1, sb)
                    stg2 = allreduce_stats(s2, q2, 1, sb)
                    A2, C2 = affine_from_stats(stg2, 1, c1b[1], c1gn[1], sb)

                    # pass 2: L2 -> h_full (in place); L3 stats
                    st3 = hp.tile([128, NSEG_T * 6], F32, tag="st3")
                    zs3 = sb.tile([128, 1], F32, tag="zs3")
                    for g in range(NSEG_T):
                        zp = ps.tile([128, 512], F32, tag="zp")
                        nc.tensor.matmul(zp[:], c1w[1][:],
                                         h_full[:, g * 512:(g + 1) * 512],
                                         start=True, stop=True)
                        nc.scalar.activation(h_full[:, g * 512:(g + 1) * 512], zp[:],
                                             AFT.Relu, bias=C2[0], scale=A2[0])
                        zp3 = ps2.tile([128, 512], F32, tag="zp2")
                        nc.tensor.matmul(zp3[:], c1w[2][:],
                                         h_full[:, g * 512:(g + 1) * 512],
                                         start=True, stop=True)
                        nc.vector.bn_stats(st3[:, g * 6:(g + 1) * 6], zp3[:])
                        if g == NSEG_T - 1:
                            nc.vector.tensor_copy(zs3[:], zp3[:, 511:512])
                    s3, q3 = bn_to_sq(st3, "b3")
                    sentinel_correct(s3, q3, [zs3[:]], 1, sb)
                    stg3 = allreduce_stats(s3, q3, 1, sb)
                    A3, C3 = affine_from_stats(stg3, 1, c1b[2], c1gn[2], sb)

                    # pass 3: L3 + fused scatter
                    with tc.tile_pool(name="c1sc", bufs=2) as scb, \
                         tc.tile_pool(name="c1tp", bufs=2, space="PSUM") as ps_tp, \
                         tc.tile_pool(name="c1s2", bufs=2, space="PSUM") as ps_sc:
                        for b in range(NBLK):
                            if b in AG_FIRE:
                                fire_ag(tab1_loc, tab1, AG_FIRE.index(b))
                            h3 = scb.tile([128, BLK], BF16, tag="c1h3")
                            for s in range(NSEG):
                                g = b * NSEG + s
                                zp = ps.tile([128, 512], F32, tag="zp")
                                nc.tensor.matmul(zp[:], c1w[2][:],
                                                 h_full[:, g * 512:(g + 1) * 512],
                                                 start=True, stop=True)
                                nc.scalar.activation(h3[:, s * 512:(s + 1) * 512],
                                                     zp[:], AFT.Relu,
                                                     bias=C3[0], scale=A3[0])
                            hE = scb.tile([128, NCHUNK * 128], BF16, tag="c1hE")
                            for ch in range(NCHUNK):
                                tp = ps_tp.tile([128, 128], BF16, tag="c1tp",
                                                space="PSUM")
                                nc.tensor.transpose(tp[:], h3[:, ch * 128:(ch + 1) * 128],
                                                    ident[:])
                                nc.vector.tensor_copy(hE[:, ch * 128:(ch + 1) * 128],
                                                      tp[:])
                            for w in range(NW_BLK):
                                gw = b * NW_BLK + w
                                sc = ps_sc.tile([128, 128], F32, tag="c1sc",
                                                space="PSUM")
                                for cb in range(B):
                                    ch = w * B + cb
                                    col = b * NCHUNK + ch
                                    oh = scb.tile([128, 128], BF16, tag="c1oh")
                                    nc.vector.tensor_tensor(
                                        out=oh[:],
                                        in0=dwin[:, col:col + 1].to_broadcast([128, 128]),
                                        in1=iota[:], op=AOP.is_equal)
                                    nc.tensor.matmul(sc[:], oh[:],
                                                     hE[:, ch * 128:(ch + 1) * 128],
                                                     start=(cb == 0), stop=(cb == B - 1))
                                nt = scb.tile([128, 128], BF16, tag="c1nt")
                                nc.vector.tensor_scalar(nt[:], sc[:],
                                                        invcnt[:, gw:gw + 1],
                                                        None, AOP.mult)
                                nc.sync.dma_start(tab1_loc[gw * WIN:(gw + 1) * WIN, :],
                                                  nt[:])
                        fire_ag(tab1_loc, tab1, 3)

            if debug:
                nc.sync.dma_start(dbg["x1"][:], tab1[:])

            # ============== gather-based first layer (conv2/conv3) ==============
            def gather_layer(tab_full, tab_loc, Cin, wa_t, wb_t, n_kb, zdst, sb):
                mb_in = Cin // 128
                s_acc = sb.tile([128, 2], F32, tag="gl_s")
                q_acc = sb.tile([128, 2], F32, tag="gl_q")
                sts = [sb.tile([128, (E_PAD // 512) * 6], F32, tag=f"gl_st{mo}",
                               name=f"gl_st{mo}")
                       for mo in range(2)]
                with tc.tile_pool(name="gl_g2", bufs=3) as g2, \
                     tc.tile_pool(name="gl_g1", bufs=2) as g1, \
                     tc.tile_pool(name="gl_zw", bufs=2) as zwp, \
                     tc.tile_pool(name="gl_ps", bufs=2, space="PSUM") as ps, \
                     tc.tile_pool(name="gl_tp", bufs=2, space="PSUM") as ps_tp, \
                     tc.tile_pool(name="gl_xp", bufs=2, space="PSUM") as ps_xp:
                    was, wbs = [], []
                    for ki in range(n_kb):
                        for mo in range(2):
                            wta = sb.tile([128, 128], BF16, tag=f"gl_wa{ki}{mo}")
                            nc.sync.dma_start(wta[:], wa_t[ki, mo] if n_kb > 1 else wa_t[mo])
                            was.append(wta)
                            wtb = sb.tile([128, 128], BF16, tag=f"gl_wb{ki}{mo}")
                            nc.sync.dma_start(wtb[:], wb_t[ki, mo] if n_kb > 1 else wb_t[mo])
                            wbs.append(wtb)
                    ixj = sb.tile([128, NBLK * NCHUNK], mybir.dt.int32,
                                  tag="gl_ixj")
                    nc.sync.dma_start(ixj[:], t_xj[:])
                    for b in range(NBLK):
                        gxj = g2.tile([128, NCHUNK * Cin], BF16, tag="gl_gxj")
                        for ch in range(NCHUNK):
                            gch = b * NCHUNK + ch
                            nc.gpsimd.indirect_dma_start(
                                out=gxj[:, ch * Cin:(ch + 1) * Cin],
                                out_offset=None,
                                in_=tab_full[:],
                                in_offset=bass.IndirectOffsetOnAxis(
                                    ap=ixj[:, gch:gch + 1], axis=0))
                        xjT = g1.tile([128, mb_in * BLK], BF16, tag="gl_xjT")
                        for ch in range(NCHUNK):
                            for kb in range(mb_in):
                                tp2 = ps_tp.tile([128, 128], BF16, tag="gl_ohp",
                                                 space="PSUM")
                                nc.tensor.transpose(
                                    tp2[:],
                                    gxj[:, ch * Cin + kb * 128:ch * Cin + (kb + 1) * 128],
                                    ident[:])
                                dst = xjT[:, kb * BLK + ch * 128:
                                          kb * BLK + (ch + 1) * 128]
                                if kb == 0 and (mb_in == 2 or ch % 2 == 0):
                                    nc.scalar.copy(dst, tp2[:])
                                else:
                                    nc.vector.tensor_copy(dst, tp2[:])
                        # xi via window expansion (transposed one-hot built
                        # directly from the replicated dstwin row)
                        dwb = g1.tile([128, BLK], BF16, tag="gl_dwb")
                        nc.sync.dma_start(dwb[:], t_dwinR[:, b * BLK:(b + 1) * BLK])
                        xiT = g1.tile([128, mb_in * BLK], BF16, tag="gl_xiT")
                        for w in range(NW_BLK):
                            gw = b * NW_BLK + w
                            twin = g2.tile([128, Cin], BF16, tag="gl_twin")
                            nc.sync.dma_start(twin[:], tab_loc[gw * WIN:(gw + 1) * WIN, :])
                            for cb in range(B):
                                ch = w * B + cb
                                oh2 = g2.tile([128, 128], BF16, tag="gl_oh2")
                                nc.vector.tensor_scalar(
                                    oh2[:], dwb[:, ch * 128:(ch + 1) * 128],
                                    iotap[:, 0:1], None, AOP.is_equal)
                                for kb in range(mb_in):
                                    xp = ps_xp.tile([128, 128], F32, tag="gl_xp", space="PSUM")
                                    nc.tensor.matmul(xp[:], twin[:, kb * 128:(kb + 1) * 128],
                                                     oh2[:], start=True, stop=True)
                                    nc.vector.tensor_copy(
                                        xiT[:, kb * BLK + ch * 128:kb * BLK + (ch + 1) * 128],
                                        xp[:])
                        for mo in range(2):
                            zw = zwp.tile([128, BLK], BF16, tag=f"gl_z{mo}")
                            for sg in range(NSEG):
                                g6 = (b * NSEG + sg) * 6
                                zp = ps.tile([128, 512], F32, tag="gl_zp")
                                for ki in range(mb_in):
                                    nc.tensor.matmul(
                                        zp[:], was[ki * 2 + mo][:],
                                        xiT[:, ki * BLK + sg * 512:ki * BLK + (sg + 1) * 512],
                                        start=(ki == 0), stop=False)
                                for ki in range(mb_in):
                                    nc.tensor.matmul(
                                        zp[:], wbs[ki * 2 + mo][:],
                                        xjT[:, ki * BLK + sg * 512:ki * BLK + (sg + 1) * 512],
                                        start=False, stop=(ki == mb_in - 1))
                                nc.vector.bn_stats(sts[mo][:, g6:g6 + 6], zp[:])
                                nc.scalar.copy(zw[:, sg * 512:(sg + 1) * 512], zp[:])
                            nc.sync.dma_start(zdst[mo, :, b * BLK:(b + 1) * BLK], zw[:])
                for mo in range(2):
                    bn_finish(sts[mo], s_acc[:, mo:mo + 1], q_acc[:, mo:mo + 1],
                              sb, f"glf{mo}")
                return s_acc, q_acc

            # ======================= CONV 2 =======================
            if phases >= 2:
              with tc.tile_pool(name="c2sb", bufs=2) as sb:
                  c2b = [[load_vec(t_c2b[i, mb], sb, f"c2b{i}{mb}") for mb in range(2)]
                         for i in range(2)]
                  c2gn = [[[load_vec(t_c2gn[i, j, mb], sb, f"c2gn{i}{j}{mb}")
                            for mb in range(2)] for j in range(3)] for i in range(2)]
                  sA, qA = gather_layer(tab1, tab1_loc, 128, t_c2wa, t_c2wb, 1,
                                        z_scr[0], sb)
                  stg = allreduce_stats(sA, qA, 2, sb)
                  A1, C1 = affine_from_stats(stg, 2, c2b[0], c2gn[0], sb)

                  s2 = sb.tile([128, 2], F32, tag="c2s2")
                  q2 = sb.tile([128, 2], F32, tag="c2q2")
                  st2s = [sb.tile([128, (E_PAD // 512) * 6], F32, tag=f"c2st{mo}",
                                  name=f"c2st{mo}")
                          for mo in range(2)]
                  zlast = [None, None]
                  with tc.tile_pool(name="c2mid", bufs=2) as mp, \
                       tc.tile_pool(name="c2ps", bufs=2, space="PSUM") as ps:
                      w2s = []
                      for ki in range(2):
                          for mo in range(2):
                              w = sb.tile([128, 128], BF16, tag=f"c2w2{ki}{mo}")
                              nc.sync.dma_start(w[:], t_c2w2[ki, mo])
                              w2s.append(w)
                      for b in range(NBLK):
                          h1 = []
                          for mb in range(2):
                              z = mp.tile([128, BLK], BF16, tag=f"c2z1r{mb}")
                              nc.sync.dma_start(z[:], z_scr[0][mb, :, b * BLK:(b + 1) * BLK])
                              hh = mp.tile([128, BLK], BF16, tag=f"c2h1{mb}")
                              nc.scalar.activation(hh[:], z[:], AFT.Relu,
                                                   bias=C1[mb], scale=A1[mb])
                              h1.append(hh)
                          for mo in range(2):
                              zw = mp.tile([128, BLK], BF16, tag=f"c2z2w{mo}")
                              for s in range(NSEG):
                                  g6 = (b * NSEG + s) * 6
                                  zp = ps.tile([128, 512], F32, tag="c2zp")
                                  for ki in range(2):
                                      nc.tensor.matmul(zp[:], w2s[ki * 2 + mo][:],
                                                       h1[ki][:, s * 512:(s + 1) * 512],
                                                       start=(ki == 0), stop=(ki == 1))
                                  nc.vector.bn_stats(st2s[mo][:, g6:g6 + 6], zp[:])
                                  if s % 2 == 0:
                                      nc.scalar.copy(zw[:, s * 512:(s + 1) * 512],
                                                     zp[:])
                                  else:
                                      nc.vector.tensor_copy(
                                          zw[:, s * 512:(s + 1) * 512], zp[:])
                              nc.sync.dma_start(z_scr[1][mo, :, b * BLK:(b + 1) * BLK], zw[:])
                              zlast[mo] = zw
                      zsent = []
                      for mo in range(2):
                          zc = sb.tile([128, 1], F32, tag=f"c2zs{mo}")
                          nc.vector.tensor_copy(zc[:], zlast[mo][:, BLK - 1:BLK])
                          zsent.append(zc[:])
                  for mo in range(2):
                      bn_finish(st2s[mo], s2[:, mo:mo + 1], q2[:, mo:mo + 1],
                                sb, f"c2f{mo}")
                  sentinel_correct(s2, q2, zsent, 2, sb)
                  stg2 = allreduce_stats(s2, q2, 2, sb)
                  A2, C2 = affine_from_stats(stg2, 2, c2b[1], c2gn[1], sb)
                  scatter_pass(z_scr[1], 2, A2, C2, tab2_loc, 256, ag=tab2)

            if debug:
                nc.sync.dma_start(dbg["x2"][:], tab2[:])

            # ======================= CONV 3 =======================
            if phases >= 3:
              with tc.tile_pool(name="c3sb", bufs=2) as sb:
                  c3b = [load_vec(t_c3b[mb], sb, f"c3b{mb}") for mb in range(2)]
                  c3gn = [[load_vec(t_c3gn[j, mb], sb, f"c3gn{j}{mb}") for mb in range(2)]
                          for j in range(3)]
                  sA, qA = gather_layer(tab2, tab2_loc, 256, t_c3wa, t_c3wb, 2,
                                        z_scr[0], sb)
                  stg = allreduce_stats(sA, qA, 2, sb)
                  A1, C1 = affine_from_stats(stg, 2, c3b, c3gn, sb)
                  scatter_pass(z_scr[0], 2, A1, C1, tab3_loc, 256)

            if debug:
                nc.sync.dma_start(dbg["x3"][:], tab3_loc[:])

            # ======================= POOL + HEAD =======================
            if phases >= 4:
              with tc.tile_pool(name="p_sb", bufs=2) as sb, \
                 tc.tile_pool(name="p_ps", bufs=2, space="PSUM") as ps:
                  pgwl = sb.tile([128, 8 * Bg], F32, tag="p_pgwl")
                  nc.sync.dma_start(pgwl[:], t_pgwl[:])
                  NPG = Bg * 128
                  pidxw = sb.tile([128, 8 * NPG // 16], I16, tag="p_idx16")
                  nc.sync.dma_start(pidxw[:], t_pidx16[:])
                  for gw in range(8):
                      gp = sb.tile([128, Bg, 256], BF16, tag="p_gp")
                      nc.gpsimd.dma_gather(
                          out_ap=gp[:], in_ap=tab3_loc[:],
                          idxs_ap=pidxw[:, gw * (NPG // 16):(gw + 1) * (NPG // 16)],
                          num_idxs=NPG, num_idxs_reg=NPG, elem_size=256,
                          transpose=False, single_packet=(NPG <= 896))
                      pp = ps.tile([128, 256], F32, tag="p_pp", space="PSUM")
                      for c in range(Bg):
                          oh = sb.tile([128, 128], BF16, tag="p_oh")
                          nc.vector.tensor_tensor(
                              out=oh[:],
                              in0=pgwl[:, gw * Bg + c:gw * Bg + c + 1].to_broadcast([128, 128]),
                              in1=iota[:], op=AOP.is_equal)
                          nc.tensor.matmul(pp[:], oh[:], gp[:, c, :],
                                           start=(c == 0), stop=(c == Bg - 1))
                      pf = sb.tile([128, 256], F32, tag="p_pf")
                      nc.vector.tensor_copy(pf[:], pp[:])
                      nc.sync.dma_start(pool_in[gw * 128:(gw + 1) * 128, :], pf[:])
                  nc.gpsimd.collective_compute(
                      "AllReduce", AOP.add, replica_groups=[list(range(NC))],
                      ins=[pool_in.opt()], outs=[pool_out.opt()])
                  if debug:
                      nc.sync.dma_start(dbg["pool"][:], pool_out[:])

                  invg = sb.tile([128, 8], F32, tag="p_invg")
                  nc.sync.dma_start(invg[:], t_invg[:])
                  lw1 = []
                  for ki in range(2):
                      for mo in range(2):
                          w = sb.tile([128, 128], BF16, tag=f"p_lw1{ki}{mo}")
                          nc.sync.dma_start(w[:], t_lw1[ki, mo])
                          lw1.append(w)
                  lw2 = []
                  for ki in range(2):
                      w = sb.tile([128, 2], BF16, tag=f"p_lw2{ki}")
                      nc.sync.dma_start(w[:], t_lw2[ki])
                      lw2.append(w)
                  lb1 = [load_vec(t_lb1[mb], sb, f"p_lb1{mb}") for mb in range(2)]
                  lb2 = sb.tile([2, 1], F32, tag="p_lb2")
                  nc.sync.dma_start(lb2[:], t_lb2[:])
                  ofin = sb.tile([2, 1024], F32, tag="p_out")
                  for gw in range(8):
                      g = sb.tile([128, 256], F32, tag="p_g")
                      nc.sync.dma_start(g[:], pool_out[gw * 128:(gw + 1) * 128, :])
                      gm = sb.tile([128, 256], BF16, tag="p_gm")
                      nc.vector.tensor_scalar(gm[:], g[:], invg[:, gw:gw + 1], None, AOP.mult)
                      gT = sb.tile([128, 2 * 128], BF16, tag="p_gT")
                      for kb in range(2):
                          tp = ps.tile([128, 128], BF16, tag="p_tp", space="PSUM")
                          nc.tensor.transpose(tp[:], gm[:, kb * 128:(kb + 1) * 128], ident[:])
                          nc.vector.tensor_copy(gT[:, kb * 128:(kb + 1) * 128], tp[:])
                      hT = sb.tile([128, 2 * 128], BF16, tag="p_hT")
                      for mo in range(2):
                          hp = ps.tile([128, 128], F32, tag="p_hp", space="PSUM")
                          for ki in range(2):
                              nc.tensor.matmul(hp[:], lw1[ki * 2 + mo][:],
                                               gT[:, ki * 128:(ki + 1) * 128],
                                               start=(ki == 0), stop=(ki == 1))
                          nc.scalar.activation(hT[:, mo * 128:(mo + 1) * 128], hp[:],
                                               AFT.Relu, bias=lb1[mo])
                      op_ = ps.tile([2, 128], F32, tag="p_op", space="PSUM")
                      for ki in range(2):
                          nc.tensor.matmul(op_[:], lw2[ki][:],
                                           hT[:, ki * 128:(ki + 1) * 128],
                                           start=(ki == 0), stop=(ki == 1))
                      nc.vector.tensor_scalar(ofin[:, gw * 128:(gw + 1) * 128],
                                              op_[:], lb2[:], None, AOP.add)
                  nc.sync.dma_start(o_out[:], ofin[:, :N_GRAPHS])

    nc.compile()
    return nc


# ============================ entry point ============================


def kernel(**inputs):
    x = np.asarray(inputs["x"], dtype=np.float32)
    edge_index = np.asarray(inputs["edge_index"])
    batch = np.asarray(inputs["batch"])

    meta = _pack(edge_index, batch)
    Bg = meta["Bg"]

    import os as _os
    phases = int(_os.environ.get("KPHASES", "4"))
    key = ("mod", Bg, phases, _DEBUG[0])
    if key not in _cache:
        _cache[key] = _build(Bg, debug=bool(inputs.get("_debug", False)) or _DEBUG[0],
                             phases=phases)
    nc = _cache[key]

    # ---- per-core input arrays ----
    slot_of_node = meta["slot_of_node"]
    src = np.asarray(edge_index[0], dtype=np.int64)
    dst = np.asarray(edge_index[1], dtype=np.int64)

    # conv1 msgT: [core, 20, E_PAD//2] bf16; edge e<EHALF -> rows 0..9 col e,
    # e>=EHALF -> rows 10..19 col e-EHALF
    EHALF = E_PAD // 2
    xi_v = x[dst]
    xj_v = x[src]
    msg = np.concatenate([xi_v, xj_v - xi_v], axis=1)       # [E, 10]

    # exact conv1-L1 GraphNorm stats on host (tiny 10-dim Gram)
    msg64 = msg.astype(np.float64)
    W1 = np.asarray(inputs["c1_w1"], np.float64)            # [10, 128]
    b1 = np.asarray(inputs["c1_b1"], np.float64)            # [128]
    S = msg64.sum(0)
    G = msg64.T @ msg64
    SW = S @ W1
    qz = np.einsum('ij,ik,kj->j', W1, G, W1) + 2 * b1 * SW + N_EDGES * b1 * b1
    m1 = (SW + N_EDGES * b1) / N_EDGES
    e2 = qz / N_EDGES
    gn1 = np.asarray(inputs["c1_gn1"], np.float64)          # [3, 128]
    msm = gn1[2] * m1
    var1 = e2 - 2 * msm * m1 + msm * msm
    A1h = gn1[0] / np.sqrt(var1 + EPS)
    C1h = gn1[1] + A1h * (b1 - msm)
    c1a_in = np.stack([A1h, C1h]).astype(np.float32).reshape(2, 128, 1)

    msg_full = np.zeros((NC, E_PAD, 10), dtype=np.float32)
    ec, pos = meta["ec"], meta["pos"]
    msg_full[ec, pos] = msg[meta["eorder"]]
    msgT = np.zeros((NC, 48, EHALF), dtype=ml_dtypes.bfloat16)
    msgT[:, :10, :] = _bf(msg_full[:, :EHALF].transpose(0, 2, 1))
    msgT[:, 32:42, :] = _bf(msg_full[:, EHALF:].transpose(0, 2, 1))

    dstwin = meta["dstwin"]  # [NC, E_PAD]
    dwin_in = np.ascontiguousarray(
        dstwin.reshape(NC, E_PAD // 128, 128).transpose(0, 2, 1)).astype(np.float32)
    invcnt_in = np.ascontiguousarray(
        meta["inv_cnt"].reshape(NC, NWIN, 128).transpose(0, 2, 1)).astype(np.float32)
    padcnt_in = np.repeat(meta["pad_cnt"][:, None], 128, axis=1)[:, :, None].astype(np.float32)

    iota_in = np.broadcast_to(np.arange(128, dtype=np.float32)[None, :], (128, 128))
    iota_in = np.ascontiguousarray(iota_in)
    ident_in = np.eye(128, dtype=np.float32).astype(ml_dtypes.bfloat16)
    iotap_in = np.arange(128, dtype=np.float32).reshape(128, 1)
    dwinR_in = np.ascontiguousarray(np.broadcast_to(
        dstwin[:, None, :], (NC, 128, E_PAD))).astype(ml_dtypes.bfloat16)

    xj_row = meta["row_of_slot"][meta["xj_glob"]]  # [NC, E_PAD] chunk-major rows
    xj_in = np.ascontiguousarray(
        xj_row.reshape(NC, E_PAD // 128, 128).transpose(0, 2, 1)).astype(np.int32)

    # weights
    c1w = np.zeros((3, 128, 128), dtype=ml_dtypes.bfloat16)
    c1w[0, :10, :] = _bf(inputs["c1_w1"])
    c1w[0, 32:42, :] = _bf(inputs["c1_w1"])
    c1w[1] = _bf(inputs["c1_w2"])
    c1w[2] = _bf(inputs["c1_w3"])
    c1b = np.stack([np.asarray(inputs[f"c1_b{i}"], dtype=np.float32).reshape(128, 1)
                    for i in (1, 2, 3)])
    c1gn = np.stack([np.asarray(inputs[f"c1_gn{i}"], dtype=np.float32).reshape(3, 128, 1)
                     for i in (1, 2, 3)])

    w2a = np.asarray(inputs["c2_w1"], dtype=np.float32)   # [256, 256]
    WA2 = w2a[:128] - w2a[128:]
    WB2 = w2a[128:]
    c2wa = _tile_w(WA2)[0]                                # [2, 128, 128]
    c2wb = _tile_w(WB2)[0]
    c2w2 = _tile_w(np.asarray(inputs["c2_w2"], dtype=np.float32))  # [2,2,128,128]
    c2b = np.stack([np.asarray(inputs["c2_b1"], dtype=np.float32).reshape(2, 128, 1),
                    np.asarray(inputs["c2_b2"], dtype=np.float32).reshape(2, 128, 1)])
    c2gn = np.stack([np.asarray(inputs["c2_gn1"], dtype=np.float32).reshape(3, 2, 128, 1),
                     np.asarray(inputs["c2_gn2"], dtype=np.float32).reshape(3, 2, 128, 1)])

    w3a = np.asarray(inputs["c3_w1"], dtype=np.float32)   # [512, 256]
    WA3 = w3a[:256] - w3a[256:]
    WB3 = w3a[256:]
    c3wa = _tile_w(WA3)                                   # [2,2,128,128]
    c3wb = _tile_w(WB3)
    c3b = np.asarray(inputs["c3_b1"], dtype=np.float32).reshape(2, 128, 1)
    c3gn = np.asarray(inputs["c3_gn1"], dtype=np.float32).reshape(3, 2, 128, 1)

    lw1 = _tile_w(np.asarray(inputs["lin_w1"], dtype=np.float32))
    lb1 = np.asarray(inputs["lin_b1"], dtype=np.float32).reshape(2, 128, 1)
    lw2_f = np.asarray(inputs["lin_w2"], dtype=np.float32)  # [256, 2]
    lw2 = np.stack([_bf(lw2_f[:128]), _bf(lw2_f[128:])])    # [2, 128, 2]
    lb2 = np.asarray(inputs["lin_b2"], dtype=np.float32).reshape(2, 1)

    Bg0 = meta["Bg"]
    pidx16_in = _wrap_idx(meta["pool_idx"].reshape(NC, 8 * Bg0 * 128))
    pidx16_in = pidx16_in.reshape(NC, 128, -1)
    pgwl = meta["pool_gwl"]                # [NC, 8, NPG]
    Bg_ = meta["Bg"]
    pgwl_in = np.ascontiguousarray(
        pgwl.reshape(NC, 8, Bg_, 128).transpose(0, 3, 1, 2)).reshape(NC, 128, 8 * Bg_)
    invg_in = np.broadcast_to(
        meta["inv_g"].reshape(8, 128).T[None], (NC, 128, 8)).astype(np.float32)
    invg_in = np.ascontiguousarray(invg_in)

    in_maps = []
    for c in range(NC):
        im = {
            "msgT": msgT[c],
            "xj_idx": xj_in[c],
            "dstwin": dwin_in[c],
            "invcnt": invcnt_in[c],
            "padcnt": padcnt_in[c],
            "iota": iota_in,
            "ident": ident_in,
            "iotap": iotap_in,
            "dwinR": dwinR_in[c],
            "c1w": c1w, "c1a": c1a_in, "c1b": c1b, "c1gn": c1gn,
            "c2wa": c2wa, "c2wb": c2wb, "c2w2": c2w2, "c2b": c2b, "c2gn": c2gn,
            "c3wa": c3wa, "c3wb": c3wb, "c3b": c3b, "c3gn": c3gn,
            "lw1": lw1, "lb1": lb1, "lw2": lw2, "lb2": lb2,
            "pidx16": pidx16_in[c],
            "pool_gwl": pgwl_in[c].astype(np.float32),
            "invg": invg_in[c],
        }
        in_maps.append(im)

    res = run_bass_kernel_spmd(nc, in_maps, core_ids=list(range(NC)),
                               trace=_TRACE[0])
    kernel.last_result = res
    kernel.last_meta = meta
    out = res.results[0]["out"]            # [2, 1000]
    return np.ascontiguousarray(out.T).astype(np.float32)


_DEBUG = [False]
_TRACE = [False]

